# revision 10
# baseline (speedup 1.0000x reference)
"""Trainium2 Bass kernel for nn_DiffuRNNLayer (B=8, N=2048, D=1024).

Sharding: data-parallel over batch — one batch element per NeuronCore (8 cores).
v3: mixed fp8-e4m3 DoubleRow / bf16 matmuls, chosen per-path by error budget:
  fp8 DR: wq/wk/wv (phase A), KV (B), f2+f1h (D), numerator+norm (D),
          LN-mean matmuls for tokenmixer-LN/LN1/LN2.
  bf16:   lu_w1/lu_w2 (local MLP, residual-critical), ff_w1, conv diag taps.
Scale bookkeeping: wq,wk x32; wv,f2 x64; qp_all=32*Qp, kp_all=32*Kp, v_all=V,
kv_sb=KV/2, ksum_sb=Ksum/16, rep=2^16/norm, numerator PSUM=2^20*attn.
Kp/V/Qp are SBUF-resident fp8 (no HBM spill); KV+K_sum run back-to-back from
SBUF.  DVE ops are merged across channel chunks wherever gains are 1
(broadcast [P,1,*] operands), and PSUM consumers are paired into [P,2,NT]
tiles to halve instruction count.
"""

import math
import numpy as np
import ml_dtypes
from contextlib import ExitStack

import concourse.bass as bass
import concourse.bacc as bacc
import concourse.tile as tile
import concourse.mybir as mybir
from concourse.bass_utils import run_bass_kernel_spmd

F32 = mybir.dt.float32
BF16 = mybir.dt.bfloat16
FP8 = mybir.dt.float8e4
AF = mybir.ActivationFunctionType
OP = mybir.AluOpType
DR = mybir.MatmulPerfMode.DoubleRow
BF16_NP = ml_dtypes.bfloat16
FP8_NP = ml_dtypes.float8_e4m3

P = 128
D = 1024
DO = D // P  # 8 chunks of the channel dim

SQK = 32.0       # weight scale for wq/wk (keeps 32*Qp < 240 in fp8)
SW = 64.0        # weight scale for wv/f2
LN_SQK = math.log(SQK)
KS_SC = 2.0 ** -9    # ksum fp8 = (sum of 32*Kp) * 2^-9 = Ksum/16
KV_SC = 1.0 / SW     # kv fp8 = (sum 32Kp*V)/64 = KV/2
REP_SC = 2.0 ** 17   # rep = rr * 2^17 = 2^16/norm  (rr = 1/(2*norm))
NUM_SC = 2.0 ** -20  # ps_u = 2^20 * attn  ->  acc += ps_u * 2^-20

# pp param-plane indices (per-partition params, laid out [128, DO, NP])
(C0, C1, C2, CB, T0, T1, T2, TCB1, U0, U1, U2,
 TMG, TMB, N1G, N1B, N2G, N2B, LUB1, FFB1, FFB2) = range(20)
NPARAM = 20



def build_nc(N=2048, NT=256, use_bq=False, use_bk=False, use_bv=False,
             use_tmb=False, use_n1b=False, use_n2b=False, use_fb2=False,
             use_tmg=False, use_n1g=False, use_n2g=False, debug=False):
    NTILES = N // NT
    NTA = 512              # phase-A tile size (independent of C/D tiling)
    NTILES_A = N // NTA
    NCH = NTA // P         # 128-token chunks per phase-A tile
    TOTCH = N // P
    W = NT + 4             # phase-C tile width with +-2 halo
    WST = (W + 15) // 16 * 16  # fp8 stats tile width (16B-aligned)
    assert N % NT == 0 and NT % P == 0 and N % NTA == 0

    nc = bacc.Bacc(None, target_bir_lowering=False, debug=debug)

    xT_d = nc.dram_tensor("x_T", [D, N], BF16, kind="ExternalInput")
    xqT_d = nc.dram_tensor("xq_T", [D, N], FP8, kind="ExternalInput")
    w_d = {}
    for name, dt_ in (("wqT", FP8), ("wkT", FP8), ("wvT", FP8),
                      ("w1T", BF16), ("w2T", BF16), ("f1T", BF16),
                      ("f2T", FP8)):
        w_d[name] = nc.dram_tensor(name, [D, D], dt_, kind="ExternalInput")
    pp_d = nc.dram_tensor("pp", [P, DO, NPARAM], F32, kind="ExternalInput")
    diags_d = nc.dram_tensor("diags", [P, 3, DO, P], BF16, kind="ExternalInput")
    rows_d = nc.dram_tensor("rows", [1, 3 * D], FP8, kind="ExternalInput")
    yT_d = nc.dram_tensor("y_T", [D, N], F32, kind="ExternalOutput")

    acc_sp = nc.dram_tensor("acc_sp", [D, N], F32)

    xT = xT_d.rearrange("(o p) n -> p o n", p=P)
    xqT = xqT_d.rearrange("(o p) n -> p o n", p=P)
    wr = {k: v.rearrange("(o p) n -> p o n", p=P) for k, v in w_d.items()}
    acc_r = acc_sp.rearrange("(o p) n -> p o n", p=P)
    yT = yT_d.rearrange("(o p) n -> p o n", p=P)

    with tile.TileContext(nc) as tc, ExitStack() as top:
        persist = top.enter_context(tc.tile_pool(name="persist", bufs=1))
        pp = persist.tile([P, DO, NPARAM], F32)
        nc.sync.dma_start(pp, pp_d[:])
        rows = ones_row = None
        if use_bq or use_bk or use_bv:
            rows = persist.tile([1, 3 * D], FP8)
            nc.sync.dma_start(rows, rows_d[:])
            ones_row = persist.tile([1, max(NT, 512)], FP8)
            nc.vector.memset(ones_row, 1.0)
        ones_1p_bf = persist.tile([1, P], BF16)
        nc.vector.memset(ones_1p_bf, 1.0)
        ones_pair = persist.tile([P, 2, 16], FP8)
        nc.vector.memset(ones_pair, 1.0)
        ones_q = persist.tile([P, 2, P], FP8)
        nc.vector.memset(ones_q, 1.0)
        ones_one = persist.tile([1, 1], BF16)
        nc.vector.memset(ones_one, 1.0)
        ksrow_sb = persist.tile([1, D], BF16)
        onesD_bf = persist.tile([P, P], BF16)
        nc.vector.memset(onesD_bf, 1.0 / D)
        eps_ln = persist.tile([P, 1], F32)
        nc.vector.memset(eps_ln, 1e-5)
        ln32_b = persist.tile([P, 1], F32)
        nc.vector.memset(ln32_b, LN_SQK)
        kv_sb = persist.tile([P, DO, D], FP8)
        ksum_sb = persist.tile([P, DO, 16], FP8)
        qp_all = persist.tile([P, DO, N], FP8)
        diags = persist.tile([P, 3, DO, P], BF16)
        nc.sync.dma_start(diags, diags_d[:])

        def stats_q(psum, rhs3, width):
            """Sum over channels (lhs=1.0 fp8 DoubleRow); caller scales by 1/D."""
            for c0 in range(0, width, 512):
                cw = min(512, width - c0)
                for j in range(DO // 2):
                    nc.tensor.matmul(psum[:, c0:c0 + cw], ones_q,
                                     rhs3[:, 2 * j:2 * j + 2, c0:c0 + cw],
                                     start=(j == 0), stop=(j == DO // 2 - 1),
                                     perf_mode=DR)

        # ---------------- Phases A+B scope ----------------
        with ExitStack() as phab:
            kvpool = phab.enter_context(tc.tile_pool(name="kvres", bufs=1))
            kp_all = kvpool.tile([P, TOTCH, D], FP8, tag="kp")
            v_all = kvpool.tile([P, TOTCH, D], FP8, tag="v")

            # ---------------- Phase A: QKV ----------------
            with ExitStack() as ph:
                wpool = ph.enter_context(tc.tile_pool(name="wA", bufs=1))
                wq_sb = wpool.tile([P, DO, D], FP8, tag="wq")
                nc.sync.dma_start(wq_sb, wr["wqT"])
                wk_sb = wpool.tile([P, DO, D], FP8, tag="wk")
                nc.sync.dma_start(wk_sb, wr["wkT"])
                wv_sb = wpool.tile([P, DO, D], FP8, tag="wv")
                nc.sync.dma_start(wv_sb, wr["wvT"])
                io = ph.enter_context(tc.tile_pool(name="ioA", bufs=2))
                ev = ph.enter_context(tc.tile_pool(name="evA", bufs=3))
                ps = ph.enter_context(tc.tile_pool(name="psA", bufs=3, space="PSUM"))

                for it in range(NTILES_A):
                    n0 = it * NTA
                    x_t = io.tile([P, DO, NTA], FP8, tag="xA")
                    nc.sync.dma_start(x_t, xqT[:, :, n0:n0 + NTA])

                    # ---- Q: layout B, out [dout-pair, n]; psum [P, 2*NT] flat
                    for dc0 in range(0, DO, 2):
                        ps_q = ps.tile([P, 2 * NTA], F32, tag="psA")
                        for h in range(2):
                            dc = dc0 + h
                            for j in range(DO // 2):
                                nc.tensor.matmul(
                                    ps_q[:, h * NTA:(h + 1) * NTA],
                                    wq_sb[:, 2 * j:2 * j + 2, dc * P:(dc + 1) * P],
                                    x_t[:, 2 * j:2 * j + 2, :],
                                    start=(j == 0),
                                    stop=(j == DO // 2 - 1 and not use_bq),
                                    perf_mode=DR)
                            if use_bq:
                                nc.tensor.matmul(ps_q[:, h * NTA:(h + 1) * NTA],
                                                 rows[0:1, dc * P:(dc + 1) * P],
                                                 ones_row[0:1, 0:NTA],
                                                 start=False, stop=True)
                        m_t = ev.tile([P, 2 * NTA], BF16, tag="mA")
                        nc.vector.tensor_scalar_min(m_t, ps_q, 0.0)
                        e_t = ev.tile([P, 2 * NTA], BF16, tag="eA")
                        # 32*exp(min(q,0)) = exp(min(32q,0)/32 + ln32)
                        nc.scalar.activation(e_t, m_t, AF.Exp, scale=1.0 / SQK,
                                             bias=ln32_b[:, 0:1])
                        # qp = max(32q,0) + 32*exp(min(q,0)) = 32*(elu(q)+1)
                        nc.vector.scalar_tensor_tensor(
                            qp_all[:, dc0:dc0 + 2, n0:n0 + NTA], ps_q, 0.0, e_t,
                            OP.max, OP.add)

                    # ---- K, V: layout A, out [token-chunk, 1024 douts] ----
                    for ch in range(NCH):
                        c = it * NCH + ch
                        cs = slice(ch * P, (ch + 1) * P)
                        ps_k = ps.tile([P, D], F32, tag="psA")
                        for h in range(2):
                            hs = slice(h * 512, (h + 1) * 512)
                            for j in range(DO // 2):
                                nc.tensor.matmul(
                                    ps_k[:, hs],
                                    x_t[:, 2 * j:2 * j + 2, cs],
                                    wk_sb[:, 2 * j:2 * j + 2, hs],
                                    start=(j == 0),
                                    stop=(j == DO // 2 - 1 and not use_bk),
                                    perf_mode=DR)
                            if use_bk:
                                nc.tensor.matmul(ps_k[:, hs], ones_pair[:, 0, 0:1],
                                                 rows[0:1, D + h * 512:D + (h + 1) * 512],
                                                 start=False, stop=True)
                        m2 = ev.tile([P, D], BF16, tag="mA2")
                        nc.vector.tensor_scalar_min(m2, ps_k, 0.0)
                        e2 = ev.tile([P, D], BF16, tag="eA2")
                        nc.scalar.activation(e2, m2, AF.Exp, scale=1.0 / SQK,
                                             bias=ln32_b[:, 0:1])
                        nc.vector.scalar_tensor_tensor(kp_all[:, c, :], ps_k, 0.0,
                                                       e2, OP.max, OP.add)

                        ps_v = ps.tile([P, D], F32, tag="psA")
                        for h in range(2):
                            hs = slice(h * 512, (h + 1) * 512)
                            for j in range(DO // 2):
                                nc.tensor.matmul(
                                    ps_v[:, hs],
                                    x_t[:, 2 * j:2 * j + 2, cs],
                                    wv_sb[:, 2 * j:2 * j + 2, hs],
                                    start=(j == 0),
                                    stop=(j == DO // 2 - 1 and not use_bv),
                                    perf_mode=DR)
                            if use_bv:
                                nc.tensor.matmul(ps_v[:, hs], ones_pair[:, 0, 0:1],
                                                 rows[0:1, 2 * D + h * 512:2 * D + (h + 1) * 512],
                                                 start=False, stop=True)
                        nc.scalar.activation(v_all[:, c, :], ps_v, AF.Copy,
                                             scale=1.0 / SW)

            # ---------------- Phase B: KV accumulation + K_sum ----------------
            with ExitStack() as ph:
                ps = ph.enter_context(tc.tile_pool(name="psB", bufs=1, space="PSUM"))
                for dcg in range(2):
                    kv_ps = [ps.tile([P, 2, 512], F32, tag=f"kvps{d}",
                                     name=f"kvps{dcg}_{d}")
                             for d in range(4)]
                    for cp in range(TOTCH // 2):
                        for di in range(4):
                            dc = dcg * 4 + di
                            for eh in range(2):
                                nc.tensor.matmul(
                                    kv_ps[di][:, eh, :],
                                    kp_all[:, 2 * cp:2 * cp + 2, dc * P:(dc + 1) * P],
                                    v_all[:, 2 * cp:2 * cp + 2, eh * 512:(eh + 1) * 512],
                                    start=(cp == 0), stop=(cp == TOTCH // 2 - 1),
                                    perf_mode=DR)
                    for di in range(4):
                        dc = dcg * 4 + di
                        nc.scalar.activation(kv_sb[:, dc, :], kv_ps[di], AF.Copy,
                                             scale=KV_SC)
            with ExitStack() as ph:
                ksp = ph.enter_context(tc.tile_pool(name="ksB", bufs=1, space="PSUM"))
                ps_ks = ksp.tile([1, D], F32, tag="ksrow")
                for cp in range(TOTCH // 2):
                    for h in range(2):
                        hs = slice(h * 512, (h + 1) * 512)
                        nc.tensor.matmul(ps_ks[0:1, hs], ones_pair[:, :, 0:1],
                                         kp_all[:, 2 * cp:2 * cp + 2, hs],
                                         start=(cp == 0), stop=(cp == TOTCH // 2 - 1),
                                         perf_mode=DR)
                nc.scalar.activation(ksrow_sb, ps_ks[0:1, :], AF.Copy)
                # transpose K_sum row -> per-partition column layout [P, DO]
                ps_ksc = ksp.tile([P, DO], F32, tag="kscol")
                for dc in range(DO):
                    nc.tensor.matmul(ps_ksc[:, dc:dc + 1],
                                     ksrow_sb[0:1, dc * P:(dc + 1) * P],
                                     ones_one[0:1, 0:1], start=True, stop=True)
                nc.scalar.activation(ksum_sb[:, :, 0], ps_ksc, AF.Copy,
                                     scale=KS_SC)

        # ---------------- Phases C+D (shared FFN-weight prefetch) ----------------
        cd = top.enter_context(ExitStack())
        wpoolD = cd.enter_context(tc.tile_pool(name="wDpre", bufs=1))
        f1_sb = wpoolD.tile([P, DO, D], BF16, tag="f1")
        nc.sync.dma_start(f1_sb, wr["f1T"])
        f2_sb = wpoolD.tile([P, DO, D], FP8, tag="f2")
        nc.sync.dma_start(f2_sb, wr["f2T"])

        # ---------------- Phase C: conv'' + local MLP + token mixer ----------------
        with ExitStack() as ph:
            wpool = ph.enter_context(tc.tile_pool(name="wC", bufs=1))
            w1_sb = wpool.tile([P, DO, D], BF16, tag="w1")
            nc.sync.dma_start(w1_sb, wr["w1T"])
            w2_sb = wpool.tile([P, DO, D], BF16, tag="w2")
            nc.sync.dma_start(w2_sb, wr["w2T"])
            io = ph.enter_context(tc.tile_pool(name="ioC", bufs=2))
            pipe = ph.enter_context(tc.tile_pool(name="pipeC", bufs=2))
            mid = ph.enter_context(tc.tile_pool(name="midC", bufs=1))
            sm = ph.enter_context(tc.tile_pool(name="smC", bufs=1))
            ps = ph.enter_context(tc.tile_pool(name="psC", bufs=3, space="PSUM"))
            pst = ph.enter_context(tc.tile_pool(name="pstC", bufs=2, space="PSUM"))

            def c_front(it):
                n0 = it * NT
                x_t = io.tile([P, DO, W], BF16, tag="xC", name=f"x_{it}")
                xq_t = io.tile([P, DO, WST], FP8, tag="xqC", name=f"xq_{it}")
                lo, hi = n0 - 2, n0 + NT + 2
                if lo < 0:
                    nc.vector.memset(x_t[:, :, 0:2], 0.0)
                    nc.sync.dma_start(x_t[:, :, 2:W], xT[:, :, 0:hi])
                    nc.vector.memset(xq_t[:, :, 0:2], 0.0)
                    nc.sync.dma_start(xq_t[:, :, 2:W], xqT[:, :, 0:hi])
                elif hi > N:
                    nc.vector.memset(x_t[:, :, W - 2:W], 0.0)
                    nc.sync.dma_start(x_t[:, :, 0:W - 2], xT[:, :, lo:N])
                    nc.vector.memset(xq_t[:, :, W - 2:W], 0.0)
                    nc.sync.dma_start(xq_t[:, :, 0:W - 2], xqT[:, :, lo:N])
                else:
                    nc.sync.dma_start(x_t, xT[:, :, lo:hi])
                    nc.sync.dma_start(xq_t[:, :, 0:W], xqT[:, :, lo:hi])

                acc = io.tile([P, DO, NT], F32, tag="accC", name=f"acc_{it}")
                # diffusion dwconv'': center tap on ACT, side taps on DVE
                for o in range(DO):
                    nc.scalar.activation(acc[:, o, :], x_t[:, o, 2:NT + 2],
                                         AF.Identity, bias=pp[:, o, CB:CB + 1],
                                         scale=pp[:, o, C1:C1 + 1])
                for o in range(DO):
                    nc.vector.scalar_tensor_tensor(acc[:, o, :], x_t[:, o, 1:NT + 1],
                                                   pp[:, o, C0:C0 + 1], acc[:, o, :],
                                                   OP.mult, OP.add)
                for o in range(DO):
                    nc.vector.scalar_tensor_tensor(acc[:, o, :], x_t[:, o, 3:NT + 3],
                                                   pp[:, o, C2:C2 + 1], acc[:, o, :],
                                                   OP.mult, OP.add)

                # local MLP first half (bf16, paired dc groups)
                h1_t = pipe.tile([P, DO, NT], BF16, tag="h1", name=f"h1_{it}")
                for dc0 in range(0, DO, 2):
                    ps_h = ps.tile([P, 2, NT], F32, tag="psC",
                                   name=f"psh1_{it}_{dc0}")
                    for h in range(2):
                        dc = dc0 + h
                        for kc in range(DO):
                            nc.tensor.matmul(ps_h[:, h, :],
                                             w1_sb[:, kc, dc * P:(dc + 1) * P],
                                             x_t[:, kc, 2:NT + 2],
                                             start=(kc == 0), stop=(kc == DO - 1))
                    for h in range(2):
                        nc.scalar.activation(h1_t[:, dc0 + h, :], ps_h[:, h, :],
                                             AF.Gelu,
                                             bias=pp[:, dc0 + h, LUB1:LUB1 + 1])

                # token mixer LN stats (fp8 sum-matmuls on xq/sq)
                sq_t = mid.tile([P, DO, WST], FP8, tag="sqC", name=f"sq_{it}")
                nc.scalar.activation(sq_t[:, :, 0:W], x_t, AF.Square)
                ps_m = pst.tile([P, W], F32, tag="psm", name=f"psm_{it}")
                stats_q(ps_m, xq_t, W)
                ps_s = pst.tile([P, W], F32, tag="pss", name=f"pss_{it}")
                stats_q(ps_s, sq_t, W)
                m_sb = sm.tile([P, 1, W], BF16, tag="msb", name=f"msb_{it}")
                nc.scalar.activation(m_sb[:, 0, :], ps_m, AF.Copy, scale=1.0 / D)
                var = sm.tile([P, 1, W], F32, tag="var", name=f"var_{it}")
                nc.scalar.activation(var[:, 0, :], ps_m, AF.Square, scale=1.0 / D)
                nc.vector.scalar_tensor_tensor(var[:, 0, :], ps_s, 1.0 / D,
                                               var[:, 0, :], OP.mult, OP.subtract)
                nc.scalar.activation(var[:, 0, :], var[:, 0, :], AF.Sqrt,
                                     bias=eps_ln[:, 0:1])
                nc.vector.reciprocal_approx_fast(out=var[:, 0, :], in_=var[:, 0, :])
                rstd = sm.tile([P, 1, W], BF16, tag="rstd", name=f"rstd_{it}")
                nc.vector.tensor_copy(rstd, var)
                u_t = mid.tile([P, DO, W], BF16, tag="tokA", name=f"u_{it}")
                nc.vector.tensor_sub(u_t, x_t, m_sb.to_broadcast((P, DO, W)))
                xm_t = mid.tile([P, DO, W], BF16, tag="tokC", name=f"xm_{it}")
                if use_tmg:
                    for o in range(DO):
                        nc.vector.scalar_tensor_tensor(
                            xm_t[:, o, :], u_t[:, o, :], pp[:, o, TMG:TMG + 1],
                            rstd[:, 0, :], OP.mult, OP.mult)
                else:
                    nc.vector.tensor_mul(xm_t, u_t, rstd.to_broadcast((P, DO, W)))
                if use_tmb:
                    for o in range(DO):
                        nc.vector.tensor_scalar_add(xm_t[:, o, :], xm_t[:, o, :],
                                                    pp[:, o, TMB:TMB + 1])
                # conv1: t_s[k] = conv1(xm)[k+1], k in [0, W-2)
                # (reuses u_t's buffer — u is dead once xm is computed)
                t_full = mid.tile([P, DO, W], BF16, tag="tokA", name=f"t_{it}")
                t_t = t_full[:, :, 0:W - 2]
                for o in range(DO):
                    nc.scalar.activation(t_t[:, o, :], xm_t[:, o, 1:W - 1],
                                         AF.Identity, bias=pp[:, o, TCB1:TCB1 + 1],
                                         scale=pp[:, o, T1:T1 + 1])
                for o in range(DO):
                    nc.vector.scalar_tensor_tensor(t_t[:, o, :], xm_t[:, o, 0:W - 2],
                                                   pp[:, o, T0:T0 + 1],
                                                   t_t[:, o, :], OP.mult, OP.add)
                for o in range(DO):
                    nc.vector.scalar_tensor_tensor(t_t[:, o, :], xm_t[:, o, 2:W],
                                                   pp[:, o, T2:T2 + 1],
                                                   t_t[:, o, :], OP.mult, OP.add)
                t2_t = pipe.tile([P, DO, W - 2], BF16, tag="t2", name=f"t2_{it}")
                nc.scalar.activation(t2_t, t_t, AF.Gelu)
                if it == 0:
                    nc.vector.memset(t2_t[:, :, 0:1], 0.0)
                if it == NTILES - 1:
                    nc.vector.memset(t2_t[:, :, W - 3:W - 2], 0.0)
                return x_t, acc, h1_t, t2_t

            def c_back(it, tiles):
                n0 = it * NT
                x_t, acc, h1_t, t2_t = tiles
                for dc0 in range(0, DO, 2):
                    ps_h = ps.tile([P, 2, NT], F32, tag="psC",
                                   name=f"psh2_{it}_{dc0}")
                    for h in range(2):
                        dc = dc0 + h
                        for kc in range(DO):
                            nc.tensor.matmul(ps_h[:, h, :],
                                             w2_sb[:, kc, dc * P:(dc + 1) * P],
                                             h1_t[:, kc, :],
                                             start=(kc == 0), stop=False)
                        for tap in range(3):
                            nc.tensor.matmul(ps_h[:, h, :], diags[:, tap, dc, :],
                                             t2_t[:, dc, tap:NT + tap],
                                             start=False, stop=(tap == 2))
                    nc.vector.tensor_add(acc[:, dc0:dc0 + 2, :],
                                         acc[:, dc0:dc0 + 2, :], ps_h)
                nc.sync.dma_start(acc_r[:, :, n0:n0 + NT], acc)

            pend = {0: c_front(0)}
            for it in range(NTILES):
                if it + 1 < NTILES:
                    pend[it + 1] = c_front(it + 1)
                c_back(it, pend.pop(it))

        # ---------------- Phase D: attention + LN1 + FFN + LN2 ----------------
        with ExitStack() as ph:
            io = ph.enter_context(tc.tile_pool(name="ioD", bufs=2))
            mid = ph.enter_context(tc.tile_pool(name="midD", bufs=2))
            sm = ph.enter_context(tc.tile_pool(name="smD", bufs=2))
            ps = ph.enter_context(tc.tile_pool(name="psD", bufs=2, space="PSUM"))
            pst = ph.enter_context(tc.tile_pool(name="pstD", bufs=2, space="PSUM"))

            def d_front_a(it):
                """loads, norm row, rep fold, numerator halves 0-3."""
                n0 = it * NT
                acc_t = io.tile([P, DO, NT], F32, tag="accD", name=f"accD_{it}")
                nc.sync.dma_start(acc_t, acc_r[:, :, n0:n0 + NT])
                ps_n = pst.tile([P, NT], F32, tag="psrep", name=f"psn_{it}")
                for j in range(DO // 2):
                    nc.tensor.matmul(ps_n[0:1, :], ksum_sb[:, 2 * j:2 * j + 2, 0:1],
                                     qp_all[:, 2 * j:2 * j + 2, n0:n0 + NT],
                                     start=(j == 0), stop=(j == DO // 2 - 1),
                                     perf_mode=DR)
                rr32 = sm.tile([1, NT], F32, tag="rr32D", name=f"rr32_{it}")
                nc.vector.reciprocal_approx_fast(out=rr32, in_=ps_n[0:1, :])
                rr = sm.tile([1, NT], BF16, tag="rrD", name=f"rr_{it}")
                nc.vector.tensor_copy(rr, rr32)
                ps_rep = pst.tile([P, NT], F32, tag="psrep", name=f"psrep_{it}")
                nc.tensor.matmul(ps_rep, ones_1p_bf[0:1, :], rr, start=True,
                                 stop=True)
                rep_sb = mid.tile([P, 1, NT], BF16, tag="repsb", name=f"rep_{it}")
                nc.scalar.activation(rep_sb[:, 0, :], ps_rep, AF.Copy,
                                     scale=REP_SC)
                qp2_t = mid.tile([P, DO, NT], FP8, tag="qp2", name=f"qp2_{it}")
                nc.vector.tensor_mul(qp2_t, qp_all[:, :, n0:n0 + NT],
                                     rep_sb.to_broadcast((P, DO, NT)))
                for ec0 in range(0, DO // 2, 2):
                    ps_u = ps.tile([P, 2, NT], F32, tag="psD",
                                   name=f"psnum_{it}_{ec0}")
                    for h in range(2):
                        ec = ec0 + h
                        for j in range(DO // 2):
                            nc.tensor.matmul(
                                ps_u[:, h, :],
                                kv_sb[:, 2 * j:2 * j + 2, ec * P:(ec + 1) * P],
                                qp2_t[:, 2 * j:2 * j + 2, :],
                                start=(j == 0), stop=(j == DO // 2 - 1),
                                perf_mode=DR)
                    nc.vector.scalar_tensor_tensor(acc_t[:, ec0:ec0 + 2, :], ps_u,
                                                   NUM_SC, acc_t[:, ec0:ec0 + 2, :],
                                                   OP.mult, OP.add)
                return qp2_t, acc_t

            def d_front_b(it, T):
                qp2_t, acc_t = T
                for ec0 in range(DO // 2, DO, 2):
                    ps_u = ps.tile([P, 2, NT], F32, tag="psD",
                                   name=f"psnum_{it}_{ec0}")
                    for h in range(2):
                        ec = ec0 + h
                        for j in range(DO // 2):
                            nc.tensor.matmul(
                                ps_u[:, h, :],
                                kv_sb[:, 2 * j:2 * j + 2, ec * P:(ec + 1) * P],
                                qp2_t[:, 2 * j:2 * j + 2, :],
                                start=(j == 0), stop=(j == DO // 2 - 1),
                                perf_mode=DR)
                    nc.vector.scalar_tensor_tensor(acc_t[:, ec0:ec0 + 2, :], ps_u,
                                                   NUM_SC, acc_t[:, ec0:ec0 + 2, :],
                                                   OP.mult, OP.add)
                return acc_t

            def d_mid(it, acc_t):
                """LN1 stats (fp8 sum-matmuls) + apply -> y1 (bf16)."""
                accq = mid.tile([P, DO, NT], FP8, tag="accq", name=f"accq_{it}")
                nc.scalar.activation(accq, acc_t, AF.Copy)
                sqq = mid.tile([P, DO, NT], FP8, tag="sqD", name=f"sqD_{it}")
                nc.scalar.activation(sqq, acc_t, AF.Square)
                ps_m1 = pst.tile([P, NT], F32, tag="psm1", name=f"psm1_{it}")
                stats_q(ps_m1, accq, NT)
                ps_s1 = pst.tile([P, NT], F32, tag="pss1", name=f"pss1_{it}")
                stats_q(ps_s1, sqq, NT)
                m1_sb = sm.tile([P, 1, NT], F32, tag="m1sb", name=f"m1_{it}")
                nc.scalar.activation(m1_sb[:, 0, :], ps_m1, AF.Copy, scale=1.0 / D)
                var1 = sm.tile([P, 1, NT], F32, tag="varD", name=f"var1_{it}")
                nc.scalar.activation(var1[:, 0, :], ps_m1, AF.Square, scale=1.0 / D)
                nc.vector.scalar_tensor_tensor(var1[:, 0, :], ps_s1, 1.0 / D,
                                               var1[:, 0, :], OP.mult, OP.subtract)
                nc.scalar.activation(var1[:, 0, :], var1[:, 0, :], AF.Sqrt,
                                     bias=eps_ln[:, 0:1])
                nc.vector.reciprocal_approx_fast(out=var1[:, 0, :],
                                                 in_=var1[:, 0, :])
                u1_t = mid.tile([P, DO, NT], BF16, tag="u1", name=f"u1_{it}")
                nc.vector.tensor_sub(u1_t, acc_t, m1_sb.to_broadcast((P, DO, NT)))
                y1_t = mid.tile([P, DO, NT], BF16, tag="y1", name=f"y1_{it}")
                if use_n1g:
                    for o in range(DO):
                        nc.vector.scalar_tensor_tensor(
                            y1_t[:, o, :], u1_t[:, o, :], pp[:, o, N1G:N1G + 1],
                            var1[:, 0, :], OP.mult, OP.mult)
                else:
                    nc.vector.tensor_mul(y1_t, u1_t,
                                         var1.to_broadcast((P, DO, NT)))
                if use_n1b:
                    for o in range(DO):
                        nc.vector.tensor_scalar_add(y1_t[:, o, :], y1_t[:, o, :],
                                                    pp[:, o, N1B:N1B + 1])
                return y1_t

            def d_ffn(it, y1_t):
                f1h_t = mid.tile([P, DO, NT], FP8, tag="f1h", name=f"f1h_{it}")
                for dc0 in range(0, DO, 2):
                    ps_f = ps.tile([P, 2, NT], F32, tag="psD",
                                   name=f"psf1_{it}_{dc0}")
                    for h in range(2):
                        dc = dc0 + h
                        for kc in range(DO):
                            nc.tensor.matmul(ps_f[:, h, :],
                                             f1_sb[:, kc, dc * P:(dc + 1) * P],
                                             y1_t[:, kc, :],
                                             start=(kc == 0), stop=(kc == DO - 1))
                    for h in range(2):
                        nc.scalar.activation(f1h_t[:, dc0 + h, :], ps_f[:, h, :],
                                             AF.Gelu,
                                             bias=pp[:, dc0 + h, FFB1:FFB1 + 1])
                y2_t = mid.tile([P, DO, NT], BF16, tag="y2", name=f"y2_{it}")
                for dc0 in range(0, DO, 2):
                    ps_f = ps.tile([P, 2, NT], F32, tag="psD",
                                   name=f"psf2_{it}_{dc0}")
                    for h in range(2):
                        dc = dc0 + h
                        for j in range(DO // 2):
                            nc.tensor.matmul(
                                ps_f[:, h, :],
                                f2_sb[:, 2 * j:2 * j + 2, dc * P:(dc + 1) * P],
                                f1h_t[:, 2 * j:2 * j + 2, :],
                                start=(j == 0), stop=(j == DO // 2 - 1),
                                perf_mode=DR)
                    # y2 = y1 + psum/64
                    nc.vector.scalar_tensor_tensor(y2_t[:, dc0:dc0 + 2, :], ps_f,
                                                   1.0 / SW,
                                                   y1_t[:, dc0:dc0 + 2, :],
                                                   OP.mult, OP.add)
                if use_fb2:
                    for dc in range(DO):
                        nc.vector.tensor_scalar_add(y2_t[:, dc, :], y2_t[:, dc, :],
                                                    pp[:, dc, FFB2:FFB2 + 1])
                return y2_t

            def d_back(it, y2_t):
                n0 = it * NT
                y2q = mid.tile([P, DO, NT], FP8, tag="y2q", name=f"y2q_{it}")
                nc.scalar.activation(y2q, y2_t, AF.Copy)
                sq2_t = mid.tile([P, DO, NT], FP8, tag="sq2", name=f"sq2_{it}")
                nc.scalar.activation(sq2_t, y2_t, AF.Square)
                ps_m2 = pst.tile([P, NT], F32, tag="psm1", name=f"psm2_{it}")
                stats_q(ps_m2, y2q, NT)
                ps_s2 = pst.tile([P, NT], F32, tag="pss1", name=f"pss2_{it}")
                stats_q(ps_s2, sq2_t, NT)
                m2_sb = sm.tile([P, 1, NT], F32, tag="m2sb", name=f"m2_{it}")
                nc.scalar.activation(m2_sb[:, 0, :], ps_m2, AF.Copy, scale=1.0 / D)
                var2 = sm.tile([P, 1, NT], F32, tag="varD", name=f"var2_{it}")
                nc.scalar.activation(var2[:, 0, :], ps_m2, AF.Square, scale=1.0 / D)
                nc.vector.scalar_tensor_tensor(var2[:, 0, :], ps_s2, 1.0 / D,
                                               var2[:, 0, :], OP.mult, OP.subtract)
                nc.scalar.activation(var2[:, 0, :], var2[:, 0, :], AF.Sqrt,
                                     bias=eps_ln[:, 0:1])
                nc.vector.reciprocal_approx_fast(out=var2[:, 0, :],
                                                 in_=var2[:, 0, :])
                yo_t = mid.tile([P, DO, NT], F32, tag="yo", name=f"yo_{it}")
                nc.vector.tensor_sub(yo_t, y2_t, m2_sb.to_broadcast((P, DO, NT)))
                if use_n2g:
                    for o in range(DO):
                        nc.vector.scalar_tensor_tensor(
                            yo_t[:, o, :], yo_t[:, o, :], pp[:, o, N2G:N2G + 1],
                            var2[:, 0, :], OP.mult, OP.mult)
                else:
                    nc.vector.tensor_mul(yo_t, yo_t,
                                         var2.to_broadcast((P, DO, NT)))
                if use_n2b:
                    for o in range(DO):
                        nc.vector.tensor_scalar_add(yo_t[:, o, :], yo_t[:, o, :],
                                                    pp[:, o, N2B:N2B + 1])
                nc.sync.dma_start(yT[:, :, n0:n0 + NT], yo_t)

            Tcur = d_front_a(0)
            acc_cur = d_front_b(0, Tcur)
            Tnext = None
            for it in range(NTILES):
                y1_cur = d_mid(it, acc_cur)
                if it + 1 < NTILES:
                    Tnext = d_front_a(it + 1)
                y2_cur = d_ffn(it, y1_cur)
                if it + 1 < NTILES:
                    acc_cur = d_front_b(it + 1, Tnext)
                d_back(it, y2_cur)

    nc.compile()
    return nc


def make_in_maps(inputs, n_cores=8):
    """Host-side preprocessing: fold constants, transpose, cast, shard."""
    x = np.asarray(inputs["x"], np.float32)
    B, N, D_ = x.shape
    dt = float(np.asarray(inputs["delta_t"]))

    def g(k):
        return np.asarray(inputs[k], np.float32)

    diff_w, diff_b = g("diff_w"), g("diff_b")
    tm_w1, tm_cb1 = g("tm_w1"), g("tm_cb1")
    tm_w2, tm_cb2 = g("tm_w2"), g("tm_cb2")

    pp = np.zeros((P, DO, NPARAM), np.float32)

    def put(i, v):
        pp[:, :, i] = v.reshape(DO, P).T

    put(C0, dt * diff_w[:, 0, 0])
    put(C1, dt * diff_w[:, 0, 1] + (1.0 - dt))
    put(C2, dt * diff_w[:, 0, 2])
    put(CB, dt * diff_b + g("lu_b2") + tm_cb2)
    put(T0, tm_w1[:, 0, 0])
    put(T1, tm_w1[:, 0, 1])
    put(T2, tm_w1[:, 0, 2])
    put(TCB1, tm_cb1)
    put(U0, tm_w2[:, 0, 0])
    put(U1, tm_w2[:, 0, 1])
    put(U2, tm_w2[:, 0, 2])
    put(TMG, g("tm_g"))
    put(TMB, g("tm_beta"))
    put(N1G, g("n1_g"))
    put(N1B, g("n1_b"))
    put(N2G, g("n2_g"))
    put(N2B, g("n2_b"))
    put(LUB1, g("lu_b1"))
    put(FFB1, g("ff_b1"))
    put(FFB2, g("ff_b2"))

    diags = np.zeros((P, 3, DO, P), np.float32)
    idx = np.arange(P)
    for tap in range(3):
        for dc in range(DO):
            diags[idx, tap, dc, idx] = tm_w2[dc * P + idx, 0, tap]
    diags = diags.astype(BF16_NP)

    rows = np.zeros((1, 3 * D), np.float32)
    rows[0, 0:D] = SQK * g("bq")
    rows[0, D:2 * D] = SQK * g("bk")
    rows[0, 2 * D:3 * D] = SW * g("bv")
    rows = np.clip(rows, -240, 240).astype(FP8_NP)

    wt = {}
    for name, key, sc in (("wqT", "wq", SQK), ("wkT", "wk", SQK),
                          ("wvT", "wv", SW), ("f2T", "ff_w2", SW)):
        wt[name] = np.clip(
            np.ascontiguousarray(g(key).T) * sc, -240, 240).astype(FP8_NP)
    for name, key in (("w1T", "lu_w1"), ("w2T", "lu_w2"), ("f1T", "ff_w1")):
        wt[name] = np.ascontiguousarray(g(key).T).astype(BF16_NP)

    xt_f = np.ascontiguousarray(x.transpose(0, 2, 1))
    xT = xt_f.astype(BF16_NP)
    xqT = np.clip(xt_f, -240, 240).astype(FP8_NP)

    ones = np.ones((D_,), np.float32)
    flags = dict(
        use_bq=bool(np.any(g("bq"))),
        use_bk=bool(np.any(g("bk"))),
        use_bv=bool(np.any(g("bv"))),
        use_tmb=bool(np.any(g("tm_beta"))),
        use_n1b=bool(np.any(g("n1_b"))),
        use_n2b=bool(np.any(g("n2_b"))),
        use_fb2=bool(np.any(g("ff_b2"))),
        use_tmg=bool(np.any(g("tm_g") != ones)),
        use_n1g=bool(np.any(g("n1_g") != ones)),
        use_n2g=bool(np.any(g("n2_g") != ones)),
    )

    shared = {**wt, "pp": pp, "rows": rows, "diags": diags}
    in_maps = [{**shared, "x_T": xT[b], "xq_T": xqT[b]} for b in range(n_cores)]
    return in_maps, flags, (B, N)


_NC_CACHE = {}


def kernel(**inputs):
    in_maps, flags, (B, N) = make_in_maps(inputs)
    key = (N, tuple(sorted(flags.items())))
    if key not in _NC_CACHE:
        _NC_CACHE[key] = build_nc(N=N, NT=512, **flags)
    nc = _NC_CACHE[key]
    res = run_bass_kernel_spmd(nc, in_maps, list(range(B)))
    y = np.stack([res.results[b]["y_T"] for b in range(B)])
    return np.ascontiguousarray(y.transpose(0, 2, 1)).astype(np.float32)


# revision 14
# speedup vs baseline: 1.0566x; 1.0566x over previous
"""Trainium2 Bass kernel for nn_DiffuRNNLayer (B=8, N=2048, D=1024).

Sharding: data-parallel over batch — one batch element per NeuronCore (8 cores).
v3: mixed fp8-e4m3 DoubleRow / bf16 matmuls, chosen per-path by error budget:
  fp8 DR: wq/wk/wv (phase A), KV (B), f2+f1h (D), numerator+norm (D),
          LN-mean matmuls for tokenmixer-LN/LN1/LN2.
  bf16:   lu_w1/lu_w2 (local MLP, residual-critical), ff_w1, conv diag taps.
Scale bookkeeping: wq,wk x32; wv,f2 x64; qp_all=32*Qp, kp_all=32*Kp, v_all=V,
kv_sb=KV/2, ksum_sb=Ksum/16, rep=2^16/norm, numerator PSUM=2^20*attn.
Kp/V/Qp are SBUF-resident fp8 (no HBM spill); KV+K_sum run back-to-back from
SBUF.  DVE ops are merged across channel chunks wherever gains are 1
(broadcast [P,1,*] operands), and PSUM consumers are paired into [P,2,NT]
tiles to halve instruction count.
"""

import math
import numpy as np
import ml_dtypes
from contextlib import ExitStack

import concourse.bass as bass
import concourse.bacc as bacc
import concourse.tile as tile
import concourse.mybir as mybir
from concourse.bass_utils import run_bass_kernel_spmd

F32 = mybir.dt.float32
BF16 = mybir.dt.bfloat16
FP8 = mybir.dt.float8e4
AF = mybir.ActivationFunctionType
OP = mybir.AluOpType
DR = mybir.MatmulPerfMode.DoubleRow
BF16_NP = ml_dtypes.bfloat16
FP8_NP = ml_dtypes.float8_e4m3

P = 128
D = 1024
DO = D // P  # 8 chunks of the channel dim

SQK = 32.0       # weight scale for wq/wk (keeps 32*Qp < 240 in fp8)
SW = 64.0        # weight scale for wv/f2
LN_SQK = math.log(SQK)
KS_SC = 2.0 ** -9    # ksum fp8 = (sum of 32*Kp) * 2^-9 = Ksum/16
KV_SC = 1.0 / SW     # kv fp8 = (sum 32Kp*V)/64 = KV/2
REP_SC = 2.0 ** 17   # rep = rr * 2^17 = 2^16/norm  (rr = 1/(2*norm))
NUM_SC = 2.0 ** -20  # ps_u = 2^20 * attn  ->  acc += ps_u * 2^-20

# pp param-plane indices (per-partition params, laid out [128, DO, NP])
(C0, C1, C2, CB, T0, T1, T2, TCB1, U0, U1, U2,
 TMG, TMB, N1G, N1B, N2G, N2B, LUB1, FFB1, FFB2) = range(20)
NPARAM = 20



def build_nc(N=2048, NT=512, use_bq=False, use_bk=False, use_bv=False,
             use_tmb=False, use_n1b=False, use_n2b=False, use_fb2=False,
             use_tmg=False, use_n1g=False, use_n2g=False, debug=False):
    NTILES = N // NT
    NTA = 512              # phase-A tile size (independent of C/D tiling)
    NTILES_A = N // NTA
    NCH = NTA // P         # 128-token chunks per phase-A tile
    TOTCH = N // P
    W = NT + 4             # phase-C tile width with +-2 halo
    WST = (W + 15) // 16 * 16  # fp8 stats tile width (16B-aligned)
    assert N % NT == 0 and NT % P == 0 and N % NTA == 0

    nc = bacc.Bacc(None, target_bir_lowering=False, debug=debug)

    xT_d = nc.dram_tensor("x_T", [D, N], BF16, kind="ExternalInput")
    xqT_d = nc.dram_tensor("xq_T", [D, N], FP8, kind="ExternalInput")
    w_d = {}
    for name, dt_ in (("wqT", FP8), ("wkT", FP8), ("wvT", FP8),
                      ("w1T", BF16), ("w2T", BF16), ("f1T", BF16),
                      ("f2T", FP8)):
        w_d[name] = nc.dram_tensor(name, [D, D], dt_, kind="ExternalInput")
    pp_d = nc.dram_tensor("pp", [P, DO, NPARAM], F32, kind="ExternalInput")
    diags_d = nc.dram_tensor("diags", [P, 3, DO, P], BF16, kind="ExternalInput")
    rows_d = nc.dram_tensor("rows", [1, 3 * D], FP8, kind="ExternalInput")
    yT_d = nc.dram_tensor("y_T", [D, N], F32, kind="ExternalOutput")

    acc_sp = nc.dram_tensor("acc_sp", [D, N], F32)

    xT = xT_d.rearrange("(o p) n -> p o n", p=P)
    xqT = xqT_d.rearrange("(o p) n -> p o n", p=P)
    wr = {k: v.rearrange("(o p) n -> p o n", p=P) for k, v in w_d.items()}
    acc_r = acc_sp.rearrange("(o p) n -> p o n", p=P)
    yT = yT_d.rearrange("(o p) n -> p o n", p=P)

    with tile.TileContext(nc) as tc, ExitStack() as top:
        persist = top.enter_context(tc.tile_pool(name="persist", bufs=1))
        pp = persist.tile([P, DO, NPARAM], F32)
        nc.sync.dma_start(pp, pp_d[:])
        rows = ones_row = None
        if use_bq or use_bk or use_bv:
            rows = persist.tile([1, 3 * D], FP8)
            nc.sync.dma_start(rows, rows_d[:])
            ones_row = persist.tile([1, max(NT, 512)], FP8)
            nc.vector.memset(ones_row, 1.0)
        ones_1p_bf = persist.tile([1, P], BF16)
        nc.vector.memset(ones_1p_bf, 1.0)
        ones_pair = persist.tile([P, 2, 16], FP8)
        nc.vector.memset(ones_pair, 1.0)
        ones_q = persist.tile([P, 2, P], FP8)
        nc.vector.memset(ones_q, 1.0)
        ones_one = persist.tile([1, 1], BF16)
        nc.vector.memset(ones_one, 1.0)
        ksrow_sb = persist.tile([1, D], BF16)
        onesD_bf = persist.tile([P, P], BF16)
        nc.vector.memset(onesD_bf, 1.0 / D)
        onesD_f32 = persist.tile([P, P], F32)
        nc.vector.memset(onesD_f32, 1.0 / D)
        eps_ln = persist.tile([P, 1], F32)
        nc.vector.memset(eps_ln, 1e-5)
        ln32_b = persist.tile([P, 1], F32)
        nc.vector.memset(ln32_b, LN_SQK)
        kv_sb = persist.tile([P, DO, D], FP8)
        ksum_sb = persist.tile([P, DO, 16], FP8)
        qp_all = persist.tile([P, DO, N], FP8)
        diags = persist.tile([P, 3, DO, P], BF16)
        nc.sync.dma_start(diags, diags_d[:])

        def stats_q(psum, rhs3, width):
            """Sum over channels (lhs=1.0 fp8 DoubleRow); caller scales by 1/D."""
            for c0 in range(0, width, 512):
                cw = min(512, width - c0)
                for j in range(DO // 2):
                    nc.tensor.matmul(psum[:, c0:c0 + cw], ones_q,
                                     rhs3[:, 2 * j:2 * j + 2, c0:c0 + cw],
                                     start=(j == 0), stop=(j == DO // 2 - 1),
                                     perf_mode=DR)

        def stats_bf(psum, rhs3, width):
            """Mean over channels (lhs=1/D bf16), replicated across partitions."""
            for c0 in range(0, width, 512):
                cw = min(512, width - c0)
                for kc in range(DO):
                    nc.tensor.matmul(psum[:, c0:c0 + cw], onesD_bf,
                                     rhs3[:, kc, c0:c0 + cw],
                                     start=(kc == 0), stop=(kc == DO - 1))

        F32R = mybir.dt.float32r

        # ---------------- Phases A+B scope ----------------
        with ExitStack() as phab:
            kvpool = phab.enter_context(tc.tile_pool(name="kvres", bufs=1))
            kp_all = kvpool.tile([P, TOTCH, D], FP8, tag="kp")
            v_all = kvpool.tile([P, TOTCH, D], FP8, tag="v")

            # ---------------- Phase A: QKV ----------------
            with ExitStack() as ph:
                wpool = ph.enter_context(tc.tile_pool(name="wA", bufs=1))
                wq_sb = wpool.tile([P, DO, D], FP8, tag="wq")
                nc.sync.dma_start(wq_sb, wr["wqT"])
                wk_sb = wpool.tile([P, DO, D], FP8, tag="wk")
                nc.sync.dma_start(wk_sb, wr["wkT"])
                wv_sb = wpool.tile([P, DO, D], FP8, tag="wv")
                nc.sync.dma_start(wv_sb, wr["wvT"])
                io = ph.enter_context(tc.tile_pool(name="ioA", bufs=2))
                ev = ph.enter_context(tc.tile_pool(name="evA", bufs=3))
                ps = ph.enter_context(tc.tile_pool(name="psA", bufs=3, space="PSUM"))

                for it in range(NTILES_A):
                    n0 = it * NTA
                    x_t = io.tile([P, DO, NTA], FP8, tag="xA")
                    nc.sync.dma_start(x_t, xqT[:, :, n0:n0 + NTA])

                    # ---- Q: layout B, out [dout-pair, n]; psum [P, 2*NT] flat
                    for dc0 in range(0, DO, 2):
                        ps_q = ps.tile([P, 2 * NTA], F32, tag="psA")
                        for h in range(2):
                            dc = dc0 + h
                            for j in range(DO // 2):
                                nc.tensor.matmul(
                                    ps_q[:, h * NTA:(h + 1) * NTA],
                                    wq_sb[:, 2 * j:2 * j + 2, dc * P:(dc + 1) * P],
                                    x_t[:, 2 * j:2 * j + 2, :],
                                    start=(j == 0),
                                    stop=(j == DO // 2 - 1 and not use_bq),
                                    perf_mode=DR)
                            if use_bq:
                                nc.tensor.matmul(ps_q[:, h * NTA:(h + 1) * NTA],
                                                 rows[0:1, dc * P:(dc + 1) * P],
                                                 ones_row[0:1, 0:NTA],
                                                 start=False, stop=True)
                        m_t = ev.tile([P, 2 * NTA], BF16, tag="mA")
                        nc.vector.tensor_scalar_min(m_t, ps_q, 0.0)
                        e_t = ev.tile([P, 2 * NTA], BF16, tag="eA")
                        # 32*exp(min(q,0)) = exp(min(32q,0)/32 + ln32)
                        nc.scalar.activation(e_t, m_t, AF.Exp, scale=1.0 / SQK,
                                             bias=ln32_b[:, 0:1])
                        # qp = max(32q,0) + 32*exp(min(q,0)) = 32*(elu(q)+1)
                        nc.vector.scalar_tensor_tensor(
                            qp_all[:, dc0:dc0 + 2, n0:n0 + NTA], ps_q, 0.0, e_t,
                            OP.max, OP.add)

                    # ---- K, V: layout A, out [token-chunk, 1024 douts] ----
                    for ch in range(NCH):
                        c = it * NCH + ch
                        cs = slice(ch * P, (ch + 1) * P)
                        ps_k = ps.tile([P, D], F32, tag="psA")
                        for h in range(2):
                            hs = slice(h * 512, (h + 1) * 512)
                            for j in range(DO // 2):
                                nc.tensor.matmul(
                                    ps_k[:, hs],
                                    x_t[:, 2 * j:2 * j + 2, cs],
                                    wk_sb[:, 2 * j:2 * j + 2, hs],
                                    start=(j == 0),
                                    stop=(j == DO // 2 - 1 and not use_bk),
                                    perf_mode=DR)
                            if use_bk:
                                nc.tensor.matmul(ps_k[:, hs], ones_pair[:, 0, 0:1],
                                                 rows[0:1, D + h * 512:D + (h + 1) * 512],
                                                 start=False, stop=True)
                        m2 = ev.tile([P, D], BF16, tag="mA2")
                        nc.vector.tensor_scalar_min(m2, ps_k, 0.0)
                        e2 = ev.tile([P, D], BF16, tag="eA2")
                        nc.scalar.activation(e2, m2, AF.Exp, scale=1.0 / SQK,
                                             bias=ln32_b[:, 0:1])
                        nc.vector.scalar_tensor_tensor(kp_all[:, c, :], ps_k, 0.0,
                                                       e2, OP.max, OP.add)

                        ps_v = ps.tile([P, D], F32, tag="psA")
                        for h in range(2):
                            hs = slice(h * 512, (h + 1) * 512)
                            for j in range(DO // 2):
                                nc.tensor.matmul(
                                    ps_v[:, hs],
                                    x_t[:, 2 * j:2 * j + 2, cs],
                                    wv_sb[:, 2 * j:2 * j + 2, hs],
                                    start=(j == 0),
                                    stop=(j == DO // 2 - 1 and not use_bv),
                                    perf_mode=DR)
                            if use_bv:
                                nc.tensor.matmul(ps_v[:, hs], ones_pair[:, 0, 0:1],
                                                 rows[0:1, 2 * D + h * 512:2 * D + (h + 1) * 512],
                                                 start=False, stop=True)
                        nc.scalar.activation(v_all[:, c, :], ps_v, AF.Copy,
                                             scale=1.0 / SW)

            # ---------------- Phase B: KV accumulation + K_sum ----------------
            with ExitStack() as ph:
                ps = ph.enter_context(tc.tile_pool(name="psB", bufs=1, space="PSUM"))
                for dcg in range(2):
                    kv_ps = [ps.tile([P, 2, 512], F32, tag=f"kvps{d}",
                                     name=f"kvps{dcg}_{d}")
                             for d in range(4)]
                    for cp in range(TOTCH // 2):
                        for di in range(4):
                            dc = dcg * 4 + di
                            for eh in range(2):
                                nc.tensor.matmul(
                                    kv_ps[di][:, eh, :],
                                    kp_all[:, 2 * cp:2 * cp + 2, dc * P:(dc + 1) * P],
                                    v_all[:, 2 * cp:2 * cp + 2, eh * 512:(eh + 1) * 512],
                                    start=(cp == 0), stop=(cp == TOTCH // 2 - 1),
                                    perf_mode=DR)
                    for di in range(4):
                        dc = dcg * 4 + di
                        nc.scalar.activation(kv_sb[:, dc, :], kv_ps[di], AF.Copy,
                                             scale=KV_SC)
            with ExitStack() as ph:
                ksp = ph.enter_context(tc.tile_pool(name="ksB", bufs=1, space="PSUM"))
                ps_ks = ksp.tile([1, D], F32, tag="ksrow")
                for cp in range(TOTCH // 2):
                    for h in range(2):
                        hs = slice(h * 512, (h + 1) * 512)
                        nc.tensor.matmul(ps_ks[0:1, hs], ones_pair[:, :, 0:1],
                                         kp_all[:, 2 * cp:2 * cp + 2, hs],
                                         start=(cp == 0), stop=(cp == TOTCH // 2 - 1),
                                         perf_mode=DR)
                nc.scalar.activation(ksrow_sb, ps_ks[0:1, :], AF.Copy)
                # transpose K_sum row -> per-partition column layout [P, DO]
                ps_ksc = ksp.tile([P, DO], F32, tag="kscol")
                for dc in range(DO):
                    nc.tensor.matmul(ps_ksc[:, dc:dc + 1],
                                     ksrow_sb[0:1, dc * P:(dc + 1) * P],
                                     ones_one[0:1, 0:1], start=True, stop=True)
                nc.scalar.activation(ksum_sb[:, :, 0], ps_ksc, AF.Copy,
                                     scale=KS_SC)

        # ---------------- Phases C+D (shared FFN-weight prefetch) ----------------
        cd = top.enter_context(ExitStack())
        wpoolD = cd.enter_context(tc.tile_pool(name="wDpre", bufs=1))
        f1_sb = wpoolD.tile([P, DO, D], BF16, tag="f1")
        nc.sync.dma_start(f1_sb, wr["f1T"])
        f2_sb = wpoolD.tile([P, DO, D], FP8, tag="f2")
        nc.sync.dma_start(f2_sb, wr["f2T"])

        # ---------------- Phase C: conv'' + local MLP + token mixer ----------------
        with ExitStack() as ph:
            wpool = ph.enter_context(tc.tile_pool(name="wC", bufs=1))
            w1_sb = wpool.tile([P, DO, D], BF16, tag="w1")
            nc.sync.dma_start(w1_sb, wr["w1T"])
            w2_sb = wpool.tile([P, DO, D], BF16, tag="w2")
            nc.sync.dma_start(w2_sb, wr["w2T"])
            io = ph.enter_context(tc.tile_pool(name="ioC", bufs=2))
            pipe = ph.enter_context(tc.tile_pool(name="pipeC", bufs=2))
            mid = ph.enter_context(tc.tile_pool(name="midC", bufs=1))
            sm = ph.enter_context(tc.tile_pool(name="smC", bufs=1))
            ps = ph.enter_context(tc.tile_pool(name="psC", bufs=2, space="PSUM"))
            pst = ph.enter_context(tc.tile_pool(name="pstC", bufs=1, space="PSUM"))

            def c_front(it):
                n0 = it * NT
                x_t = io.tile([P, DO, W], BF16, tag="xC", name=f"x_{it}")
                xq_t = io.tile([P, DO, WST], FP8, tag="xqC", name=f"xq_{it}")
                lo, hi = n0 - 2, n0 + NT + 2
                if lo < 0:
                    nc.vector.memset(x_t[:, :, 0:2], 0.0)
                    nc.sync.dma_start(x_t[:, :, 2:W], xT[:, :, 0:hi])
                    nc.vector.memset(xq_t[:, :, 0:2], 0.0)
                    nc.sync.dma_start(xq_t[:, :, 2:W], xqT[:, :, 0:hi])
                elif hi > N:
                    nc.vector.memset(x_t[:, :, W - 2:W], 0.0)
                    nc.sync.dma_start(x_t[:, :, 0:W - 2], xT[:, :, lo:N])
                    nc.vector.memset(xq_t[:, :, W - 2:W], 0.0)
                    nc.sync.dma_start(xq_t[:, :, 0:W - 2], xqT[:, :, lo:N])
                else:
                    nc.sync.dma_start(x_t, xT[:, :, lo:hi])
                    nc.sync.dma_start(xq_t[:, :, 0:W], xqT[:, :, lo:hi])

                acc = io.tile([P, DO, NT], F32, tag="accC", name=f"acc_{it}")
                # diffusion dwconv'': center tap on ACT, side taps on DVE
                for o in range(DO):
                    nc.scalar.activation(acc[:, o, :], x_t[:, o, 2:NT + 2],
                                         AF.Identity, bias=pp[:, o, CB:CB + 1],
                                         scale=pp[:, o, C1:C1 + 1])
                for o in range(DO):
                    nc.vector.scalar_tensor_tensor(acc[:, o, :], x_t[:, o, 1:NT + 1],
                                                   pp[:, o, C0:C0 + 1], acc[:, o, :],
                                                   OP.mult, OP.add)
                for o in range(DO):
                    nc.vector.scalar_tensor_tensor(acc[:, o, :], x_t[:, o, 3:NT + 3],
                                                   pp[:, o, C2:C2 + 1], acc[:, o, :],
                                                   OP.mult, OP.add)

                # local MLP first half (bf16, paired dc groups)
                h1_t = pipe.tile([P, DO, NT], BF16, tag="h1", name=f"h1_{it}")
                for dc0 in range(0, DO, 2):
                    ps_h = ps.tile([P, 2, NT], F32, tag="psC",
                                   name=f"psh1_{it}_{dc0}")
                    for h in range(2):
                        dc = dc0 + h
                        for kc in range(DO):
                            nc.tensor.matmul(ps_h[:, h, :],
                                             w1_sb[:, kc, dc * P:(dc + 1) * P],
                                             x_t[:, kc, 2:NT + 2],
                                             start=(kc == 0), stop=(kc == DO - 1))
                    for h in range(2):
                        nc.scalar.activation(h1_t[:, dc0 + h, :], ps_h[:, h, :],
                                             AF.Gelu,
                                             bias=pp[:, dc0 + h, LUB1:LUB1 + 1])

                # token mixer LN stats (fp8 sum-matmuls on xq/sq)
                sq_t = mid.tile([P, DO, WST], FP8, tag="sqC", name=f"sq_{it}")
                nc.scalar.activation(sq_t[:, :, 0:W], x_t, AF.Square)
                ps_m = pst.tile([P, W], F32, tag="psm", name=f"psm_{it}")
                stats_q(ps_m, xq_t, W)
                ps_s = pst.tile([P, W], F32, tag="pss", name=f"pss_{it}")
                stats_q(ps_s, sq_t, W)
                m_sb = sm.tile([P, 1, W], BF16, tag="msb", name=f"msb_{it}")
                nc.scalar.activation(m_sb[:, 0, :], ps_m, AF.Copy, scale=1.0 / D)
                var = sm.tile([P, 1, W], F32, tag="var", name=f"var_{it}")
                nc.scalar.activation(var[:, 0, :], ps_m, AF.Square, scale=1.0 / D)
                nc.vector.scalar_tensor_tensor(var[:, 0, :], ps_s, 1.0 / D,
                                               var[:, 0, :], OP.mult, OP.subtract)
                nc.scalar.activation(var[:, 0, :], var[:, 0, :], AF.Sqrt,
                                     bias=eps_ln[:, 0:1])
                nc.vector.reciprocal_approx_fast(out=var[:, 0, :], in_=var[:, 0, :])
                rstd = sm.tile([P, 1, W], BF16, tag="rstd", name=f"rstd_{it}")
                nc.vector.tensor_copy(rstd, var)
                u_t = mid.tile([P, DO, W], BF16, tag="tokA", name=f"u_{it}")
                nc.vector.tensor_sub(u_t, x_t, m_sb.to_broadcast((P, DO, W)))
                xm_t = mid.tile([P, DO, W], BF16, tag="tokC", name=f"xm_{it}")
                if use_tmg:
                    for o in range(DO):
                        nc.vector.scalar_tensor_tensor(
                            xm_t[:, o, :], u_t[:, o, :], pp[:, o, TMG:TMG + 1],
                            rstd[:, 0, :], OP.mult, OP.mult)
                else:
                    nc.vector.tensor_mul(xm_t, u_t, rstd.to_broadcast((P, DO, W)))
                if use_tmb:
                    for o in range(DO):
                        nc.vector.tensor_scalar_add(xm_t[:, o, :], xm_t[:, o, :],
                                                    pp[:, o, TMB:TMB + 1])
                # conv1: t_s[k] = conv1(xm)[k+1], k in [0, W-2)
                # (reuses u_t's buffer — u is dead once xm is computed)
                t_full = mid.tile([P, DO, W], BF16, tag="tokA", name=f"t_{it}")
                t_t = t_full[:, :, 0:W - 2]
                for o in range(DO):
                    nc.scalar.activation(t_t[:, o, :], xm_t[:, o, 1:W - 1],
                                         AF.Identity, bias=pp[:, o, TCB1:TCB1 + 1],
                                         scale=pp[:, o, T1:T1 + 1])
                for o in range(DO):
                    nc.vector.scalar_tensor_tensor(t_t[:, o, :], xm_t[:, o, 0:W - 2],
                                                   pp[:, o, T0:T0 + 1],
                                                   t_t[:, o, :], OP.mult, OP.add)
                for o in range(DO):
                    nc.vector.scalar_tensor_tensor(t_t[:, o, :], xm_t[:, o, 2:W],
                                                   pp[:, o, T2:T2 + 1],
                                                   t_t[:, o, :], OP.mult, OP.add)
                t2_t = pipe.tile([P, DO, W - 2], BF16, tag="t2", name=f"t2_{it}")
                nc.scalar.activation(t2_t, t_t, AF.Gelu)
                if it == 0:
                    nc.vector.memset(t2_t[:, :, 0:1], 0.0)
                if it == NTILES - 1:
                    nc.vector.memset(t2_t[:, :, W - 3:W - 2], 0.0)
                return x_t, acc, h1_t, t2_t

            def c_back(it, tiles):
                n0 = it * NT
                x_t, acc, h1_t, t2_t = tiles
                for dc0 in range(0, DO, 2):
                    ps_h = ps.tile([P, 2, NT], F32, tag="psC",
                                   name=f"psh2_{it}_{dc0}")
                    for h in range(2):
                        dc = dc0 + h
                        for kc in range(DO):
                            nc.tensor.matmul(ps_h[:, h, :],
                                             w2_sb[:, kc, dc * P:(dc + 1) * P],
                                             h1_t[:, kc, :],
                                             start=(kc == 0), stop=False)
                        for tap in range(3):
                            nc.tensor.matmul(ps_h[:, h, :], diags[:, tap, dc, :],
                                             t2_t[:, dc, tap:NT + tap],
                                             start=False, stop=(tap == 2))
                    nc.vector.tensor_add(acc[:, dc0:dc0 + 2, :],
                                         acc[:, dc0:dc0 + 2, :], ps_h)
                nc.sync.dma_start(acc_r[:, :, n0:n0 + NT], acc)

            pend = {0: c_front(0)}
            for it in range(NTILES):
                if it + 1 < NTILES:
                    pend[it + 1] = c_front(it + 1)
                c_back(it, pend.pop(it))

        # ---------------- Phase D: attention + LN1 + FFN + LN2 ----------------
        with ExitStack() as ph:
            io = ph.enter_context(tc.tile_pool(name="ioD", bufs=2))
            mid = ph.enter_context(tc.tile_pool(name="midD", bufs=2))
            sm = ph.enter_context(tc.tile_pool(name="smD", bufs=2))
            ps = ph.enter_context(tc.tile_pool(name="psD", bufs=2, space="PSUM"))
            pst = ph.enter_context(tc.tile_pool(name="pstD", bufs=1, space="PSUM"))

            def d_front_a(it):
                """loads, norm row, rep fold, numerator halves 0-3."""
                n0 = it * NT
                acc_t = io.tile([P, DO, NT], F32, tag="accD", name=f"accD_{it}")
                nc.sync.dma_start(acc_t, acc_r[:, :, n0:n0 + NT])
                ps_n = pst.tile([P, NT], F32, tag="psrep", name=f"psn_{it}")
                for j in range(DO // 2):
                    nc.tensor.matmul(ps_n[0:1, :], ksum_sb[:, 2 * j:2 * j + 2, 0:1],
                                     qp_all[:, 2 * j:2 * j + 2, n0:n0 + NT],
                                     start=(j == 0), stop=(j == DO // 2 - 1),
                                     perf_mode=DR)
                rr32 = sm.tile([1, NT], F32, tag="rr32D", name=f"rr32_{it}")
                nc.vector.reciprocal_approx_fast(out=rr32, in_=ps_n[0:1, :])
                rr = sm.tile([1, NT], BF16, tag="rrD", name=f"rr_{it}")
                nc.vector.tensor_copy(rr, rr32)
                ps_rep = pst.tile([P, NT], F32, tag="psrep", name=f"psrep_{it}")
                nc.tensor.matmul(ps_rep, ones_1p_bf[0:1, :], rr, start=True,
                                 stop=True)
                rep_sb = mid.tile([P, 1, NT], BF16, tag="repsb", name=f"rep_{it}")
                nc.scalar.activation(rep_sb[:, 0, :], ps_rep, AF.Copy,
                                     scale=REP_SC)
                qp2_t = mid.tile([P, DO, NT], FP8, tag="qp2", name=f"qp2_{it}")
                nc.vector.tensor_mul(qp2_t, qp_all[:, :, n0:n0 + NT],
                                     rep_sb.to_broadcast((P, DO, NT)))
                for ec0 in range(0, DO // 2, 2):
                    ps_u = ps.tile([P, 2, NT], F32, tag="psD",
                                   name=f"psnum_{it}_{ec0}")
                    for h in range(2):
                        ec = ec0 + h
                        for j in range(DO // 2):
                            nc.tensor.matmul(
                                ps_u[:, h, :],
                                kv_sb[:, 2 * j:2 * j + 2, ec * P:(ec + 1) * P],
                                qp2_t[:, 2 * j:2 * j + 2, :],
                                start=(j == 0), stop=(j == DO // 2 - 1),
                                perf_mode=DR)
                    nc.vector.scalar_tensor_tensor(acc_t[:, ec0:ec0 + 2, :], ps_u,
                                                   NUM_SC, acc_t[:, ec0:ec0 + 2, :],
                                                   OP.mult, OP.add)
                return qp2_t, acc_t

            def d_front_b(it, T):
                qp2_t, acc_t = T
                for ec0 in range(DO // 2, DO, 2):
                    ps_u = ps.tile([P, 2, NT], F32, tag="psD",
                                   name=f"psnum_{it}_{ec0}")
                    for h in range(2):
                        ec = ec0 + h
                        for j in range(DO // 2):
                            nc.tensor.matmul(
                                ps_u[:, h, :],
                                kv_sb[:, 2 * j:2 * j + 2, ec * P:(ec + 1) * P],
                                qp2_t[:, 2 * j:2 * j + 2, :],
                                start=(j == 0), stop=(j == DO // 2 - 1),
                                perf_mode=DR)
                    nc.vector.scalar_tensor_tensor(acc_t[:, ec0:ec0 + 2, :], ps_u,
                                                   NUM_SC, acc_t[:, ec0:ec0 + 2, :],
                                                   OP.mult, OP.add)
                return acc_t

            def d_mid(it, acc_t):
                """LN1: mean via f32r matmuls on acc, sq-stats bf16; in-place
                apply -> y1 (bf16, two halves so f1 can start early)."""
                accq = mid.tile([P, DO, NT], FP8, tag="accq", name=f"accq_{it}")
                nc.scalar.activation(accq, acc_t, AF.Copy)
                sqq = mid.tile([P, DO, NT], FP8, tag="sqD", name=f"sqD_{it}")
                nc.scalar.activation(sqq, acc_t, AF.Square)
                psst = pst.tile([P, 2, NT], F32, tag="psst", name=f"psst1_{it}")
                stats_q(psst[:, 0, :], accq, NT)
                stats_q(psst[:, 1, :], sqq, NT)
                m1_sb = sm.tile([P, 1, NT], F32, tag="m1sb", name=f"m1_{it}")
                nc.scalar.activation(m1_sb[:, 0, :], psst[:, 0, :], AF.Copy,
                                     scale=1.0 / D)
                var1 = sm.tile([P, 1, NT], F32, tag="varD", name=f"var1_{it}")
                nc.scalar.activation(var1[:, 0, :], psst[:, 0, :], AF.Square,
                                     scale=1.0 / D)
                nc.vector.scalar_tensor_tensor(var1[:, 0, :], psst[:, 1, :],
                                               1.0 / D, var1[:, 0, :],
                                               OP.mult, OP.subtract)
                nc.scalar.activation(var1[:, 0, :], var1[:, 0, :], AF.Sqrt,
                                     bias=eps_ln[:, 0:1])
                nc.vector.reciprocal_approx_fast(out=var1[:, 0, :],
                                                 in_=var1[:, 0, :])
                # u1 in place of acc (acc is dead after the stats)
                nc.vector.tensor_sub(acc_t, acc_t, m1_sb.to_broadcast((P, DO, NT)))
                y1_t = mid.tile([P, DO, NT], BF16, tag="y1", name=f"y1_{it}")
                if use_n1g:
                    for o in range(DO):
                        nc.vector.scalar_tensor_tensor(
                            y1_t[:, o, :], acc_t[:, o, :], pp[:, o, N1G:N1G + 1],
                            var1[:, 0, :], OP.mult, OP.mult)
                else:
                    for hh in range(0, DO, 4):
                        nc.vector.tensor_mul(
                            y1_t[:, hh:hh + 4, :], acc_t[:, hh:hh + 4, :],
                            var1.to_broadcast((P, 4, NT)))
                if use_n1b:
                    for o in range(DO):
                        nc.vector.tensor_scalar_add(y1_t[:, o, :], y1_t[:, o, :],
                                                    pp[:, o, N1B:N1B + 1])
                return y1_t

            def d_ffn(it, y1_t):
                f1h_t = mid.tile([P, DO, NT], FP8, tag="f1h", name=f"f1h_{it}")
                for dc0 in range(0, DO, 2):
                    ps_f = ps.tile([P, 2, NT], F32, tag="psD",
                                   name=f"psf1_{it}_{dc0}")
                    for h in range(2):
                        dc = dc0 + h
                        for kc in range(DO):
                            nc.tensor.matmul(ps_f[:, h, :],
                                             f1_sb[:, kc, dc * P:(dc + 1) * P],
                                             y1_t[:, kc, :],
                                             start=(kc == 0), stop=(kc == DO - 1))
                    for h in range(2):
                        nc.scalar.activation(f1h_t[:, dc0 + h, :], ps_f[:, h, :],
                                             AF.Gelu,
                                             bias=pp[:, dc0 + h, FFB1:FFB1 + 1])
                y2_t = mid.tile([P, DO, NT], BF16, tag="y2", name=f"y2_{it}")
                for dc0 in range(0, DO, 2):
                    ps_f = ps.tile([P, 2, NT], F32, tag="psD",
                                   name=f"psf2_{it}_{dc0}")
                    for h in range(2):
                        dc = dc0 + h
                        for j in range(DO // 2):
                            nc.tensor.matmul(
                                ps_f[:, h, :],
                                f2_sb[:, 2 * j:2 * j + 2, dc * P:(dc + 1) * P],
                                f1h_t[:, 2 * j:2 * j + 2, :],
                                start=(j == 0), stop=(j == DO // 2 - 1),
                                perf_mode=DR)
                    # y2 = y1 + psum/64
                    nc.vector.scalar_tensor_tensor(y2_t[:, dc0:dc0 + 2, :], ps_f,
                                                   1.0 / SW,
                                                   y1_t[:, dc0:dc0 + 2, :],
                                                   OP.mult, OP.add)
                if use_fb2:
                    for dc in range(DO):
                        nc.vector.tensor_scalar_add(y2_t[:, dc, :], y2_t[:, dc, :],
                                                    pp[:, dc, FFB2:FFB2 + 1])
                return y2_t

            def d_back(it, y2_t):
                n0 = it * NT
                sq2_t = mid.tile([P, DO, NT], BF16, tag="sq2", name=f"sq2_{it}")
                nc.scalar.activation(sq2_t, y2_t, AF.Square)
                psst = pst.tile([P, 2, NT], F32, tag="psst", name=f"psst2_{it}")
                stats_bf(psst[:, 0, :], y2_t, NT)
                stats_bf(psst[:, 1, :], sq2_t, NT)
                m2_sb = sm.tile([P, 1, NT], F32, tag="m2sb", name=f"m2_{it}")
                nc.scalar.activation(m2_sb[:, 0, :], psst[:, 0, :], AF.Copy)
                var2 = sm.tile([P, 1, NT], F32, tag="varD", name=f"var2_{it}")
                nc.scalar.activation(var2[:, 0, :], psst[:, 0, :], AF.Square)
                nc.vector.tensor_sub(var2[:, 0, :], psst[:, 1, :], var2[:, 0, :])
                nc.scalar.activation(var2[:, 0, :], var2[:, 0, :], AF.Sqrt,
                                     bias=eps_ln[:, 0:1])
                nc.vector.reciprocal_approx_fast(out=var2[:, 0, :],
                                                 in_=var2[:, 0, :])
                yo_t = mid.tile([P, DO, NT], F32, tag="yo", name=f"yo_{it}",
                                bufs=1)
                nc.vector.tensor_sub(yo_t, y2_t, m2_sb.to_broadcast((P, DO, NT)))
                if use_n2g:
                    for o in range(DO):
                        nc.vector.scalar_tensor_tensor(
                            yo_t[:, o, :], yo_t[:, o, :], pp[:, o, N2G:N2G + 1],
                            var2[:, 0, :], OP.mult, OP.mult)
                else:
                    nc.vector.tensor_mul(yo_t, yo_t,
                                         var2.to_broadcast((P, DO, NT)))
                if use_n2b:
                    for o in range(DO):
                        nc.vector.tensor_scalar_add(yo_t[:, o, :], yo_t[:, o, :],
                                                    pp[:, o, N2B:N2B + 1])
                nc.sync.dma_start(yT[:, :, n0:n0 + NT], yo_t)

            Tcur = d_front_a(0)
            acc_cur = d_front_b(0, Tcur)
            Tnext = None
            for it in range(NTILES):
                y1_cur = d_mid(it, acc_cur)
                if it + 1 < NTILES:
                    Tnext = d_front_a(it + 1)
                y2_cur = d_ffn(it, y1_cur)
                if it + 1 < NTILES:
                    acc_cur = d_front_b(it + 1, Tnext)
                d_back(it, y2_cur)

    nc.compile()
    return nc


def make_in_maps(inputs, n_cores=8):
    """Host-side preprocessing: fold constants, transpose, cast, shard."""
    x = np.asarray(inputs["x"], np.float32)
    B, N, D_ = x.shape
    dt = float(np.asarray(inputs["delta_t"]))

    def g(k):
        return np.asarray(inputs[k], np.float32)

    diff_w, diff_b = g("diff_w"), g("diff_b")
    tm_w1, tm_cb1 = g("tm_w1"), g("tm_cb1")
    tm_w2, tm_cb2 = g("tm_w2"), g("tm_cb2")

    pp = np.zeros((P, DO, NPARAM), np.float32)

    def put(i, v):
        pp[:, :, i] = v.reshape(DO, P).T

    put(C0, dt * diff_w[:, 0, 0])
    put(C1, dt * diff_w[:, 0, 1] + (1.0 - dt))
    put(C2, dt * diff_w[:, 0, 2])
    put(CB, dt * diff_b + g("lu_b2") + tm_cb2)
    put(T0, tm_w1[:, 0, 0])
    put(T1, tm_w1[:, 0, 1])
    put(T2, tm_w1[:, 0, 2])
    put(TCB1, tm_cb1)
    put(U0, tm_w2[:, 0, 0])
    put(U1, tm_w2[:, 0, 1])
    put(U2, tm_w2[:, 0, 2])
    put(TMG, g("tm_g"))
    put(TMB, g("tm_beta"))
    put(N1G, g("n1_g"))
    put(N1B, g("n1_b"))
    put(N2G, g("n2_g"))
    put(N2B, g("n2_b"))
    put(LUB1, g("lu_b1"))
    put(FFB1, g("ff_b1"))
    put(FFB2, g("ff_b2"))

    diags = np.zeros((P, 3, DO, P), np.float32)
    idx = np.arange(P)
    for tap in range(3):
        for dc in range(DO):
            diags[idx, tap, dc, idx] = tm_w2[dc * P + idx, 0, tap]
    diags = diags.astype(BF16_NP)

    rows = np.zeros((1, 3 * D), np.float32)
    rows[0, 0:D] = SQK * g("bq")
    rows[0, D:2 * D] = SQK * g("bk")
    rows[0, 2 * D:3 * D] = SW * g("bv")
    rows = np.clip(rows, -240, 240).astype(FP8_NP)

    wt = {}
    for name, key, sc in (("wqT", "wq", SQK), ("wkT", "wk", SQK),
                          ("wvT", "wv", SW), ("f2T", "ff_w2", SW)):
        wt[name] = np.clip(
            np.ascontiguousarray(g(key).T) * sc, -240, 240).astype(FP8_NP)
    for name, key in (("w1T", "lu_w1"), ("w2T", "lu_w2"), ("f1T", "ff_w1")):
        wt[name] = np.ascontiguousarray(g(key).T).astype(BF16_NP)

    xt_f = np.ascontiguousarray(x.transpose(0, 2, 1))
    xT = xt_f.astype(BF16_NP)
    xqT = np.clip(xt_f, -240, 240).astype(FP8_NP)

    ones = np.ones((D_,), np.float32)
    flags = dict(
        use_bq=bool(np.any(g("bq"))),
        use_bk=bool(np.any(g("bk"))),
        use_bv=bool(np.any(g("bv"))),
        use_tmb=bool(np.any(g("tm_beta"))),
        use_n1b=bool(np.any(g("n1_b"))),
        use_n2b=bool(np.any(g("n2_b"))),
        use_fb2=bool(np.any(g("ff_b2"))),
        use_tmg=bool(np.any(g("tm_g") != ones)),
        use_n1g=bool(np.any(g("n1_g") != ones)),
        use_n2g=bool(np.any(g("n2_g") != ones)),
    )

    shared = {**wt, "pp": pp, "rows": rows, "diags": diags}
    in_maps = [{**shared, "x_T": xT[b], "xq_T": xqT[b]} for b in range(n_cores)]
    return in_maps, flags, (B, N)


_NC_CACHE = {}


def kernel(**inputs):
    in_maps, flags, (B, N) = make_in_maps(inputs)
    key = (N, tuple(sorted(flags.items())))
    if key not in _NC_CACHE:
        _NC_CACHE[key] = build_nc(N=N, NT=512, **flags)
    nc = _NC_CACHE[key]
    res = run_bass_kernel_spmd(nc, in_maps, list(range(B)))
    y = np.stack([res.results[b]["y_T"] for b in range(B)])
    return np.ascontiguousarray(y.transpose(0, 2, 1)).astype(np.float32)


# revision 18
# speedup vs baseline: 1.1057x; 1.0465x over previous
"""Trainium2 Bass kernel for nn_DiffuRNNLayer (B=8, N=2048, D=1024).

Sharding: data-parallel over batch — one batch element per NeuronCore (8 cores).
v3: mixed fp8-e4m3 DoubleRow / bf16 matmuls, chosen per-path by error budget:
  fp8 DR: wq/wk/wv (phase A), KV (B), f2+f1h (D), numerator+norm (D),
          LN-mean matmuls for tokenmixer-LN/LN1/LN2.
  bf16:   lu_w1/lu_w2 (local MLP, residual-critical), ff_w1, conv diag taps.
Scale bookkeeping: wq,wk x32; wv,f2 x64; qp_all=32*Qp, kp_all=32*Kp, v_all=V,
kv_sb=KV/2, ksum_sb=Ksum/16, rep=2^16/norm, numerator PSUM=2^20*attn.
Kp/V/Qp are SBUF-resident fp8 (no HBM spill); KV+K_sum run back-to-back from
SBUF.  DVE ops are merged across channel chunks wherever gains are 1
(broadcast [P,1,*] operands), and PSUM consumers are paired into [P,2,NT]
tiles to halve instruction count.
"""

import math
import numpy as np
import ml_dtypes
from contextlib import ExitStack

import concourse.bass as bass
import concourse.bacc as bacc
import concourse.tile as tile
import concourse.mybir as mybir
from concourse.bass_utils import run_bass_kernel_spmd

F32 = mybir.dt.float32
BF16 = mybir.dt.bfloat16
FP8 = mybir.dt.float8e4
AF = mybir.ActivationFunctionType
OP = mybir.AluOpType
DR = mybir.MatmulPerfMode.DoubleRow
BF16_NP = ml_dtypes.bfloat16
FP8_NP = ml_dtypes.float8_e4m3

P = 128
D = 1024
DO = D // P  # 8 chunks of the channel dim

SQK = 32.0       # weight scale for wq/wk (keeps 32*Qp < 240 in fp8)
SW = 64.0        # weight scale for wv/f2
LN_SQK = math.log(SQK)
KS_SC = 2.0 ** -9    # ksum fp8 = (sum of 32*Kp) * 2^-9 = Ksum/16
KV_SC = 1.0 / SW     # kv fp8 = (sum 32Kp*V)/64 = KV/2
REP_SC = 2.0 ** 17   # rep = rr * 2^17 = 2^16/norm  (rr = 1/(2*norm))
NUM_SC = 2.0 ** -20  # ps_u = 2^20 * attn  ->  acc += ps_u * 2^-20

# pp param-plane indices (per-partition params, laid out [128, DO, NP])
(C0, C1, C2, CB, T0, T1, T2, TCB1, U0, U1, U2,
 TMG, TMB, N1G, N1B, N2G, N2B, LUB1, FFB1, FFB2) = range(20)
NPARAM = 20



def build_nc(N=2048, NT=512, use_bq=False, use_bk=False, use_bv=False,
             use_tmb=False, use_n1b=False, use_n2b=False, use_fb2=False,
             use_tmg=False, use_n1g=False, use_n2g=False, debug=False):
    NTILES = N // NT
    NTA = 512              # phase-A tile size (independent of C/D tiling)
    NTILES_A = N // NTA
    NCH = NTA // P         # 128-token chunks per phase-A tile
    TOTCH = N // P
    W = NT + 4             # phase-C tile width with +-2 halo
    WST = (W + 15) // 16 * 16  # fp8 stats tile width (16B-aligned)
    assert N % NT == 0 and NT % P == 0 and N % NTA == 0

    nc = bacc.Bacc(None, target_bir_lowering=False, debug=debug)

    xT_d = nc.dram_tensor("x_T", [D, N], BF16, kind="ExternalInput")
    xqT_d = nc.dram_tensor("xq_T", [D, N], FP8, kind="ExternalInput")
    w_d = {}
    for name, dt_ in (("wqT", FP8), ("wkT", FP8), ("wvT", FP8),
                      ("w1T", BF16), ("w2T", BF16), ("f1T", BF16),
                      ("f2T", FP8)):
        w_d[name] = nc.dram_tensor(name, [D, D], dt_, kind="ExternalInput")
    pp_d = nc.dram_tensor("pp", [P, DO, NPARAM], F32, kind="ExternalInput")
    diags_d = nc.dram_tensor("diags", [P, 3, DO, P], BF16, kind="ExternalInput")
    rows_d = nc.dram_tensor("rows", [1, 3 * D], FP8, kind="ExternalInput")
    yT_d = nc.dram_tensor("y_T", [D, N], F32, kind="ExternalOutput")

    acc_sp = nc.dram_tensor("acc_sp", [D, N], F32)

    xT = xT_d.rearrange("(o p) n -> p o n", p=P)
    xqT = xqT_d.rearrange("(o p) n -> p o n", p=P)
    wr = {k: v.rearrange("(o p) n -> p o n", p=P) for k, v in w_d.items()}
    acc_r = acc_sp.rearrange("(o p) n -> p o n", p=P)
    yT = yT_d.rearrange("(o p) n -> p o n", p=P)

    with tile.TileContext(nc) as tc, ExitStack() as top:
        persist = top.enter_context(tc.tile_pool(name="persist", bufs=1))
        pp = persist.tile([P, DO, NPARAM], F32)
        nc.sync.dma_start(pp, pp_d[:])
        rows = ones_row = None
        if use_bq or use_bk or use_bv:
            rows = persist.tile([1, 3 * D], FP8)
            nc.sync.dma_start(rows, rows_d[:])
            ones_row = persist.tile([1, max(NT, 512)], FP8)
            nc.vector.memset(ones_row, 1.0)
        ones_1p_bf = persist.tile([1, P], BF16)
        nc.vector.memset(ones_1p_bf, 1.0)
        ones_pair = persist.tile([P, 2, 16], FP8)
        nc.vector.memset(ones_pair, 1.0)
        ones_q = persist.tile([P, 2, P], FP8)
        nc.vector.memset(ones_q, 1.0)
        ones_one = persist.tile([1, 1], BF16)
        nc.vector.memset(ones_one, 1.0)
        ksrow_sb = persist.tile([1, D], BF16)
        onesD_bf = persist.tile([P, P], BF16)
        nc.vector.memset(onesD_bf, 1.0 / D)
        onesD_f32 = persist.tile([P, P], F32)
        nc.vector.memset(onesD_f32, 1.0 / D)
        eps_ln = persist.tile([P, 1], F32)
        nc.vector.memset(eps_ln, 1e-5)
        ln32_b = persist.tile([P, 1], F32)
        nc.vector.memset(ln32_b, LN_SQK)
        kv_sb = persist.tile([P, DO, D], FP8)
        ksum_sb = persist.tile([P, DO, 16], FP8)
        qp_all = persist.tile([P, DO, N], FP8)
        diags = persist.tile([P, 3, DO, P], BF16)
        nc.sync.dma_start(diags, diags_d[:])

        def stats_q(psum, rhs3, width):
            """Sum over channels (lhs=1.0 fp8 DoubleRow); caller scales by 1/D."""
            for c0 in range(0, width, 512):
                cw = min(512, width - c0)
                for j in range(DO // 2):
                    nc.tensor.matmul(psum[:, c0:c0 + cw], ones_q,
                                     rhs3[:, 2 * j:2 * j + 2, c0:c0 + cw],
                                     start=(j == 0), stop=(j == DO // 2 - 1),
                                     perf_mode=DR)

        def stats_bf(psum, rhs3, width):
            """Mean over channels (lhs=1/D bf16), replicated across partitions."""
            for c0 in range(0, width, 512):
                cw = min(512, width - c0)
                for kc in range(DO):
                    nc.tensor.matmul(psum[:, c0:c0 + cw], onesD_bf,
                                     rhs3[:, kc, c0:c0 + cw],
                                     start=(kc == 0), stop=(kc == DO - 1))

        F32R = mybir.dt.float32r

        # ---------------- Phases A+B scope ----------------
        with ExitStack() as phab:
            kvpool = phab.enter_context(tc.tile_pool(name="kvres", bufs=1))
            kp_all = kvpool.tile([P, TOTCH, D], FP8, tag="kp")
            v_all = kvpool.tile([P, TOTCH, D], FP8, tag="v")

            # ---------------- Phase A: QKV ----------------
            with ExitStack() as ph:
                wpool = ph.enter_context(tc.tile_pool(name="wA", bufs=1))
                wq_sb = wpool.tile([P, DO, D], FP8, tag="wq")
                nc.sync.dma_start(wq_sb, wr["wqT"])
                wk_sb = wpool.tile([P, DO, D], FP8, tag="wk")
                nc.sync.dma_start(wk_sb, wr["wkT"])
                wv_sb = wpool.tile([P, DO, D], FP8, tag="wv")
                nc.sync.dma_start(wv_sb, wr["wvT"])
                io = ph.enter_context(tc.tile_pool(name="ioA", bufs=2))
                ev = ph.enter_context(tc.tile_pool(name="evA", bufs=3))
                ps = ph.enter_context(tc.tile_pool(name="psA", bufs=3, space="PSUM"))

                for it in range(NTILES_A):
                    n0 = it * NTA
                    x_t = io.tile([P, DO, NTA], FP8, tag="xA")
                    nc.sync.dma_start(x_t, xqT[:, :, n0:n0 + NTA])

                    # ---- Q: layout B, out [dout-pair, n]; psum [P, 2*NT] flat
                    for dc0 in range(0, DO, 2):
                        ps_q = ps.tile([P, 2 * NTA], F32, tag="psA")
                        for h in range(2):
                            dc = dc0 + h
                            for j in range(DO // 2):
                                nc.tensor.matmul(
                                    ps_q[:, h * NTA:(h + 1) * NTA],
                                    wq_sb[:, 2 * j:2 * j + 2, dc * P:(dc + 1) * P],
                                    x_t[:, 2 * j:2 * j + 2, :],
                                    start=(j == 0),
                                    stop=(j == DO // 2 - 1 and not use_bq),
                                    perf_mode=DR)
                            if use_bq:
                                nc.tensor.matmul(ps_q[:, h * NTA:(h + 1) * NTA],
                                                 rows[0:1, dc * P:(dc + 1) * P],
                                                 ones_row[0:1, 0:NTA],
                                                 start=False, stop=True)
                        m_t = ev.tile([P, 2 * NTA], BF16, tag="mA")
                        # relu(-q) = -min(q,0) on ACT (frees a DVE psum read)
                        nc.scalar.activation(m_t, ps_q, AF.Relu,
                                             scale=-1.0 / SQK)
                        e_t = ev.tile([P, 2 * NTA], BF16, tag="eA")
                        # 32*exp(min(q,0)) = exp(-relu(-q) + ln32)
                        nc.scalar.activation(e_t, m_t, AF.Exp, scale=-1.0,
                                             bias=ln32_b[:, 0:1])
                        # qp = max(32q,0) + 32*exp(min(q,0)) = 32*(elu(q)+1)
                        nc.vector.scalar_tensor_tensor(
                            qp_all[:, dc0:dc0 + 2, n0:n0 + NTA], ps_q, 0.0, e_t,
                            OP.max, OP.add)

                    # ---- K, V: layout A, out [token-chunk, 1024 douts] ----
                    for ch in range(NCH):
                        c = it * NCH + ch
                        cs = slice(ch * P, (ch + 1) * P)
                        ps_k = ps.tile([P, D], F32, tag="psA")
                        for h in range(2):
                            hs = slice(h * 512, (h + 1) * 512)
                            for j in range(DO // 2):
                                nc.tensor.matmul(
                                    ps_k[:, hs],
                                    x_t[:, 2 * j:2 * j + 2, cs],
                                    wk_sb[:, 2 * j:2 * j + 2, hs],
                                    start=(j == 0),
                                    stop=(j == DO // 2 - 1 and not use_bk),
                                    perf_mode=DR)
                            if use_bk:
                                nc.tensor.matmul(ps_k[:, hs], ones_pair[:, 0, 0:1],
                                                 rows[0:1, D + h * 512:D + (h + 1) * 512],
                                                 start=False, stop=True)
                        m2 = ev.tile([P, D], BF16, tag="mA2")
                        nc.scalar.activation(m2, ps_k, AF.Relu,
                                             scale=-1.0 / SQK)
                        e2 = ev.tile([P, D], BF16, tag="eA2")
                        nc.scalar.activation(e2, m2, AF.Exp, scale=-1.0,
                                             bias=ln32_b[:, 0:1])
                        nc.vector.scalar_tensor_tensor(kp_all[:, c, :], ps_k, 0.0,
                                                       e2, OP.max, OP.add)

                        ps_v = ps.tile([P, D], F32, tag="psA")
                        for h in range(2):
                            hs = slice(h * 512, (h + 1) * 512)
                            for j in range(DO // 2):
                                nc.tensor.matmul(
                                    ps_v[:, hs],
                                    x_t[:, 2 * j:2 * j + 2, cs],
                                    wv_sb[:, 2 * j:2 * j + 2, hs],
                                    start=(j == 0),
                                    stop=(j == DO // 2 - 1 and not use_bv),
                                    perf_mode=DR)
                            if use_bv:
                                nc.tensor.matmul(ps_v[:, hs], ones_pair[:, 0, 0:1],
                                                 rows[0:1, 2 * D + h * 512:2 * D + (h + 1) * 512],
                                                 start=False, stop=True)
                        nc.scalar.activation(v_all[:, c, :], ps_v, AF.Copy,
                                             scale=1.0 / SW)

            # ---------------- Phase B: KV accumulation + K_sum ----------------
            with ExitStack() as ph:
                ps = ph.enter_context(tc.tile_pool(name="psB", bufs=1, space="PSUM"))
                for dcg in range(2):
                    kv_ps = [ps.tile([P, 2, 512], F32, tag=f"kvps{d}",
                                     name=f"kvps{dcg}_{d}")
                             for d in range(4)]
                    for cp in range(TOTCH // 2):
                        for di in range(4):
                            dc = dcg * 4 + di
                            for eh in range(2):
                                nc.tensor.matmul(
                                    kv_ps[di][:, eh, :],
                                    kp_all[:, 2 * cp:2 * cp + 2, dc * P:(dc + 1) * P],
                                    v_all[:, 2 * cp:2 * cp + 2, eh * 512:(eh + 1) * 512],
                                    start=(cp == 0), stop=(cp == TOTCH // 2 - 1),
                                    perf_mode=DR)
                    for di in range(4):
                        dc = dcg * 4 + di
                        nc.scalar.activation(kv_sb[:, dc, :], kv_ps[di], AF.Copy,
                                             scale=KV_SC)
            with ExitStack() as ph:
                ksp = ph.enter_context(tc.tile_pool(name="ksB", bufs=1, space="PSUM"))
                ps_ks = ksp.tile([1, D], F32, tag="ksrow")
                for cp in range(TOTCH // 2):
                    for h in range(2):
                        hs = slice(h * 512, (h + 1) * 512)
                        nc.tensor.matmul(ps_ks[0:1, hs], ones_pair[:, :, 0:1],
                                         kp_all[:, 2 * cp:2 * cp + 2, hs],
                                         start=(cp == 0), stop=(cp == TOTCH // 2 - 1),
                                         perf_mode=DR)
                nc.scalar.activation(ksrow_sb, ps_ks[0:1, :], AF.Copy)
                # transpose K_sum row -> per-partition column layout [P, DO]
                ps_ksc = ksp.tile([P, DO], F32, tag="kscol")
                for dc in range(DO):
                    nc.tensor.matmul(ps_ksc[:, dc:dc + 1],
                                     ksrow_sb[0:1, dc * P:(dc + 1) * P],
                                     ones_one[0:1, 0:1], start=True, stop=True)
                nc.scalar.activation(ksum_sb[:, :, 0], ps_ksc, AF.Copy,
                                     scale=KS_SC)

        # ---------------- Phases C+D (shared FFN-weight prefetch) ----------------
        cd = top.enter_context(ExitStack())
        wpoolD = cd.enter_context(tc.tile_pool(name="wDpre", bufs=1))
        f1_sb = wpoolD.tile([P, DO, D], BF16, tag="f1")
        nc.sync.dma_start(f1_sb, wr["f1T"])
        f2_sb = wpoolD.tile([P, DO, D], FP8, tag="f2")
        nc.sync.dma_start(f2_sb, wr["f2T"])

        # ---------------- Phase C: conv'' + local MLP + token mixer ----------------
        with ExitStack() as ph:
            wpool = ph.enter_context(tc.tile_pool(name="wC", bufs=1))
            w1_sb = wpool.tile([P, DO, D], BF16, tag="w1")
            nc.sync.dma_start(w1_sb, wr["w1T"])
            w2_sb = wpool.tile([P, DO, D], BF16, tag="w2")
            nc.sync.dma_start(w2_sb, wr["w2T"])
            io = ph.enter_context(tc.tile_pool(name="ioC", bufs=2))
            pipe = ph.enter_context(tc.tile_pool(name="pipeC", bufs=2))
            mid = ph.enter_context(tc.tile_pool(name="midC", bufs=1))
            sm = ph.enter_context(tc.tile_pool(name="smC", bufs=1))
            ps = ph.enter_context(tc.tile_pool(name="psC", bufs=2, space="PSUM"))
            pst = ph.enter_context(tc.tile_pool(name="pstC", bufs=1, space="PSUM"))

            def c_front(it):
                n0 = it * NT
                x_t = io.tile([P, DO, W], BF16, tag="xC", name=f"x_{it}")
                xq_t = io.tile([P, DO, WST], FP8, tag="xqC", name=f"xq_{it}")
                lo, hi = n0 - 2, n0 + NT + 2
                if lo < 0:
                    nc.vector.memset(x_t[:, :, 0:2], 0.0)
                    nc.sync.dma_start(x_t[:, :, 2:W], xT[:, :, 0:hi])
                    nc.vector.memset(xq_t[:, :, 0:2], 0.0)
                    nc.sync.dma_start(xq_t[:, :, 2:W], xqT[:, :, 0:hi])
                elif hi > N:
                    nc.vector.memset(x_t[:, :, W - 2:W], 0.0)
                    nc.sync.dma_start(x_t[:, :, 0:W - 2], xT[:, :, lo:N])
                    nc.vector.memset(xq_t[:, :, W - 2:W], 0.0)
                    nc.sync.dma_start(xq_t[:, :, 0:W - 2], xqT[:, :, lo:N])
                else:
                    nc.sync.dma_start(x_t, xT[:, :, lo:hi])
                    nc.sync.dma_start(xq_t[:, :, 0:W], xqT[:, :, lo:hi])

                acc = io.tile([P, DO, NT], F32, tag="accC", name=f"acc_{it}")
                # diffusion dwconv'': center tap on ACT, side taps on DVE
                for o in range(DO):
                    nc.scalar.activation(acc[:, o, :], x_t[:, o, 2:NT + 2],
                                         AF.Identity, bias=pp[:, o, CB:CB + 1],
                                         scale=pp[:, o, C1:C1 + 1])
                for o in range(DO):
                    nc.vector.scalar_tensor_tensor(acc[:, o, :], x_t[:, o, 1:NT + 1],
                                                   pp[:, o, C0:C0 + 1], acc[:, o, :],
                                                   OP.mult, OP.add)
                for o in range(DO):
                    nc.vector.scalar_tensor_tensor(acc[:, o, :], x_t[:, o, 3:NT + 3],
                                                   pp[:, o, C2:C2 + 1], acc[:, o, :],
                                                   OP.mult, OP.add)

                # local MLP first half (bf16, paired dc groups)
                h1_t = pipe.tile([P, DO, NT], BF16, tag="h1", name=f"h1_{it}")
                for dc0 in range(0, DO, 2):
                    ps_h = ps.tile([P, 2, NT], F32, tag="psC",
                                   name=f"psh1_{it}_{dc0}")
                    for h in range(2):
                        dc = dc0 + h
                        for kc in range(DO):
                            nc.tensor.matmul(ps_h[:, h, :],
                                             w1_sb[:, kc, dc * P:(dc + 1) * P],
                                             x_t[:, kc, 2:NT + 2],
                                             start=(kc == 0), stop=(kc == DO - 1))
                    for h in range(2):
                        nc.scalar.activation(h1_t[:, dc0 + h, :], ps_h[:, h, :],
                                             AF.Gelu,
                                             bias=pp[:, dc0 + h, LUB1:LUB1 + 1])

                # token mixer LN stats (fp8 sum-matmuls on xq/sq)
                sq_t = mid.tile([P, DO, WST], FP8, tag="sqC", name=f"sq_{it}")
                nc.scalar.activation(sq_t[:, :, 0:W], x_t, AF.Square)
                ps_m = pst.tile([P, W], F32, tag="psm", name=f"psm_{it}")
                stats_q(ps_m, xq_t, W)
                ps_s = pst.tile([P, W], F32, tag="pss", name=f"pss_{it}")
                stats_q(ps_s, sq_t, W)
                m_sb = sm.tile([P, 1, W], BF16, tag="msb", name=f"msb_{it}")
                nc.scalar.activation(m_sb[:, 0, :], ps_m, AF.Copy, scale=1.0 / D)
                var = sm.tile([P, 1, W], F32, tag="var", name=f"var_{it}")
                nc.scalar.activation(var[:, 0, :], ps_m, AF.Square, scale=1.0 / D)
                nc.vector.scalar_tensor_tensor(var[:, 0, :], ps_s, 1.0 / D,
                                               var[:, 0, :], OP.mult, OP.subtract)
                nc.scalar.activation(var[:, 0, :], var[:, 0, :], AF.Sqrt,
                                     bias=eps_ln[:, 0:1])
                nc.vector.reciprocal_approx_fast(out=var[:, 0, :], in_=var[:, 0, :])
                rstd = sm.tile([P, 1, W], BF16, tag="rstd", name=f"rstd_{it}")
                nc.vector.tensor_copy(rstd, var)
                u_t = mid.tile([P, DO, W], BF16, tag="tokA", name=f"u_{it}")
                nc.vector.tensor_sub(u_t, x_t, m_sb.to_broadcast((P, DO, W)))
                xm_t = mid.tile([P, DO, W], BF16, tag="tokC", name=f"xm_{it}")
                if use_tmg:
                    for o in range(DO):
                        nc.vector.scalar_tensor_tensor(
                            xm_t[:, o, :], u_t[:, o, :], pp[:, o, TMG:TMG + 1],
                            rstd[:, 0, :], OP.mult, OP.mult)
                else:
                    nc.vector.tensor_mul(xm_t, u_t, rstd.to_broadcast((P, DO, W)))
                if use_tmb:
                    for o in range(DO):
                        nc.vector.tensor_scalar_add(xm_t[:, o, :], xm_t[:, o, :],
                                                    pp[:, o, TMB:TMB + 1])
                # conv1: t_s[k] = conv1(xm)[k+1], k in [0, W-2)
                # (reuses u_t's buffer — u is dead once xm is computed)
                t_full = mid.tile([P, DO, W], BF16, tag="tokA", name=f"t_{it}")
                t_t = t_full[:, :, 0:W - 2]
                for o in range(DO):
                    nc.scalar.activation(t_t[:, o, :], xm_t[:, o, 1:W - 1],
                                         AF.Identity, bias=pp[:, o, TCB1:TCB1 + 1],
                                         scale=pp[:, o, T1:T1 + 1])
                for o in range(DO):
                    nc.vector.scalar_tensor_tensor(t_t[:, o, :], xm_t[:, o, 0:W - 2],
                                                   pp[:, o, T0:T0 + 1],
                                                   t_t[:, o, :], OP.mult, OP.add)
                for o in range(DO):
                    nc.vector.scalar_tensor_tensor(t_t[:, o, :], xm_t[:, o, 2:W],
                                                   pp[:, o, T2:T2 + 1],
                                                   t_t[:, o, :], OP.mult, OP.add)
                t2_t = pipe.tile([P, DO, W - 2], BF16, tag="t2", name=f"t2_{it}")
                nc.scalar.activation(t2_t, t_t, AF.Gelu)
                if it == 0:
                    nc.vector.memset(t2_t[:, :, 0:1], 0.0)
                if it == NTILES - 1:
                    nc.vector.memset(t2_t[:, :, W - 3:W - 2], 0.0)
                return x_t, acc, h1_t, t2_t

            def c_back(it, tiles):
                n0 = it * NT
                x_t, acc, h1_t, t2_t = tiles
                for dc0 in range(0, DO, 2):
                    ps_h = ps.tile([P, 2, NT], F32, tag="psC",
                                   name=f"psh2_{it}_{dc0}")
                    for h in range(2):
                        dc = dc0 + h
                        for kc in range(DO):
                            nc.tensor.matmul(ps_h[:, h, :],
                                             w2_sb[:, kc, dc * P:(dc + 1) * P],
                                             h1_t[:, kc, :],
                                             start=(kc == 0), stop=False)
                        for tap in range(3):
                            nc.tensor.matmul(ps_h[:, h, :], diags[:, tap, dc, :],
                                             t2_t[:, dc, tap:NT + tap],
                                             start=False, stop=(tap == 2))
                    nc.vector.tensor_add(acc[:, dc0:dc0 + 2, :],
                                         acc[:, dc0:dc0 + 2, :], ps_h)
                nc.sync.dma_start(acc_r[:, :, n0:n0 + NT], acc)

            pend = {0: c_front(0)}
            for it in range(NTILES):
                if it + 1 < NTILES:
                    pend[it + 1] = c_front(it + 1)
                c_back(it, pend.pop(it))

        # ---------------- Phase D: attention + LN1 + FFN + LN2 ----------------
        with ExitStack() as ph:
            io = ph.enter_context(tc.tile_pool(name="ioD", bufs=2))
            mid = ph.enter_context(tc.tile_pool(name="midD", bufs=2))
            sm = ph.enter_context(tc.tile_pool(name="smD", bufs=2))
            ps = ph.enter_context(tc.tile_pool(name="psD", bufs=2, space="PSUM"))
            pst = ph.enter_context(tc.tile_pool(name="pstD", bufs=1, space="PSUM"))

            def d_front_a(it):
                """loads, norm row, rep fold, numerator halves 0-3."""
                n0 = it * NT
                acc_t = io.tile([P, DO, NT], F32, tag="accD", name=f"accD_{it}")
                nc.sync.dma_start(acc_t, acc_r[:, :, n0:n0 + NT])
                ps_n = pst.tile([P, NT], F32, tag="psrep", name=f"psn_{it}")
                for j in range(DO // 2):
                    nc.tensor.matmul(ps_n[0:1, :], ksum_sb[:, 2 * j:2 * j + 2, 0:1],
                                     qp_all[:, 2 * j:2 * j + 2, n0:n0 + NT],
                                     start=(j == 0), stop=(j == DO // 2 - 1),
                                     perf_mode=DR)
                rr32 = sm.tile([1, NT], F32, tag="rr32D", name=f"rr32_{it}")
                nc.vector.reciprocal_approx_fast(out=rr32, in_=ps_n[0:1, :])
                rr = sm.tile([1, NT], BF16, tag="rrD", name=f"rr_{it}")
                nc.vector.tensor_copy(rr, rr32)
                ps_rep = pst.tile([P, NT], F32, tag="psrep", name=f"psrep_{it}")
                nc.tensor.matmul(ps_rep, ones_1p_bf[0:1, :], rr, start=True,
                                 stop=True)
                rep_sb = mid.tile([P, 1, NT], BF16, tag="repsb", name=f"rep_{it}")
                nc.scalar.activation(rep_sb[:, 0, :], ps_rep, AF.Copy,
                                     scale=REP_SC)
                qp2_t = mid.tile([P, DO, NT], FP8, tag="qp2", name=f"qp2_{it}")
                nc.vector.tensor_mul(qp2_t, qp_all[:, :, n0:n0 + NT],
                                     rep_sb.to_broadcast((P, DO, NT)))
                for ec0 in range(0, DO // 2, 2):
                    ps_u = ps.tile([P, 2, NT], F32, tag="psD",
                                   name=f"psnum_{it}_{ec0}")
                    for h in range(2):
                        ec = ec0 + h
                        for j in range(DO // 2):
                            nc.tensor.matmul(
                                ps_u[:, h, :],
                                kv_sb[:, 2 * j:2 * j + 2, ec * P:(ec + 1) * P],
                                qp2_t[:, 2 * j:2 * j + 2, :],
                                start=(j == 0), stop=(j == DO // 2 - 1),
                                perf_mode=DR)
                    nc.vector.scalar_tensor_tensor(acc_t[:, ec0:ec0 + 2, :], ps_u,
                                                   NUM_SC, acc_t[:, ec0:ec0 + 2, :],
                                                   OP.mult, OP.add)
                return qp2_t, acc_t

            def d_front_b(it, T):
                qp2_t, acc_t = T
                for ec0 in range(DO // 2, DO, 2):
                    ps_u = ps.tile([P, 2, NT], F32, tag="psD",
                                   name=f"psnum_{it}_{ec0}")
                    for h in range(2):
                        ec = ec0 + h
                        for j in range(DO // 2):
                            nc.tensor.matmul(
                                ps_u[:, h, :],
                                kv_sb[:, 2 * j:2 * j + 2, ec * P:(ec + 1) * P],
                                qp2_t[:, 2 * j:2 * j + 2, :],
                                start=(j == 0), stop=(j == DO // 2 - 1),
                                perf_mode=DR)
                    nc.vector.scalar_tensor_tensor(acc_t[:, ec0:ec0 + 2, :], ps_u,
                                                   NUM_SC, acc_t[:, ec0:ec0 + 2, :],
                                                   OP.mult, OP.add)
                return acc_t

            def d_mid(it, acc_t):
                """LN1: mean via f32r matmuls on acc, sq-stats bf16; in-place
                apply -> y1 (bf16, two halves so f1 can start early)."""
                accq = mid.tile([P, DO, NT], FP8, tag="accq", name=f"accq_{it}", bufs=1)
                nc.scalar.activation(accq, acc_t, AF.Copy)
                sqq = mid.tile([P, DO, NT], FP8, tag="sqD", name=f"sqD_{it}", bufs=1)
                nc.scalar.activation(sqq, acc_t, AF.Square)
                psst = pst.tile([P, 2, NT], F32, tag="psst", name=f"psst1_{it}")
                stats_q(psst[:, 0, :], accq, NT)
                stats_q(psst[:, 1, :], sqq, NT)
                m1_sb = sm.tile([P, 1, NT], F32, tag="m1sb", name=f"m1_{it}")
                nc.scalar.activation(m1_sb[:, 0, :], psst[:, 0, :], AF.Copy,
                                     scale=1.0 / D)
                var1 = sm.tile([P, 1, NT], F32, tag="varD", name=f"var1_{it}")
                nc.scalar.activation(var1[:, 0, :], psst[:, 0, :], AF.Square,
                                     scale=1.0 / D)
                nc.vector.scalar_tensor_tensor(var1[:, 0, :], psst[:, 1, :],
                                               1.0 / D, var1[:, 0, :],
                                               OP.mult, OP.subtract)
                nc.scalar.activation(var1[:, 0, :], var1[:, 0, :], AF.Sqrt,
                                     bias=eps_ln[:, 0:1])
                nc.vector.reciprocal_approx_fast(out=var1[:, 0, :],
                                                 in_=var1[:, 0, :])
                # u1 in place of acc (acc is dead after the stats)
                nc.vector.tensor_sub(acc_t, acc_t, m1_sb.to_broadcast((P, DO, NT)))
                y1_t = mid.tile([P, DO, NT], BF16, tag="y1", name=f"y1_{it}")
                if use_n1g:
                    for o in range(DO):
                        nc.vector.scalar_tensor_tensor(
                            y1_t[:, o, :], acc_t[:, o, :], pp[:, o, N1G:N1G + 1],
                            var1[:, 0, :], OP.mult, OP.mult)
                else:
                    for hh in range(0, DO, 4):
                        nc.vector.tensor_mul(
                            y1_t[:, hh:hh + 4, :], acc_t[:, hh:hh + 4, :],
                            var1.to_broadcast((P, 4, NT)))
                if use_n1b:
                    for o in range(DO):
                        nc.vector.tensor_scalar_add(y1_t[:, o, :], y1_t[:, o, :],
                                                    pp[:, o, N1B:N1B + 1])
                return y1_t

            def d_f1(it, y1_t, tail):
                """f1 matmul groups with the previous tile's LN2-apply (yo)
                DVE ops interleaved under the PE-heavy stretch."""
                f1h_t = mid.tile([P, DO, NT], FP8, tag="f1h", name=f"f1h_{it}")
                for gi, dc0 in enumerate(range(0, DO, 2)):
                    ps_f = ps.tile([P, 2, NT], F32, tag="psD",
                                   name=f"psf1_{it}_{dc0}")
                    for h in range(2):
                        dc = dc0 + h
                        for kc in range(DO):
                            nc.tensor.matmul(ps_f[:, h, :],
                                             f1_sb[:, kc, dc * P:(dc + 1) * P],
                                             y1_t[:, kc, :],
                                             start=(kc == 0), stop=(kc == DO - 1))
                    if gi < len(tail):
                        tail[gi]()
                    for h in range(2):
                        nc.scalar.activation(f1h_t[:, dc0 + h, :], ps_f[:, h, :],
                                             AF.Gelu,
                                             bias=pp[:, dc0 + h, FFB1:FFB1 + 1])
                for fn in tail[len(list(range(0, DO, 2))):]:
                    fn()
                return f1h_t

            def d_f2(it, y1_t, f1h_t):
                y2_t = mid.tile([P, DO, NT], BF16, tag="y2", name=f"y2_{it}")
                for dc0 in range(0, DO, 2):
                    ps_f = ps.tile([P, 2, NT], F32, tag="psD",
                                   name=f"psf2_{it}_{dc0}")
                    for h in range(2):
                        dc = dc0 + h
                        for j in range(DO // 2):
                            nc.tensor.matmul(
                                ps_f[:, h, :],
                                f2_sb[:, 2 * j:2 * j + 2, dc * P:(dc + 1) * P],
                                f1h_t[:, 2 * j:2 * j + 2, :],
                                start=(j == 0), stop=(j == DO // 2 - 1),
                                perf_mode=DR)
                    # y2 = y1 + psum/64
                    nc.vector.scalar_tensor_tensor(y2_t[:, dc0:dc0 + 2, :], ps_f,
                                                   1.0 / SW,
                                                   y1_t[:, dc0:dc0 + 2, :],
                                                   OP.mult, OP.add)
                if use_fb2:
                    for dc in range(DO):
                        nc.vector.tensor_scalar_add(y2_t[:, dc, :], y2_t[:, dc, :],
                                                    pp[:, dc, FFB2:FFB2 + 1])
                return y2_t

            def d_back_head(it, y2_t):
                sq2_t = mid.tile([P, DO, NT], BF16, tag="sq2", name=f"sq2_{it}", bufs=1)
                nc.scalar.activation(sq2_t, y2_t, AF.Square)
                psst = pst.tile([P, 2, NT], F32, tag="psst", name=f"psst2_{it}")
                stats_bf(psst[:, 0, :], y2_t, NT)
                stats_bf(psst[:, 1, :], sq2_t, NT)
                m2_sb = sm.tile([P, 1, NT], F32, tag="m2sb", name=f"m2_{it}")
                nc.scalar.activation(m2_sb[:, 0, :], psst[:, 0, :], AF.Copy)
                var2 = sm.tile([P, 1, NT], F32, tag="var2D", name=f"var2_{it}")
                nc.scalar.activation(var2[:, 0, :], psst[:, 0, :], AF.Square)
                nc.vector.tensor_sub(var2[:, 0, :], psst[:, 1, :], var2[:, 0, :])
                nc.scalar.activation(var2[:, 0, :], var2[:, 0, :], AF.Sqrt,
                                     bias=eps_ln[:, 0:1])
                nc.vector.reciprocal_approx_fast(out=var2[:, 0, :],
                                                 in_=var2[:, 0, :])
                return y2_t, m2_sb, var2

            def d_back_tail(it, state):
                """Returns closures: yo half-ops + output DMA, to be issued
                under the next tile's f1 matmul groups."""
                y2_t, m2_sb, var2 = state
                n0 = it * NT
                yo_t = mid.tile([P, DO, NT], F32, tag="yo", name=f"yo_{it}",
                                bufs=1)
                fns = []
                if use_n2g:
                    def sub_all():
                        nc.vector.tensor_sub(yo_t, y2_t,
                                             m2_sb.to_broadcast((P, DO, NT)))
                        for o in range(DO):
                            nc.vector.scalar_tensor_tensor(
                                yo_t[:, o, :], yo_t[:, o, :],
                                pp[:, o, N2G:N2G + 1], var2[:, 0, :],
                                OP.mult, OP.mult)
                        if use_n2b:
                            for o in range(DO):
                                nc.vector.tensor_scalar_add(
                                    yo_t[:, o, :], yo_t[:, o, :],
                                    pp[:, o, N2B:N2B + 1])
                        nc.sync.dma_start(yT[:, :, n0:n0 + NT], yo_t)
                    return [sub_all]
                for hh in range(0, DO, 4):
                    def half(hh=hh):
                        sl = slice(hh, hh + 4)
                        nc.vector.tensor_sub(yo_t[:, sl, :], y2_t[:, sl, :],
                                             m2_sb.to_broadcast((P, 4, NT)))
                        nc.vector.tensor_mul(yo_t[:, sl, :], yo_t[:, sl, :],
                                             var2.to_broadcast((P, 4, NT)))
                        nc.sync.dma_start(yT[:, hh:hh + 4, n0:n0 + NT],
                                          yo_t[:, sl, :])
                    fns.append(half)
                return fns

            Tcur = d_front_a(0)
            acc_cur = d_front_b(0, Tcur)
            tail = []
            for it in range(NTILES):
                y1_cur = d_mid(it, acc_cur)
                if it + 1 < NTILES:
                    Tnext = d_front_a(it + 1)
                f1h_cur = d_f1(it, y1_cur, tail)
                if it + 1 < NTILES:
                    acc_cur = d_front_b(it + 1, Tnext)
                y2_cur = d_f2(it, y1_cur, f1h_cur)
                state = d_back_head(it, y2_cur)
                tail = d_back_tail(it, state)
            for fn in tail:
                fn()
    nc.compile()
    return nc


def make_in_maps(inputs, n_cores=8):
    """Host-side preprocessing: fold constants, transpose, cast, shard."""
    x = np.asarray(inputs["x"], np.float32)
    B, N, D_ = x.shape
    dt = float(np.asarray(inputs["delta_t"]))

    def g(k):
        return np.asarray(inputs[k], np.float32)

    diff_w, diff_b = g("diff_w"), g("diff_b")
    tm_w1, tm_cb1 = g("tm_w1"), g("tm_cb1")
    tm_w2, tm_cb2 = g("tm_w2"), g("tm_cb2")

    pp = np.zeros((P, DO, NPARAM), np.float32)

    def put(i, v):
        pp[:, :, i] = v.reshape(DO, P).T

    put(C0, dt * diff_w[:, 0, 0])
    put(C1, dt * diff_w[:, 0, 1] + (1.0 - dt))
    put(C2, dt * diff_w[:, 0, 2])
    put(CB, dt * diff_b + g("lu_b2") + tm_cb2)
    put(T0, tm_w1[:, 0, 0])
    put(T1, tm_w1[:, 0, 1])
    put(T2, tm_w1[:, 0, 2])
    put(TCB1, tm_cb1)
    put(U0, tm_w2[:, 0, 0])
    put(U1, tm_w2[:, 0, 1])
    put(U2, tm_w2[:, 0, 2])
    put(TMG, g("tm_g"))
    put(TMB, g("tm_beta"))
    put(N1G, g("n1_g"))
    put(N1B, g("n1_b"))
    put(N2G, g("n2_g"))
    put(N2B, g("n2_b"))
    put(LUB1, g("lu_b1"))
    put(FFB1, g("ff_b1"))
    put(FFB2, g("ff_b2"))

    diags = np.zeros((P, 3, DO, P), np.float32)
    idx = np.arange(P)
    for tap in range(3):
        for dc in range(DO):
            diags[idx, tap, dc, idx] = tm_w2[dc * P + idx, 0, tap]
    diags = diags.astype(BF16_NP)

    rows = np.zeros((1, 3 * D), np.float32)
    rows[0, 0:D] = SQK * g("bq")
    rows[0, D:2 * D] = SQK * g("bk")
    rows[0, 2 * D:3 * D] = SW * g("bv")
    rows = np.clip(rows, -240, 240).astype(FP8_NP)

    wt = {}
    for name, key, sc in (("wqT", "wq", SQK), ("wkT", "wk", SQK),
                          ("wvT", "wv", SW), ("f2T", "ff_w2", SW)):
        wt[name] = np.clip(
            np.ascontiguousarray(g(key).T) * sc, -240, 240).astype(FP8_NP)
    for name, key in (("w1T", "lu_w1"), ("w2T", "lu_w2"), ("f1T", "ff_w1")):
        wt[name] = np.ascontiguousarray(g(key).T).astype(BF16_NP)

    xt_f = np.ascontiguousarray(x.transpose(0, 2, 1))
    xT = xt_f.astype(BF16_NP)
    xqT = np.clip(xt_f, -240, 240).astype(FP8_NP)

    ones = np.ones((D_,), np.float32)
    flags = dict(
        use_bq=bool(np.any(g("bq"))),
        use_bk=bool(np.any(g("bk"))),
        use_bv=bool(np.any(g("bv"))),
        use_tmb=bool(np.any(g("tm_beta"))),
        use_n1b=bool(np.any(g("n1_b"))),
        use_n2b=bool(np.any(g("n2_b"))),
        use_fb2=bool(np.any(g("ff_b2"))),
        use_tmg=bool(np.any(g("tm_g") != ones)),
        use_n1g=bool(np.any(g("n1_g") != ones)),
        use_n2g=bool(np.any(g("n2_g") != ones)),
    )

    shared = {**wt, "pp": pp, "rows": rows, "diags": diags}
    in_maps = [{**shared, "x_T": xT[b], "xq_T": xqT[b]} for b in range(n_cores)]
    return in_maps, flags, (B, N)


_NC_CACHE = {}


def kernel(**inputs):
    in_maps, flags, (B, N) = make_in_maps(inputs)
    key = (N, tuple(sorted(flags.items())))
    if key not in _NC_CACHE:
        _NC_CACHE[key] = build_nc(N=N, NT=512, **flags)
    nc = _NC_CACHE[key]
    res = run_bass_kernel_spmd(nc, in_maps, list(range(B)))
    y = np.stack([res.results[b]["y_T"] for b in range(B)])
    return np.ascontiguousarray(y.transpose(0, 2, 1)).astype(np.float32)


# revision 19
# speedup vs baseline: 1.1116x; 1.0053x over previous
"""Trainium2 Bass kernel for nn_DiffuRNNLayer (B=8, N=2048, D=1024).

Sharding: data-parallel over batch — one batch element per NeuronCore (8 cores).
v3: mixed fp8-e4m3 DoubleRow / bf16 matmuls, chosen per-path by error budget:
  fp8 DR: wq/wk/wv (phase A), KV (B), f2+f1h (D), numerator+norm (D),
          LN-mean matmuls for tokenmixer-LN/LN1/LN2.
  bf16:   lu_w1/lu_w2 (local MLP, residual-critical), ff_w1, conv diag taps.
Scale bookkeeping: wq,wk x32; wv,f2 x64; qp_all=32*Qp, kp_all=32*Kp, v_all=V,
kv_sb=KV/2, ksum_sb=Ksum/16, rep=2^16/norm, numerator PSUM=2^20*attn.
Kp/V/Qp are SBUF-resident fp8 (no HBM spill); KV+K_sum run back-to-back from
SBUF.  DVE ops are merged across channel chunks wherever gains are 1
(broadcast [P,1,*] operands), and PSUM consumers are paired into [P,2,NT]
tiles to halve instruction count.
"""

import math
import numpy as np
import ml_dtypes
from contextlib import ExitStack

import concourse.bass as bass
import concourse.bacc as bacc
import concourse.tile as tile
import concourse.mybir as mybir
from concourse.bass_utils import run_bass_kernel_spmd

F32 = mybir.dt.float32
BF16 = mybir.dt.bfloat16
FP8 = mybir.dt.float8e4
AF = mybir.ActivationFunctionType
OP = mybir.AluOpType
DR = mybir.MatmulPerfMode.DoubleRow
BF16_NP = ml_dtypes.bfloat16
FP8_NP = ml_dtypes.float8_e4m3

P = 128
D = 1024
DO = D // P  # 8 chunks of the channel dim

SQK = 32.0       # weight scale for wq/wk (keeps 32*Qp < 240 in fp8)
SW = 64.0        # weight scale for wv/f2
LN_SQK = math.log(SQK)
KS_SC = 2.0 ** -9    # ksum fp8 = (sum of 32*Kp) * 2^-9 = Ksum/16
KV_SC = 1.0 / SW     # kv fp8 = (sum 32Kp*V)/64 = KV/2
REP_SC = 2.0 ** 17   # rep = rr * 2^17 = 2^16/norm  (rr = 1/(2*norm))
NUM_SC = 2.0 ** -20  # ps_u = 2^20 * attn  ->  acc += ps_u * 2^-20

# pp param-plane indices (per-partition params, laid out [128, DO, NP])
(C0, C1, C2, CB, T0, T1, T2, TCB1, U0, U1, U2,
 TMG, TMB, N1G, N1B, N2G, N2B, LUB1, FFB1, FFB2) = range(20)
NPARAM = 20



def build_nc(N=2048, NT=512, use_bq=False, use_bk=False, use_bv=False,
             use_tmb=False, use_n1b=False, use_n2b=False, use_fb2=False,
             use_tmg=False, use_n1g=False, use_n2g=False, debug=False):
    NTILES = N // NT
    NTA = 512              # phase-A tile size (independent of C/D tiling)
    NTILES_A = N // NTA
    NCH = NTA // P         # 128-token chunks per phase-A tile
    TOTCH = N // P
    W = NT + 4             # phase-C tile width with +-2 halo
    WST = (W + 15) // 16 * 16  # fp8 stats tile width (16B-aligned)
    assert N % NT == 0 and NT % P == 0 and N % NTA == 0

    nc = bacc.Bacc(None, target_bir_lowering=False, debug=debug)

    xT_d = nc.dram_tensor("x_T", [D, N], BF16, kind="ExternalInput")
    xqT_d = nc.dram_tensor("xq_T", [D, N], FP8, kind="ExternalInput")
    w_d = {}
    for name, dt_ in (("wqT", FP8), ("wkT", FP8), ("wvT", FP8),
                      ("w1T", BF16), ("w2T", BF16), ("f1T", BF16),
                      ("f2T", FP8)):
        w_d[name] = nc.dram_tensor(name, [D, D], dt_, kind="ExternalInput")
    pp_d = nc.dram_tensor("pp", [P, DO, NPARAM], F32, kind="ExternalInput")
    diags_d = nc.dram_tensor("diags", [P, 3, DO, P], BF16, kind="ExternalInput")
    rows_d = nc.dram_tensor("rows", [1, 3 * D], FP8, kind="ExternalInput")
    yT_d = nc.dram_tensor("y_T", [D, N], F32, kind="ExternalOutput")

    acc_sp = nc.dram_tensor("acc_sp", [D, N], F32)

    xT = xT_d.rearrange("(o p) n -> p o n", p=P)
    xqT = xqT_d.rearrange("(o p) n -> p o n", p=P)
    wr = {k: v.rearrange("(o p) n -> p o n", p=P) for k, v in w_d.items()}
    acc_r = acc_sp.rearrange("(o p) n -> p o n", p=P)
    yT = yT_d.rearrange("(o p) n -> p o n", p=P)

    with tile.TileContext(nc) as tc, ExitStack() as top:
        persist = top.enter_context(tc.tile_pool(name="persist", bufs=1))
        pp = persist.tile([P, DO, NPARAM], F32)
        nc.sync.dma_start(pp, pp_d[:])
        rows = ones_row = None
        if use_bq or use_bk or use_bv:
            rows = persist.tile([1, 3 * D], FP8)
            nc.sync.dma_start(rows, rows_d[:])
            ones_row = persist.tile([1, max(NT, 512)], FP8)
            nc.vector.memset(ones_row, 1.0)
        ones_1p_bf = persist.tile([1, P], BF16)
        nc.vector.memset(ones_1p_bf, 1.0)
        ones_pair = persist.tile([P, 2, 16], FP8)
        nc.vector.memset(ones_pair, 1.0)
        ones_q = persist.tile([P, 2, P], FP8)
        nc.vector.memset(ones_q, 1.0)
        ones_one = persist.tile([1, 1], BF16)
        nc.vector.memset(ones_one, 1.0)
        ksrow_sb = persist.tile([1, D], BF16)
        onesD_bf = persist.tile([P, P], BF16)
        nc.vector.memset(onesD_bf, 1.0 / D)
        onesD_f32 = persist.tile([P, P], F32)
        nc.vector.memset(onesD_f32, 1.0 / D)
        eps_ln = persist.tile([P, 1], F32)
        nc.vector.memset(eps_ln, 1e-5)
        ln32_b = persist.tile([P, 1], F32)
        nc.vector.memset(ln32_b, LN_SQK)
        kv_sb = persist.tile([P, DO, D], FP8)
        ksum_sb = persist.tile([P, DO, 16], FP8)
        qp_all = persist.tile([P, DO, N], FP8)
        diags = persist.tile([P, 3, DO, P], BF16)
        nc.sync.dma_start(diags, diags_d[:])

        def stats_q(psum, rhs3, width):
            """Sum over channels (lhs=1.0 fp8 DoubleRow); caller scales by 1/D."""
            for c0 in range(0, width, 512):
                cw = min(512, width - c0)
                for j in range(DO // 2):
                    nc.tensor.matmul(psum[:, c0:c0 + cw], ones_q,
                                     rhs3[:, 2 * j:2 * j + 2, c0:c0 + cw],
                                     start=(j == 0), stop=(j == DO // 2 - 1),
                                     perf_mode=DR)

        def stats_bf(psum, rhs3, width):
            """Mean over channels (lhs=1/D bf16), replicated across partitions."""
            for c0 in range(0, width, 512):
                cw = min(512, width - c0)
                for kc in range(DO):
                    nc.tensor.matmul(psum[:, c0:c0 + cw], onesD_bf,
                                     rhs3[:, kc, c0:c0 + cw],
                                     start=(kc == 0), stop=(kc == DO - 1))

        F32R = mybir.dt.float32r

        # ---------------- Phases A+B scope ----------------
        with ExitStack() as phab:
            kvpool = phab.enter_context(tc.tile_pool(name="kvres", bufs=1))
            kp_all = kvpool.tile([P, TOTCH, D], FP8, tag="kp")
            v_all = kvpool.tile([P, TOTCH, D], FP8, tag="v")

            # ---------------- Phase A: QKV ----------------
            with ExitStack() as ph:
                wpool = ph.enter_context(tc.tile_pool(name="wA", bufs=1))
                wq_sb = wpool.tile([P, DO, D], FP8, tag="wq")
                nc.sync.dma_start(wq_sb, wr["wqT"])
                wk_sb = wpool.tile([P, DO, D], FP8, tag="wk")
                nc.sync.dma_start(wk_sb, wr["wkT"])
                wv_sb = wpool.tile([P, DO, D], FP8, tag="wv")
                nc.sync.dma_start(wv_sb, wr["wvT"])
                io = ph.enter_context(tc.tile_pool(name="ioA", bufs=2))
                ev = ph.enter_context(tc.tile_pool(name="evA", bufs=3))
                ps = ph.enter_context(tc.tile_pool(name="psA", bufs=3, space="PSUM"))

                for it in range(NTILES_A):
                    n0 = it * NTA
                    x_t = io.tile([P, DO, NTA], FP8, tag="xA")
                    nc.sync.dma_start(x_t, xqT[:, :, n0:n0 + NTA])

                    # ---- Q: layout B, out [dout-pair, n]; psum [P, 2*NT] flat
                    for dc0 in range(0, DO, 2):
                        ps_q = ps.tile([P, 2 * NTA], F32, tag="psA")
                        for h in range(2):
                            dc = dc0 + h
                            for j in range(DO // 2):
                                nc.tensor.matmul(
                                    ps_q[:, h * NTA:(h + 1) * NTA],
                                    wq_sb[:, 2 * j:2 * j + 2, dc * P:(dc + 1) * P],
                                    x_t[:, 2 * j:2 * j + 2, :],
                                    start=(j == 0),
                                    stop=(j == DO // 2 - 1 and not use_bq),
                                    perf_mode=DR)
                            if use_bq:
                                nc.tensor.matmul(ps_q[:, h * NTA:(h + 1) * NTA],
                                                 rows[0:1, dc * P:(dc + 1) * P],
                                                 ones_row[0:1, 0:NTA],
                                                 start=False, stop=True)
                        m_t = ev.tile([P, 2 * NTA], BF16, tag="mA")
                        # relu(-q) = -min(q,0) on ACT (frees a DVE psum read)
                        nc.scalar.activation(m_t, ps_q, AF.Relu,
                                             scale=-1.0 / SQK)
                        e_t = ev.tile([P, 2 * NTA], BF16, tag="eA")
                        # 32*exp(min(q,0)) = exp(-relu(-q) + ln32)
                        nc.scalar.activation(e_t, m_t, AF.Exp, scale=-1.0,
                                             bias=ln32_b[:, 0:1])
                        # qp = max(32q,0) + 32*exp(min(q,0)) = 32*(elu(q)+1)
                        nc.vector.scalar_tensor_tensor(
                            qp_all[:, dc0:dc0 + 2, n0:n0 + NTA], ps_q, 0.0, e_t,
                            OP.max, OP.add)

                    # ---- K, V: layout A, out [token-chunk, 1024 douts] ----
                    for ch in range(NCH):
                        c = it * NCH + ch
                        cs = slice(ch * P, (ch + 1) * P)
                        ps_k = ps.tile([P, D], F32, tag="psA")
                        for h in range(2):
                            hs = slice(h * 512, (h + 1) * 512)
                            for j in range(DO // 2):
                                nc.tensor.matmul(
                                    ps_k[:, hs],
                                    x_t[:, 2 * j:2 * j + 2, cs],
                                    wk_sb[:, 2 * j:2 * j + 2, hs],
                                    start=(j == 0),
                                    stop=(j == DO // 2 - 1 and not use_bk),
                                    perf_mode=DR)
                            if use_bk:
                                nc.tensor.matmul(ps_k[:, hs], ones_pair[:, 0, 0:1],
                                                 rows[0:1, D + h * 512:D + (h + 1) * 512],
                                                 start=False, stop=True)
                        m2 = ev.tile([P, D], BF16, tag="mA2")
                        nc.scalar.activation(m2, ps_k, AF.Relu,
                                             scale=-1.0 / SQK)
                        e2 = ev.tile([P, D], BF16, tag="eA2")
                        nc.scalar.activation(e2, m2, AF.Exp, scale=-1.0,
                                             bias=ln32_b[:, 0:1])
                        nc.vector.scalar_tensor_tensor(kp_all[:, c, :], ps_k, 0.0,
                                                       e2, OP.max, OP.add)

                        ps_v = ps.tile([P, D], F32, tag="psA")
                        for h in range(2):
                            hs = slice(h * 512, (h + 1) * 512)
                            for j in range(DO // 2):
                                nc.tensor.matmul(
                                    ps_v[:, hs],
                                    x_t[:, 2 * j:2 * j + 2, cs],
                                    wv_sb[:, 2 * j:2 * j + 2, hs],
                                    start=(j == 0),
                                    stop=(j == DO // 2 - 1 and not use_bv),
                                    perf_mode=DR)
                            if use_bv:
                                nc.tensor.matmul(ps_v[:, hs], ones_pair[:, 0, 0:1],
                                                 rows[0:1, 2 * D + h * 512:2 * D + (h + 1) * 512],
                                                 start=False, stop=True)
                        nc.scalar.activation(v_all[:, c, :], ps_v, AF.Copy,
                                             scale=1.0 / SW)

            # ---------------- Phase B: KV accumulation + K_sum ----------------
            with ExitStack() as ph:
                ps = ph.enter_context(tc.tile_pool(name="psB", bufs=1, space="PSUM"))
                for dcg in range(2):
                    kv_ps = [ps.tile([P, 2, 512], F32, tag=f"kvps{d}",
                                     name=f"kvps{dcg}_{d}")
                             for d in range(4)]
                    for cp in range(TOTCH // 2):
                        for di in range(4):
                            dc = dcg * 4 + di
                            for eh in range(2):
                                nc.tensor.matmul(
                                    kv_ps[di][:, eh, :],
                                    kp_all[:, 2 * cp:2 * cp + 2, dc * P:(dc + 1) * P],
                                    v_all[:, 2 * cp:2 * cp + 2, eh * 512:(eh + 1) * 512],
                                    start=(cp == 0), stop=(cp == TOTCH // 2 - 1),
                                    perf_mode=DR)
                    for di in range(4):
                        dc = dcg * 4 + di
                        nc.scalar.activation(kv_sb[:, dc, :], kv_ps[di], AF.Copy,
                                             scale=KV_SC)
            with ExitStack() as ph:
                ksp = ph.enter_context(tc.tile_pool(name="ksB", bufs=1, space="PSUM"))
                ps_ks = ksp.tile([1, D], F32, tag="ksrow")
                for cp in range(TOTCH // 2):
                    for h in range(2):
                        hs = slice(h * 512, (h + 1) * 512)
                        nc.tensor.matmul(ps_ks[0:1, hs], ones_pair[:, :, 0:1],
                                         kp_all[:, 2 * cp:2 * cp + 2, hs],
                                         start=(cp == 0), stop=(cp == TOTCH // 2 - 1),
                                         perf_mode=DR)
                nc.scalar.activation(ksrow_sb, ps_ks[0:1, :], AF.Copy)
                # transpose K_sum row -> per-partition column layout [P, DO]
                ps_ksc = ksp.tile([P, DO], F32, tag="kscol")
                for dc in range(DO):
                    nc.tensor.matmul(ps_ksc[:, dc:dc + 1],
                                     ksrow_sb[0:1, dc * P:(dc + 1) * P],
                                     ones_one[0:1, 0:1], start=True, stop=True)
                nc.scalar.activation(ksum_sb[:, :, 0], ps_ksc, AF.Copy,
                                     scale=KS_SC)

        # ---------------- Phases C+D (shared FFN-weight prefetch) ----------------
        cd = top.enter_context(ExitStack())
        wpoolD = cd.enter_context(tc.tile_pool(name="wDpre", bufs=1))
        f1_sb = wpoolD.tile([P, DO, D], BF16, tag="f1")
        nc.sync.dma_start(f1_sb, wr["f1T"])
        f2_sb = wpoolD.tile([P, DO, D], FP8, tag="f2")
        nc.sync.dma_start(f2_sb, wr["f2T"])

        # ---------------- Phase C: conv'' + local MLP + token mixer ----------------
        with ExitStack() as ph:
            wpool = ph.enter_context(tc.tile_pool(name="wC", bufs=1))
            w1_sb = wpool.tile([P, DO, D], BF16, tag="w1")
            nc.sync.dma_start(w1_sb, wr["w1T"])
            w2_sb = wpool.tile([P, DO, D], BF16, tag="w2")
            nc.sync.dma_start(w2_sb, wr["w2T"])
            io = ph.enter_context(tc.tile_pool(name="ioC", bufs=2))
            pipe = ph.enter_context(tc.tile_pool(name="pipeC", bufs=2))
            mid = ph.enter_context(tc.tile_pool(name="midC", bufs=1))
            sm = ph.enter_context(tc.tile_pool(name="smC", bufs=1))
            ps = ph.enter_context(tc.tile_pool(name="psC", bufs=2, space="PSUM"))
            pst = ph.enter_context(tc.tile_pool(name="pstC", bufs=1, space="PSUM"))

            def c_front(it):
                n0 = it * NT
                x_t = io.tile([P, DO, W], BF16, tag="xC", name=f"x_{it}")
                xq_t = io.tile([P, DO, WST], FP8, tag="xqC", name=f"xq_{it}")
                lo, hi = n0 - 2, n0 + NT + 2
                if lo < 0:
                    nc.vector.memset(x_t[:, :, 0:2], 0.0)
                    nc.sync.dma_start(x_t[:, :, 2:W], xT[:, :, 0:hi])
                    nc.vector.memset(xq_t[:, :, 0:2], 0.0)
                    nc.sync.dma_start(xq_t[:, :, 2:W], xqT[:, :, 0:hi])
                elif hi > N:
                    nc.vector.memset(x_t[:, :, W - 2:W], 0.0)
                    nc.sync.dma_start(x_t[:, :, 0:W - 2], xT[:, :, lo:N])
                    nc.vector.memset(xq_t[:, :, W - 2:W], 0.0)
                    nc.sync.dma_start(xq_t[:, :, 0:W - 2], xqT[:, :, lo:N])
                else:
                    nc.sync.dma_start(x_t, xT[:, :, lo:hi])
                    nc.sync.dma_start(xq_t[:, :, 0:W], xqT[:, :, lo:hi])

                acc = io.tile([P, DO, NT], F32, tag="accC", name=f"acc_{it}")
                # diffusion dwconv'': center tap on ACT, side taps on DVE
                for o in range(DO):
                    nc.scalar.activation(acc[:, o, :], x_t[:, o, 2:NT + 2],
                                         AF.Identity, bias=pp[:, o, CB:CB + 1],
                                         scale=pp[:, o, C1:C1 + 1])
                for o in range(DO):
                    nc.vector.scalar_tensor_tensor(acc[:, o, :], x_t[:, o, 1:NT + 1],
                                                   pp[:, o, C0:C0 + 1], acc[:, o, :],
                                                   OP.mult, OP.add)
                for o in range(DO):
                    nc.vector.scalar_tensor_tensor(acc[:, o, :], x_t[:, o, 3:NT + 3],
                                                   pp[:, o, C2:C2 + 1], acc[:, o, :],
                                                   OP.mult, OP.add)

                # local MLP first half (bf16, paired dc groups)
                h1_t = pipe.tile([P, DO, NT], BF16, tag="h1", name=f"h1_{it}")
                for dc0 in range(0, DO, 2):
                    ps_h = ps.tile([P, 2, NT], F32, tag="psC",
                                   name=f"psh1_{it}_{dc0}")
                    for h in range(2):
                        dc = dc0 + h
                        for kc in range(DO):
                            nc.tensor.matmul(ps_h[:, h, :],
                                             w1_sb[:, kc, dc * P:(dc + 1) * P],
                                             x_t[:, kc, 2:NT + 2],
                                             start=(kc == 0), stop=(kc == DO - 1))
                    for h in range(2):
                        nc.scalar.activation(h1_t[:, dc0 + h, :], ps_h[:, h, :],
                                             AF.Gelu,
                                             bias=pp[:, dc0 + h, LUB1:LUB1 + 1])

                # token mixer LN stats (fp8 sum-matmuls on xq/sq)
                sq_t = mid.tile([P, DO, WST], FP8, tag="sqC", name=f"sq_{it}")
                nc.scalar.activation(sq_t[:, :, 0:W], x_t, AF.Square)
                ps_m = pst.tile([P, W], F32, tag="psm", name=f"psm_{it}")
                stats_q(ps_m, xq_t, W)
                ps_s = pst.tile([P, W], F32, tag="pss", name=f"pss_{it}")
                stats_q(ps_s, sq_t, W)
                m_sb = sm.tile([P, 1, W], BF16, tag="msb", name=f"msb_{it}")
                nc.scalar.activation(m_sb[:, 0, :], ps_m, AF.Copy, scale=1.0 / D)
                var = sm.tile([P, 1, W], F32, tag="var", name=f"var_{it}")
                nc.scalar.activation(var[:, 0, :], ps_m, AF.Square, scale=1.0 / D)
                nc.vector.scalar_tensor_tensor(var[:, 0, :], ps_s, 1.0 / D,
                                               var[:, 0, :], OP.mult, OP.subtract)
                nc.scalar.activation(var[:, 0, :], var[:, 0, :], AF.Sqrt,
                                     bias=eps_ln[:, 0:1])
                nc.vector.reciprocal_approx_fast(out=var[:, 0, :], in_=var[:, 0, :])
                rstd = sm.tile([P, 1, W], BF16, tag="rstd", name=f"rstd_{it}")
                nc.vector.tensor_copy(rstd, var)
                u_t = mid.tile([P, DO, W], BF16, tag="tokA", name=f"u_{it}")
                nc.vector.tensor_sub(u_t, x_t, m_sb.to_broadcast((P, DO, W)))
                xm_t = mid.tile([P, DO, W], BF16, tag="tokC", name=f"xm_{it}")
                if use_tmg:
                    for o in range(DO):
                        nc.vector.scalar_tensor_tensor(
                            xm_t[:, o, :], u_t[:, o, :], pp[:, o, TMG:TMG + 1],
                            rstd[:, 0, :], OP.mult, OP.mult)
                else:
                    nc.vector.tensor_mul(xm_t, u_t, rstd.to_broadcast((P, DO, W)))
                if use_tmb:
                    for o in range(DO):
                        nc.vector.tensor_scalar_add(xm_t[:, o, :], xm_t[:, o, :],
                                                    pp[:, o, TMB:TMB + 1])
                # conv1: t_s[k] = conv1(xm)[k+1], k in [0, W-2)
                # (reuses u_t's buffer — u is dead once xm is computed)
                t_full = mid.tile([P, DO, W], BF16, tag="tokA", name=f"t_{it}")
                t_t = t_full[:, :, 0:W - 2]
                for o in range(DO):
                    nc.scalar.activation(t_t[:, o, :], xm_t[:, o, 1:W - 1],
                                         AF.Identity, bias=pp[:, o, TCB1:TCB1 + 1],
                                         scale=pp[:, o, T1:T1 + 1])
                for o in range(DO):
                    nc.vector.scalar_tensor_tensor(t_t[:, o, :], xm_t[:, o, 0:W - 2],
                                                   pp[:, o, T0:T0 + 1],
                                                   t_t[:, o, :], OP.mult, OP.add)
                for o in range(DO):
                    nc.vector.scalar_tensor_tensor(t_t[:, o, :], xm_t[:, o, 2:W],
                                                   pp[:, o, T2:T2 + 1],
                                                   t_t[:, o, :], OP.mult, OP.add)
                t2_t = pipe.tile([P, DO, W - 2], BF16, tag="t2", name=f"t2_{it}")
                nc.scalar.activation(t2_t, t_t, AF.Gelu)
                if it == 0:
                    nc.vector.memset(t2_t[:, :, 0:1], 0.0)
                if it == NTILES - 1:
                    nc.vector.memset(t2_t[:, :, W - 3:W - 2], 0.0)
                return x_t, acc, h1_t, t2_t

            def c_back(it, tiles):
                n0 = it * NT
                x_t, acc, h1_t, t2_t = tiles
                for dc0 in range(0, DO, 2):
                    ps_h = ps.tile([P, 2, NT], F32, tag="psC",
                                   name=f"psh2_{it}_{dc0}")
                    for h in range(2):
                        dc = dc0 + h
                        for kc in range(DO):
                            nc.tensor.matmul(ps_h[:, h, :],
                                             w2_sb[:, kc, dc * P:(dc + 1) * P],
                                             h1_t[:, kc, :],
                                             start=(kc == 0), stop=False)
                        for tap in range(3):
                            nc.tensor.matmul(ps_h[:, h, :], diags[:, tap, dc, :],
                                             t2_t[:, dc, tap:NT + tap],
                                             start=False, stop=(tap == 2))
                    nc.vector.tensor_add(acc[:, dc0:dc0 + 2, :],
                                         acc[:, dc0:dc0 + 2, :], ps_h)
                nc.sync.dma_start(acc_r[:, :, n0:n0 + NT], acc)

            pend = {0: c_front(0)}
            for it in range(NTILES):
                if it + 1 < NTILES:
                    pend[it + 1] = c_front(it + 1)
                c_back(it, pend.pop(it))

        # ---------------- Phase D: attention + LN1 + FFN + LN2 ----------------
        with ExitStack() as ph:
            io = ph.enter_context(tc.tile_pool(name="ioD", bufs=2))
            mid = ph.enter_context(tc.tile_pool(name="midD", bufs=2))
            sm = ph.enter_context(tc.tile_pool(name="smD", bufs=2))
            ps = ph.enter_context(tc.tile_pool(name="psD", bufs=2, space="PSUM"))
            pst = ph.enter_context(tc.tile_pool(name="pstD", bufs=1, space="PSUM"))

            def d_front_a(it):
                """loads, norm row, rep fold, numerator halves 0-3."""
                n0 = it * NT
                acc_t = io.tile([P, DO, NT], F32, tag="accD", name=f"accD_{it}")
                nc.sync.dma_start(acc_t, acc_r[:, :, n0:n0 + NT])
                ps_n = pst.tile([P, NT], F32, tag="psrep", name=f"psn_{it}")
                for j in range(DO // 2):
                    nc.tensor.matmul(ps_n[0:1, :], ksum_sb[:, 2 * j:2 * j + 2, 0:1],
                                     qp_all[:, 2 * j:2 * j + 2, n0:n0 + NT],
                                     start=(j == 0), stop=(j == DO // 2 - 1),
                                     perf_mode=DR)
                rr32 = sm.tile([1, NT], F32, tag="rr32D", name=f"rr32_{it}")
                nc.vector.reciprocal_approx_fast(out=rr32, in_=ps_n[0:1, :])
                rr = sm.tile([1, NT], BF16, tag="rrD", name=f"rr_{it}")
                nc.vector.tensor_copy(rr, rr32)
                ps_rep = pst.tile([P, NT], F32, tag="psrep", name=f"psrep_{it}")
                nc.tensor.matmul(ps_rep, ones_1p_bf[0:1, :], rr, start=True,
                                 stop=True)
                rep_sb = mid.tile([P, 1, NT], BF16, tag="repsb", name=f"rep_{it}")
                nc.scalar.activation(rep_sb[:, 0, :], ps_rep, AF.Copy,
                                     scale=REP_SC)
                qp2_t = mid.tile([P, DO, NT], FP8, tag="qp2", name=f"qp2_{it}")
                nc.vector.tensor_mul(qp2_t, qp_all[:, :, n0:n0 + NT],
                                     rep_sb.to_broadcast((P, DO, NT)))
                for ec0 in range(0, DO // 2, 2):
                    ps_u = ps.tile([P, 2, NT], F32, tag="psD",
                                   name=f"psnum_{it}_{ec0}")
                    for h in range(2):
                        ec = ec0 + h
                        for j in range(DO // 2):
                            nc.tensor.matmul(
                                ps_u[:, h, :],
                                kv_sb[:, 2 * j:2 * j + 2, ec * P:(ec + 1) * P],
                                qp2_t[:, 2 * j:2 * j + 2, :],
                                start=(j == 0), stop=(j == DO // 2 - 1),
                                perf_mode=DR)
                    nc.vector.scalar_tensor_tensor(acc_t[:, ec0:ec0 + 2, :], ps_u,
                                                   NUM_SC, acc_t[:, ec0:ec0 + 2, :],
                                                   OP.mult, OP.add)
                return qp2_t, acc_t

            def d_front_b(it, T):
                qp2_t, acc_t = T
                for ec0 in range(DO // 2, DO, 2):
                    ps_u = ps.tile([P, 2, NT], F32, tag="psD",
                                   name=f"psnum_{it}_{ec0}")
                    for h in range(2):
                        ec = ec0 + h
                        for j in range(DO // 2):
                            nc.tensor.matmul(
                                ps_u[:, h, :],
                                kv_sb[:, 2 * j:2 * j + 2, ec * P:(ec + 1) * P],
                                qp2_t[:, 2 * j:2 * j + 2, :],
                                start=(j == 0), stop=(j == DO // 2 - 1),
                                perf_mode=DR)
                    nc.vector.scalar_tensor_tensor(acc_t[:, ec0:ec0 + 2, :], ps_u,
                                                   NUM_SC, acc_t[:, ec0:ec0 + 2, :],
                                                   OP.mult, OP.add)
                return acc_t

            def d_mid(it, acc_t):
                """LN1: mean via f32r matmuls on acc, sq-stats bf16; in-place
                apply -> y1 (bf16, two halves so f1 can start early)."""
                accq = mid.tile([P, DO, NT], FP8, tag="accq", name=f"accq_{it}", bufs=1)
                nc.scalar.activation(accq, acc_t, AF.Copy)
                sqq = mid.tile([P, DO, NT], FP8, tag="sqD", name=f"sqD_{it}", bufs=1)
                nc.scalar.activation(sqq, acc_t, AF.Square)
                psst = pst.tile([P, 2, NT], F32, tag="psst", name=f"psst1_{it}")
                stats_q(psst[:, 0, :], accq, NT)
                stats_q(psst[:, 1, :], sqq, NT)
                m1_sb = sm.tile([P, 1, NT], F32, tag="m1sb", name=f"m1_{it}")
                nc.scalar.activation(m1_sb[:, 0, :], psst[:, 0, :], AF.Copy,
                                     scale=1.0 / D)
                var1 = sm.tile([P, 1, NT], F32, tag="varD", name=f"var1_{it}")
                nc.scalar.activation(var1[:, 0, :], psst[:, 0, :], AF.Square,
                                     scale=1.0 / D)
                nc.vector.scalar_tensor_tensor(var1[:, 0, :], psst[:, 1, :],
                                               1.0 / D, var1[:, 0, :],
                                               OP.mult, OP.subtract)
                nc.scalar.activation(var1[:, 0, :], var1[:, 0, :], AF.Sqrt,
                                     bias=eps_ln[:, 0:1])
                nc.vector.reciprocal_approx_fast(out=var1[:, 0, :],
                                                 in_=var1[:, 0, :])
                # u1 in place of acc (acc is dead after the stats);
                # half-chunk interleave so f1's kc-chain starts early
                y1_t = mid.tile([P, DO, NT], BF16, tag="y1", name=f"y1_{it}")
                for hh in range(0, DO, 4):
                    sl = slice(hh, hh + 4)
                    nc.vector.tensor_sub(acc_t[:, sl, :], acc_t[:, sl, :],
                                         m1_sb.to_broadcast((P, 4, NT)))
                    if use_n1g:
                        for o in range(hh, hh + 4):
                            nc.vector.scalar_tensor_tensor(
                                y1_t[:, o, :], acc_t[:, o, :],
                                pp[:, o, N1G:N1G + 1],
                                var1[:, 0, :], OP.mult, OP.mult)
                    else:
                        nc.vector.tensor_mul(y1_t[:, sl, :], acc_t[:, sl, :],
                                             var1.to_broadcast((P, 4, NT)))
                if use_n1b:
                    for o in range(DO):
                        nc.vector.tensor_scalar_add(y1_t[:, o, :], y1_t[:, o, :],
                                                    pp[:, o, N1B:N1B + 1])
                return y1_t

            def d_f1(it, y1_t, tail):
                """f1 matmul groups with the previous tile's LN2-apply (yo)
                DVE ops interleaved under the PE-heavy stretch."""
                f1h_t = mid.tile([P, DO, NT], FP8, tag="f1h", name=f"f1h_{it}")
                for gi, dc0 in enumerate(range(0, DO, 2)):
                    ps_f = ps.tile([P, 2, NT], F32, tag="psD",
                                   name=f"psf1_{it}_{dc0}")
                    for h in range(2):
                        dc = dc0 + h
                        for kc in range(DO):
                            nc.tensor.matmul(ps_f[:, h, :],
                                             f1_sb[:, kc, dc * P:(dc + 1) * P],
                                             y1_t[:, kc, :],
                                             start=(kc == 0), stop=(kc == DO - 1))
                    if gi < len(tail):
                        tail[gi]()
                    for h in range(2):
                        nc.scalar.activation(f1h_t[:, dc0 + h, :], ps_f[:, h, :],
                                             AF.Gelu,
                                             bias=pp[:, dc0 + h, FFB1:FFB1 + 1])
                for fn in tail[len(list(range(0, DO, 2))):]:
                    fn()
                return f1h_t

            def d_f2(it, y1_t, f1h_t):
                y2_t = mid.tile([P, DO, NT], BF16, tag="y2", name=f"y2_{it}")
                for dc0 in range(0, DO, 2):
                    ps_f = ps.tile([P, 2, NT], F32, tag="psD",
                                   name=f"psf2_{it}_{dc0}")
                    for h in range(2):
                        dc = dc0 + h
                        for j in range(DO // 2):
                            nc.tensor.matmul(
                                ps_f[:, h, :],
                                f2_sb[:, 2 * j:2 * j + 2, dc * P:(dc + 1) * P],
                                f1h_t[:, 2 * j:2 * j + 2, :],
                                start=(j == 0), stop=(j == DO // 2 - 1),
                                perf_mode=DR)
                    # y2 = y1 + psum/64
                    nc.vector.scalar_tensor_tensor(y2_t[:, dc0:dc0 + 2, :], ps_f,
                                                   1.0 / SW,
                                                   y1_t[:, dc0:dc0 + 2, :],
                                                   OP.mult, OP.add)
                if use_fb2:
                    for dc in range(DO):
                        nc.vector.tensor_scalar_add(y2_t[:, dc, :], y2_t[:, dc, :],
                                                    pp[:, dc, FFB2:FFB2 + 1])
                return y2_t

            def d_back_head(it, y2_t):
                sq2_t = mid.tile([P, DO, NT], BF16, tag="sq2", name=f"sq2_{it}", bufs=1)
                nc.scalar.activation(sq2_t, y2_t, AF.Square)
                psst = pst.tile([P, 2, NT], F32, tag="psst", name=f"psst2_{it}")
                stats_bf(psst[:, 0, :], y2_t, NT)
                stats_bf(psst[:, 1, :], sq2_t, NT)
                m2_sb = sm.tile([P, 1, NT], F32, tag="m2sb", name=f"m2_{it}")
                nc.scalar.activation(m2_sb[:, 0, :], psst[:, 0, :], AF.Copy)
                var2 = sm.tile([P, 1, NT], F32, tag="var2D", name=f"var2_{it}")
                nc.scalar.activation(var2[:, 0, :], psst[:, 0, :], AF.Square)
                nc.vector.tensor_sub(var2[:, 0, :], psst[:, 1, :], var2[:, 0, :])
                nc.scalar.activation(var2[:, 0, :], var2[:, 0, :], AF.Sqrt,
                                     bias=eps_ln[:, 0:1])
                nc.vector.reciprocal_approx_fast(out=var2[:, 0, :],
                                                 in_=var2[:, 0, :])
                return y2_t, m2_sb, var2

            def d_back_tail(it, state):
                """Returns closures: yo half-ops + output DMA, to be issued
                under the next tile's f1 matmul groups."""
                y2_t, m2_sb, var2 = state
                n0 = it * NT
                yo_t = mid.tile([P, DO, NT], F32, tag="yo", name=f"yo_{it}",
                                bufs=1)
                fns = []
                if use_n2g:
                    def sub_all():
                        nc.vector.tensor_sub(yo_t, y2_t,
                                             m2_sb.to_broadcast((P, DO, NT)))
                        for o in range(DO):
                            nc.vector.scalar_tensor_tensor(
                                yo_t[:, o, :], yo_t[:, o, :],
                                pp[:, o, N2G:N2G + 1], var2[:, 0, :],
                                OP.mult, OP.mult)
                        if use_n2b:
                            for o in range(DO):
                                nc.vector.tensor_scalar_add(
                                    yo_t[:, o, :], yo_t[:, o, :],
                                    pp[:, o, N2B:N2B + 1])
                        nc.sync.dma_start(yT[:, :, n0:n0 + NT], yo_t)
                    return [sub_all]
                for hh in range(0, DO, 4):
                    def half(hh=hh):
                        sl = slice(hh, hh + 4)
                        nc.vector.tensor_sub(yo_t[:, sl, :], y2_t[:, sl, :],
                                             m2_sb.to_broadcast((P, 4, NT)))
                        nc.vector.tensor_mul(yo_t[:, sl, :], yo_t[:, sl, :],
                                             var2.to_broadcast((P, 4, NT)))
                        nc.sync.dma_start(yT[:, hh:hh + 4, n0:n0 + NT],
                                          yo_t[:, sl, :])
                    fns.append(half)
                return fns

            Tnext = d_front_a(0)
            acc_next = d_front_b(0, Tnext)
            tail = []
            for it in range(NTILES):
                acc_cur = acc_next
                if it + 1 < NTILES:
                    Tnext = d_front_a(it + 1)
                    acc_next = d_front_b(it + 1, Tnext)
                y1_cur = d_mid(it, acc_cur)
                f1h_cur = d_f1(it, y1_cur, tail)
                y2_cur = d_f2(it, y1_cur, f1h_cur)
                state = d_back_head(it, y2_cur)
                tail = d_back_tail(it, state)
            for fn in tail:
                fn()
    nc.compile()
    return nc


def make_in_maps(inputs, n_cores=8):
    """Host-side preprocessing: fold constants, transpose, cast, shard."""
    x = np.asarray(inputs["x"], np.float32)
    B, N, D_ = x.shape
    dt = float(np.asarray(inputs["delta_t"]))

    def g(k):
        return np.asarray(inputs[k], np.float32)

    diff_w, diff_b = g("diff_w"), g("diff_b")
    tm_w1, tm_cb1 = g("tm_w1"), g("tm_cb1")
    tm_w2, tm_cb2 = g("tm_w2"), g("tm_cb2")

    pp = np.zeros((P, DO, NPARAM), np.float32)

    def put(i, v):
        pp[:, :, i] = v.reshape(DO, P).T

    put(C0, dt * diff_w[:, 0, 0])
    put(C1, dt * diff_w[:, 0, 1] + (1.0 - dt))
    put(C2, dt * diff_w[:, 0, 2])
    put(CB, dt * diff_b + g("lu_b2") + tm_cb2)
    put(T0, tm_w1[:, 0, 0])
    put(T1, tm_w1[:, 0, 1])
    put(T2, tm_w1[:, 0, 2])
    put(TCB1, tm_cb1)
    put(U0, tm_w2[:, 0, 0])
    put(U1, tm_w2[:, 0, 1])
    put(U2, tm_w2[:, 0, 2])
    put(TMG, g("tm_g"))
    put(TMB, g("tm_beta"))
    put(N1G, g("n1_g"))
    put(N1B, g("n1_b"))
    put(N2G, g("n2_g"))
    put(N2B, g("n2_b"))
    put(LUB1, g("lu_b1"))
    put(FFB1, g("ff_b1"))
    put(FFB2, g("ff_b2"))

    diags = np.zeros((P, 3, DO, P), np.float32)
    idx = np.arange(P)
    for tap in range(3):
        for dc in range(DO):
            diags[idx, tap, dc, idx] = tm_w2[dc * P + idx, 0, tap]
    diags = diags.astype(BF16_NP)

    rows = np.zeros((1, 3 * D), np.float32)
    rows[0, 0:D] = SQK * g("bq")
    rows[0, D:2 * D] = SQK * g("bk")
    rows[0, 2 * D:3 * D] = SW * g("bv")
    rows = np.clip(rows, -240, 240).astype(FP8_NP)

    wt = {}
    for name, key, sc in (("wqT", "wq", SQK), ("wkT", "wk", SQK),
                          ("wvT", "wv", SW), ("f2T", "ff_w2", SW)):
        wt[name] = np.clip(
            np.ascontiguousarray(g(key).T) * sc, -240, 240).astype(FP8_NP)
    for name, key in (("w1T", "lu_w1"), ("w2T", "lu_w2"), ("f1T", "ff_w1")):
        wt[name] = np.ascontiguousarray(g(key).T).astype(BF16_NP)

    xt_f = np.ascontiguousarray(x.transpose(0, 2, 1))
    xT = xt_f.astype(BF16_NP)
    xqT = np.clip(xt_f, -240, 240).astype(FP8_NP)

    ones = np.ones((D_,), np.float32)
    flags = dict(
        use_bq=bool(np.any(g("bq"))),
        use_bk=bool(np.any(g("bk"))),
        use_bv=bool(np.any(g("bv"))),
        use_tmb=bool(np.any(g("tm_beta"))),
        use_n1b=bool(np.any(g("n1_b"))),
        use_n2b=bool(np.any(g("n2_b"))),
        use_fb2=bool(np.any(g("ff_b2"))),
        use_tmg=bool(np.any(g("tm_g") != ones)),
        use_n1g=bool(np.any(g("n1_g") != ones)),
        use_n2g=bool(np.any(g("n2_g") != ones)),
    )

    shared = {**wt, "pp": pp, "rows": rows, "diags": diags}
    in_maps = [{**shared, "x_T": xT[b], "xq_T": xqT[b]} for b in range(n_cores)]
    return in_maps, flags, (B, N)


_NC_CACHE = {}


def kernel(**inputs):
    in_maps, flags, (B, N) = make_in_maps(inputs)
    key = (N, tuple(sorted(flags.items())))
    if key not in _NC_CACHE:
        _NC_CACHE[key] = build_nc(N=N, NT=512, **flags)
    nc = _NC_CACHE[key]
    res = run_bass_kernel_spmd(nc, in_maps, list(range(B)))
    y = np.stack([res.results[b]["y_T"] for b in range(B)])
    return np.ascontiguousarray(y.transpose(0, 2, 1)).astype(np.float32)


# revision 20
# speedup vs baseline: 1.1149x; 1.0029x over previous
"""Trainium2 Bass kernel for nn_DiffuRNNLayer (B=8, N=2048, D=1024).

Sharding: data-parallel over batch — one batch element per NeuronCore (8 cores).
v3: mixed fp8-e4m3 DoubleRow / bf16 matmuls, chosen per-path by error budget:
  fp8 DR: wq/wk/wv (phase A), KV (B), f2+f1h (D), numerator+norm (D),
          LN-mean matmuls for tokenmixer-LN/LN1/LN2.
  bf16:   lu_w1/lu_w2 (local MLP, residual-critical), ff_w1, conv diag taps.
Scale bookkeeping: wq,wk x32; wv,f2 x64; qp_all=32*Qp, kp_all=32*Kp, v_all=V,
kv_sb=KV/2, ksum_sb=Ksum/16, rep=2^16/norm, numerator PSUM=2^20*attn.
Kp/V/Qp are SBUF-resident fp8 (no HBM spill); KV+K_sum run back-to-back from
SBUF.  DVE ops are merged across channel chunks wherever gains are 1
(broadcast [P,1,*] operands), and PSUM consumers are paired into [P,2,NT]
tiles to halve instruction count.
"""

import math
import numpy as np
import ml_dtypes
from contextlib import ExitStack

import concourse.bass as bass
import concourse.bacc as bacc
import concourse.tile as tile
import concourse.mybir as mybir
from concourse.bass_utils import run_bass_kernel_spmd

F32 = mybir.dt.float32
BF16 = mybir.dt.bfloat16
FP8 = mybir.dt.float8e4
AF = mybir.ActivationFunctionType
OP = mybir.AluOpType
DR = mybir.MatmulPerfMode.DoubleRow
BF16_NP = ml_dtypes.bfloat16
FP8_NP = ml_dtypes.float8_e4m3

P = 128
D = 1024
DO = D // P  # 8 chunks of the channel dim

SQK = 32.0       # weight scale for wq/wk (keeps 32*Qp < 240 in fp8)
SW = 64.0        # weight scale for wv/f2
LN_SQK = math.log(SQK)
KS_SC = 2.0 ** -9    # ksum fp8 = (sum of 32*Kp) * 2^-9 = Ksum/16
KV_SC = 1.0 / SW     # kv fp8 = (sum 32Kp*V)/64 = KV/2
REP_SC = 2.0 ** 17   # rep = rr * 2^17 = 2^16/norm  (rr = 1/(2*norm))
NUM_SC = 2.0 ** -20  # ps_u = 2^20 * attn  ->  acc += ps_u * 2^-20

# pp param-plane indices (per-partition params, laid out [128, DO, NP])
(C0, C1, C2, CB, T0, T1, T2, TCB1, U0, U1, U2,
 TMG, TMB, N1G, N1B, N2G, N2B, LUB1, FFB1, FFB2) = range(20)
NPARAM = 20



def build_nc(N=2048, NT=512, use_bq=False, use_bk=False, use_bv=False,
             use_tmb=False, use_n1b=False, use_n2b=False, use_fb2=False,
             use_tmg=False, use_n1g=False, use_n2g=False, debug=False):
    NTILES = N // NT
    NTA = 512              # phase-A tile size (independent of C/D tiling)
    NTILES_A = N // NTA
    NCH = NTA // P         # 128-token chunks per phase-A tile
    TOTCH = N // P
    W = NT + 4             # phase-C tile width with +-2 halo
    WST = (W + 15) // 16 * 16  # fp8 stats tile width (16B-aligned)
    assert N % NT == 0 and NT % P == 0 and N % NTA == 0

    nc = bacc.Bacc(None, target_bir_lowering=False, debug=debug)

    xT_d = nc.dram_tensor("x_T", [D, N], BF16, kind="ExternalInput")
    xqT_d = nc.dram_tensor("xq_T", [D, N], FP8, kind="ExternalInput")
    w_d = {}
    for name, dt_ in (("wqT", FP8), ("wkT", FP8), ("wvT", FP8),
                      ("w1T", BF16), ("w2T", BF16), ("f1T", BF16),
                      ("f2T", FP8)):
        w_d[name] = nc.dram_tensor(name, [D, D], dt_, kind="ExternalInput")
    pp_d = nc.dram_tensor("pp", [P, DO, NPARAM], F32, kind="ExternalInput")
    diags_d = nc.dram_tensor("diags", [P, 3, DO, P], BF16, kind="ExternalInput")
    rows_d = nc.dram_tensor("rows", [1, 3 * D], FP8, kind="ExternalInput")
    yT_d = nc.dram_tensor("y_T", [D, N], F32, kind="ExternalOutput")

    acc_sp = nc.dram_tensor("acc_sp", [D, N], F32)

    xT = xT_d.rearrange("(o p) n -> p o n", p=P)
    xqT = xqT_d.rearrange("(o p) n -> p o n", p=P)
    wr = {k: v.rearrange("(o p) n -> p o n", p=P) for k, v in w_d.items()}
    acc_r = acc_sp.rearrange("(o p) n -> p o n", p=P)
    yT = yT_d.rearrange("(o p) n -> p o n", p=P)

    with tile.TileContext(nc) as tc, ExitStack() as top:
        persist = top.enter_context(tc.tile_pool(name="persist", bufs=1))
        pp = persist.tile([P, DO, NPARAM], F32)
        nc.sync.dma_start(pp, pp_d[:])
        rows = ones_row = None
        if use_bq or use_bk or use_bv:
            rows = persist.tile([1, 3 * D], FP8)
            nc.sync.dma_start(rows, rows_d[:])
            ones_row = persist.tile([1, max(NT, 512)], FP8)
            nc.vector.memset(ones_row, 1.0)
        ones_1p_bf = persist.tile([1, P], BF16)
        nc.vector.memset(ones_1p_bf, 1.0)
        ones_pair = persist.tile([P, 2, 16], FP8)
        nc.vector.memset(ones_pair, 1.0)
        ones_q = persist.tile([P, 2, P], FP8)
        nc.vector.memset(ones_q, 1.0)
        ones_one = persist.tile([1, 1], BF16)
        nc.vector.memset(ones_one, 1.0)
        ksrow_sb = persist.tile([1, D], BF16)
        onesD_bf = persist.tile([P, P], BF16)
        nc.vector.memset(onesD_bf, 1.0 / D)
        onesD_f32 = persist.tile([P, P], F32)
        nc.vector.memset(onesD_f32, 1.0 / D)
        eps_ln = persist.tile([P, 1], F32)
        nc.vector.memset(eps_ln, 1e-5)
        ln32_b = persist.tile([P, 1], F32)
        nc.vector.memset(ln32_b, LN_SQK)
        kv_sb = persist.tile([P, DO, D], FP8)
        ksum_sb = persist.tile([P, DO, 16], FP8)
        qp_all = persist.tile([P, DO, N], FP8)
        diags = persist.tile([P, 3, DO, P], BF16)
        nc.sync.dma_start(diags, diags_d[:])

        def stats_q(psum, rhs3, width):
            """Sum over channels (lhs=1.0 fp8 DoubleRow); caller scales by 1/D."""
            for c0 in range(0, width, 512):
                cw = min(512, width - c0)
                for j in range(DO // 2):
                    nc.tensor.matmul(psum[:, c0:c0 + cw], ones_q,
                                     rhs3[:, 2 * j:2 * j + 2, c0:c0 + cw],
                                     start=(j == 0), stop=(j == DO // 2 - 1),
                                     perf_mode=DR)

        def stats_bf(psum, rhs3, width):
            """Mean over channels (lhs=1/D bf16), replicated across partitions."""
            for c0 in range(0, width, 512):
                cw = min(512, width - c0)
                for kc in range(DO):
                    nc.tensor.matmul(psum[:, c0:c0 + cw], onesD_bf,
                                     rhs3[:, kc, c0:c0 + cw],
                                     start=(kc == 0), stop=(kc == DO - 1))

        F32R = mybir.dt.float32r

        # ---------------- Phases A+B scope ----------------
        with ExitStack() as phab:
            kvpool = phab.enter_context(tc.tile_pool(name="kvres", bufs=1))
            kp_all = kvpool.tile([P, TOTCH, D], FP8, tag="kp")
            v_all = kvpool.tile([P, TOTCH, D], FP8, tag="v")

            # ---------------- Phase A: QKV ----------------
            with ExitStack() as ph:
                wpool = ph.enter_context(tc.tile_pool(name="wA", bufs=1))
                wq_sb = wpool.tile([P, DO, D], FP8, tag="wq")
                nc.sync.dma_start(wq_sb, wr["wqT"])
                wk_sb = wpool.tile([P, DO, D], FP8, tag="wk")
                nc.sync.dma_start(wk_sb, wr["wkT"])
                wv_sb = wpool.tile([P, DO, D], FP8, tag="wv")
                nc.sync.dma_start(wv_sb, wr["wvT"])
                io = ph.enter_context(tc.tile_pool(name="ioA", bufs=2))
                ev = ph.enter_context(tc.tile_pool(name="evA", bufs=3))
                ps = ph.enter_context(tc.tile_pool(name="psA", bufs=3, space="PSUM"))

                for it in range(NTILES_A):
                    n0 = it * NTA
                    x_t = io.tile([P, DO, NTA], FP8, tag="xA")
                    nc.sync.dma_start(x_t, xqT[:, :, n0:n0 + NTA])

                    # ---- Q: layout B, out [dout-pair, n]; psum [P, 2*NT] flat
                    for dc0 in range(0, DO, 2):
                        ps_q = ps.tile([P, 2 * NTA], F32, tag="psA")
                        for h in range(2):
                            dc = dc0 + h
                            for j in range(DO // 2):
                                nc.tensor.matmul(
                                    ps_q[:, h * NTA:(h + 1) * NTA],
                                    wq_sb[:, 2 * j:2 * j + 2, dc * P:(dc + 1) * P],
                                    x_t[:, 2 * j:2 * j + 2, :],
                                    start=(j == 0),
                                    stop=(j == DO // 2 - 1 and not use_bq),
                                    perf_mode=DR)
                            if use_bq:
                                nc.tensor.matmul(ps_q[:, h * NTA:(h + 1) * NTA],
                                                 rows[0:1, dc * P:(dc + 1) * P],
                                                 ones_row[0:1, 0:NTA],
                                                 start=False, stop=True)
                        m_t = ev.tile([P, 2 * NTA], BF16, tag="mA")
                        # relu(-q) = -min(q,0) on ACT (frees a DVE psum read)
                        nc.scalar.activation(m_t, ps_q, AF.Relu,
                                             scale=-1.0 / SQK)
                        e_t = ev.tile([P, 2 * NTA], BF16, tag="eA")
                        # 32*exp(min(q,0)) = exp(-relu(-q) + ln32)
                        nc.scalar.activation(e_t, m_t, AF.Exp, scale=-1.0,
                                             bias=ln32_b[:, 0:1])
                        # qp = max(32q,0) + 32*exp(min(q,0)) = 32*(elu(q)+1)
                        nc.vector.scalar_tensor_tensor(
                            qp_all[:, dc0:dc0 + 2, n0:n0 + NTA], ps_q, 0.0, e_t,
                            OP.max, OP.add)

                    # ---- K, V: layout A, out [token-chunk, 1024 douts] ----
                    for ch in range(NCH):
                        c = it * NCH + ch
                        cs = slice(ch * P, (ch + 1) * P)
                        ps_k = ps.tile([P, D], F32, tag="psA")
                        for h in range(2):
                            hs = slice(h * 512, (h + 1) * 512)
                            for j in range(DO // 2):
                                nc.tensor.matmul(
                                    ps_k[:, hs],
                                    x_t[:, 2 * j:2 * j + 2, cs],
                                    wk_sb[:, 2 * j:2 * j + 2, hs],
                                    start=(j == 0),
                                    stop=(j == DO // 2 - 1 and not use_bk),
                                    perf_mode=DR)
                            if use_bk:
                                nc.tensor.matmul(ps_k[:, hs], ones_pair[:, 0, 0:1],
                                                 rows[0:1, D + h * 512:D + (h + 1) * 512],
                                                 start=False, stop=True)
                        m2 = ev.tile([P, D], BF16, tag="mA2")
                        nc.scalar.activation(m2, ps_k, AF.Relu,
                                             scale=-1.0 / SQK)
                        e2 = ev.tile([P, D], BF16, tag="eA2")
                        nc.scalar.activation(e2, m2, AF.Exp, scale=-1.0,
                                             bias=ln32_b[:, 0:1])
                        nc.vector.scalar_tensor_tensor(kp_all[:, c, :], ps_k, 0.0,
                                                       e2, OP.max, OP.add)

                        ps_v = ps.tile([P, D], F32, tag="psA")
                        for h in range(2):
                            hs = slice(h * 512, (h + 1) * 512)
                            for j in range(DO // 2):
                                nc.tensor.matmul(
                                    ps_v[:, hs],
                                    x_t[:, 2 * j:2 * j + 2, cs],
                                    wv_sb[:, 2 * j:2 * j + 2, hs],
                                    start=(j == 0),
                                    stop=(j == DO // 2 - 1 and not use_bv),
                                    perf_mode=DR)
                            if use_bv:
                                nc.tensor.matmul(ps_v[:, hs], ones_pair[:, 0, 0:1],
                                                 rows[0:1, 2 * D + h * 512:2 * D + (h + 1) * 512],
                                                 start=False, stop=True)
                        nc.scalar.activation(v_all[:, c, :], ps_v, AF.Copy,
                                             scale=1.0 / SW)

            # ---------------- Phase B: KV accumulation + K_sum ----------------
            with ExitStack() as ph:
                ps = ph.enter_context(tc.tile_pool(name="psB", bufs=1, space="PSUM"))
                for dcg in range(2):
                    kv_ps = [ps.tile([P, 2, 512], F32, tag=f"kvps{d}",
                                     name=f"kvps{dcg}_{d}")
                             for d in range(4)]
                    for cp in range(TOTCH // 2):
                        for di in range(4):
                            dc = dcg * 4 + di
                            for eh in range(2):
                                nc.tensor.matmul(
                                    kv_ps[di][:, eh, :],
                                    kp_all[:, 2 * cp:2 * cp + 2, dc * P:(dc + 1) * P],
                                    v_all[:, 2 * cp:2 * cp + 2, eh * 512:(eh + 1) * 512],
                                    start=(cp == 0), stop=(cp == TOTCH // 2 - 1),
                                    perf_mode=DR)
                    for di in range(4):
                        dc = dcg * 4 + di
                        nc.scalar.activation(kv_sb[:, dc, :], kv_ps[di], AF.Copy,
                                             scale=KV_SC)
            with ExitStack() as ph:
                ksp = ph.enter_context(tc.tile_pool(name="ksB", bufs=1, space="PSUM"))
                ps_ks = ksp.tile([1, D], F32, tag="ksrow")
                for cp in range(TOTCH // 2):
                    for h in range(2):
                        hs = slice(h * 512, (h + 1) * 512)
                        nc.tensor.matmul(ps_ks[0:1, hs], ones_pair[:, :, 0:1],
                                         kp_all[:, 2 * cp:2 * cp + 2, hs],
                                         start=(cp == 0), stop=(cp == TOTCH // 2 - 1),
                                         perf_mode=DR)
                nc.scalar.activation(ksrow_sb, ps_ks[0:1, :], AF.Copy)
                # transpose K_sum row -> per-partition column layout [P, DO]
                ps_ksc = ksp.tile([P, DO], F32, tag="kscol")
                for dc in range(DO):
                    nc.tensor.matmul(ps_ksc[:, dc:dc + 1],
                                     ksrow_sb[0:1, dc * P:(dc + 1) * P],
                                     ones_one[0:1, 0:1], start=True, stop=True)
                nc.scalar.activation(ksum_sb[:, :, 0], ps_ksc, AF.Copy,
                                     scale=KS_SC)

        # ---------------- Phases C+D (shared FFN-weight prefetch) ----------------
        cd = top.enter_context(ExitStack())
        wpoolD = cd.enter_context(tc.tile_pool(name="wDpre", bufs=1))
        f1_sb = wpoolD.tile([P, DO, D], BF16, tag="f1")
        nc.sync.dma_start(f1_sb, wr["f1T"])
        f2_sb = wpoolD.tile([P, DO, D], FP8, tag="f2")
        nc.sync.dma_start(f2_sb, wr["f2T"])

        # ---------------- Phase C: conv'' + local MLP + token mixer ----------------
        with ExitStack() as ph:
            wpool = ph.enter_context(tc.tile_pool(name="wC", bufs=1))
            w1_sb = wpool.tile([P, DO, D], BF16, tag="w1")
            nc.sync.dma_start(w1_sb, wr["w1T"])
            w2_sb = wpool.tile([P, DO, D], BF16, tag="w2")
            nc.sync.dma_start(w2_sb, wr["w2T"])
            io = ph.enter_context(tc.tile_pool(name="ioC", bufs=2))
            pipe = ph.enter_context(tc.tile_pool(name="pipeC", bufs=2))
            mid = ph.enter_context(tc.tile_pool(name="midC", bufs=1))
            sm = ph.enter_context(tc.tile_pool(name="smC", bufs=1))
            ps = ph.enter_context(tc.tile_pool(name="psC", bufs=2, space="PSUM"))
            pst = ph.enter_context(tc.tile_pool(name="pstC", bufs=1, space="PSUM"))

            def c_front(it):
                n0 = it * NT
                x_t = io.tile([P, DO, W], BF16, tag="xC", name=f"x_{it}")
                xq_t = io.tile([P, DO, WST], FP8, tag="xqC", name=f"xq_{it}")
                lo, hi = n0 - 2, n0 + NT + 2
                if lo < 0:
                    nc.vector.memset(x_t[:, :, 0:2], 0.0)
                    nc.sync.dma_start(x_t[:, :, 2:W], xT[:, :, 0:hi])
                    nc.vector.memset(xq_t[:, :, 0:2], 0.0)
                    nc.sync.dma_start(xq_t[:, :, 2:W], xqT[:, :, 0:hi])
                elif hi > N:
                    nc.vector.memset(x_t[:, :, W - 2:W], 0.0)
                    nc.sync.dma_start(x_t[:, :, 0:W - 2], xT[:, :, lo:N])
                    nc.vector.memset(xq_t[:, :, W - 2:W], 0.0)
                    nc.sync.dma_start(xq_t[:, :, 0:W - 2], xqT[:, :, lo:N])
                else:
                    nc.sync.dma_start(x_t, xT[:, :, lo:hi])
                    nc.sync.dma_start(xq_t[:, :, 0:W], xqT[:, :, lo:hi])

                acc = io.tile([P, DO, NT], F32, tag="accC", name=f"acc_{it}")
                # diffusion dwconv'': center tap on ACT, side taps on DVE
                for o in range(DO):
                    nc.scalar.activation(acc[:, o, :], x_t[:, o, 2:NT + 2],
                                         AF.Identity, bias=pp[:, o, CB:CB + 1],
                                         scale=pp[:, o, C1:C1 + 1])
                for o in range(DO):
                    nc.vector.scalar_tensor_tensor(acc[:, o, :], x_t[:, o, 1:NT + 1],
                                                   pp[:, o, C0:C0 + 1], acc[:, o, :],
                                                   OP.mult, OP.add)
                for o in range(DO):
                    nc.vector.scalar_tensor_tensor(acc[:, o, :], x_t[:, o, 3:NT + 3],
                                                   pp[:, o, C2:C2 + 1], acc[:, o, :],
                                                   OP.mult, OP.add)

                # local MLP first half (bf16, paired dc groups)
                h1_t = pipe.tile([P, DO, NT], BF16, tag="h1", name=f"h1_{it}")
                for dc0 in range(0, DO, 2):
                    ps_h = ps.tile([P, 2, NT], F32, tag="psC",
                                   name=f"psh1_{it}_{dc0}")
                    for h in range(2):
                        dc = dc0 + h
                        for kc in range(DO):
                            nc.tensor.matmul(ps_h[:, h, :],
                                             w1_sb[:, kc, dc * P:(dc + 1) * P],
                                             x_t[:, kc, 2:NT + 2],
                                             start=(kc == 0), stop=(kc == DO - 1))
                    for h in range(2):
                        nc.scalar.activation(h1_t[:, dc0 + h, :], ps_h[:, h, :],
                                             AF.Gelu,
                                             bias=pp[:, dc0 + h, LUB1:LUB1 + 1])

                # token mixer LN stats (fp8 sum-matmuls on xq/sq)
                sq_t = mid.tile([P, DO, WST], FP8, tag="sqC", name=f"sq_{it}")
                nc.scalar.activation(sq_t[:, :, 0:W], x_t, AF.Square)
                ps_m = pst.tile([P, W], F32, tag="psm", name=f"psm_{it}")
                stats_q(ps_m, xq_t, W)
                ps_s = pst.tile([P, W], F32, tag="pss", name=f"pss_{it}")
                stats_q(ps_s, sq_t, W)
                m_sb = sm.tile([P, 1, W], BF16, tag="msb", name=f"msb_{it}")
                nc.scalar.activation(m_sb[:, 0, :], ps_m, AF.Copy, scale=1.0 / D)
                var = sm.tile([P, 1, W], F32, tag="var", name=f"var_{it}")
                nc.scalar.activation(var[:, 0, :], ps_m, AF.Square, scale=1.0 / D)
                nc.vector.scalar_tensor_tensor(var[:, 0, :], ps_s, 1.0 / D,
                                               var[:, 0, :], OP.mult, OP.subtract)
                nc.scalar.activation(var[:, 0, :], var[:, 0, :], AF.Sqrt,
                                     bias=eps_ln[:, 0:1])
                nc.vector.reciprocal_approx_fast(out=var[:, 0, :], in_=var[:, 0, :])
                rstd = sm.tile([P, 1, W], BF16, tag="rstd", name=f"rstd_{it}")
                nc.vector.tensor_copy(rstd, var)
                u_t = mid.tile([P, DO, W], BF16, tag="tokA", name=f"u_{it}")
                nc.vector.tensor_sub(u_t, x_t, m_sb.to_broadcast((P, DO, W)))
                xm_t = mid.tile([P, DO, W], BF16, tag="tokC", name=f"xm_{it}")
                if use_tmg:
                    for o in range(DO):
                        nc.vector.scalar_tensor_tensor(
                            xm_t[:, o, :], u_t[:, o, :], pp[:, o, TMG:TMG + 1],
                            rstd[:, 0, :], OP.mult, OP.mult)
                else:
                    nc.vector.tensor_mul(xm_t, u_t, rstd.to_broadcast((P, DO, W)))
                if use_tmb:
                    for o in range(DO):
                        nc.vector.tensor_scalar_add(xm_t[:, o, :], xm_t[:, o, :],
                                                    pp[:, o, TMB:TMB + 1])
                # conv1: t_s[k] = conv1(xm)[k+1], k in [0, W-2)
                # (reuses u_t's buffer — u is dead once xm is computed)
                t_full = mid.tile([P, DO, W], BF16, tag="tokA", name=f"t_{it}")
                t_t = t_full[:, :, 0:W - 2]
                for o in range(DO):
                    nc.scalar.activation(t_t[:, o, :], xm_t[:, o, 1:W - 1],
                                         AF.Identity, bias=pp[:, o, TCB1:TCB1 + 1],
                                         scale=pp[:, o, T1:T1 + 1])
                for o in range(DO):
                    nc.vector.scalar_tensor_tensor(t_t[:, o, :], xm_t[:, o, 0:W - 2],
                                                   pp[:, o, T0:T0 + 1],
                                                   t_t[:, o, :], OP.mult, OP.add)
                for o in range(DO):
                    nc.vector.scalar_tensor_tensor(t_t[:, o, :], xm_t[:, o, 2:W],
                                                   pp[:, o, T2:T2 + 1],
                                                   t_t[:, o, :], OP.mult, OP.add)
                t2_t = pipe.tile([P, DO, W - 2], BF16, tag="t2", name=f"t2_{it}")
                nc.scalar.activation(t2_t, t_t, AF.Gelu)
                if it == 0:
                    nc.vector.memset(t2_t[:, :, 0:1], 0.0)
                if it == NTILES - 1:
                    nc.vector.memset(t2_t[:, :, W - 3:W - 2], 0.0)
                return x_t, acc, h1_t, t2_t

            def c_back(it, tiles):
                n0 = it * NT
                x_t, acc, h1_t, t2_t = tiles
                for dc0 in range(0, DO, 2):
                    ps_h = ps.tile([P, 2, NT], F32, tag="psC",
                                   name=f"psh2_{it}_{dc0}")
                    for h in range(2):
                        dc = dc0 + h
                        for kc in range(DO):
                            nc.tensor.matmul(ps_h[:, h, :],
                                             w2_sb[:, kc, dc * P:(dc + 1) * P],
                                             h1_t[:, kc, :],
                                             start=(kc == 0), stop=False)
                        for tap in range(3):
                            nc.tensor.matmul(ps_h[:, h, :], diags[:, tap, dc, :],
                                             t2_t[:, dc, tap:NT + tap],
                                             start=False, stop=(tap == 2))
                    nc.vector.tensor_add(acc[:, dc0:dc0 + 2, :],
                                         acc[:, dc0:dc0 + 2, :], ps_h)
                nc.sync.dma_start(acc_r[:, :, n0:n0 + NT], acc)

            pend = {0: c_front(0)}
            for it in range(NTILES):
                if it + 1 < NTILES:
                    pend[it + 1] = c_front(it + 1)
                c_back(it, pend.pop(it))

        # ---------------- Phase D: attention + LN1 + FFN + LN2 ----------------
        with ExitStack() as ph:
            io = ph.enter_context(tc.tile_pool(name="ioD", bufs=2))
            mid = ph.enter_context(tc.tile_pool(name="midD", bufs=2))
            sm = ph.enter_context(tc.tile_pool(name="smD", bufs=2))
            ps = ph.enter_context(tc.tile_pool(name="psD", bufs=2, space="PSUM"))
            pst = ph.enter_context(tc.tile_pool(name="pstD", bufs=1, space="PSUM"))

            def d_front_a(it):
                """loads, norm row, rep fold, numerator halves 0-3."""
                n0 = it * NT
                acc_t = io.tile([P, DO, NT], F32, tag="accD", name=f"accD_{it}")
                nc.sync.dma_start(acc_t, acc_r[:, :, n0:n0 + NT])
                ps_n = pst.tile([P, NT], F32, tag="psrep", name=f"psn_{it}")
                for j in range(DO // 2):
                    nc.tensor.matmul(ps_n[0:1, :], ksum_sb[:, 2 * j:2 * j + 2, 0:1],
                                     qp_all[:, 2 * j:2 * j + 2, n0:n0 + NT],
                                     start=(j == 0), stop=(j == DO // 2 - 1),
                                     perf_mode=DR)
                rr32 = sm.tile([1, NT], F32, tag="rr32D", name=f"rr32_{it}")
                nc.vector.reciprocal_approx_fast(out=rr32, in_=ps_n[0:1, :])
                rr = sm.tile([1, NT], BF16, tag="rrD", name=f"rr_{it}")
                nc.vector.tensor_copy(rr, rr32)
                ps_rep = pst.tile([P, NT], F32, tag="psrep", name=f"psrep_{it}")
                nc.tensor.matmul(ps_rep, ones_1p_bf[0:1, :], rr, start=True,
                                 stop=True)
                rep_sb = mid.tile([P, 1, NT], BF16, tag="repsb", name=f"rep_{it}")
                nc.scalar.activation(rep_sb[:, 0, :], ps_rep, AF.Copy,
                                     scale=REP_SC)
                qp2_t = mid.tile([P, DO, NT], FP8, tag="qp2", name=f"qp2_{it}")
                nc.vector.tensor_mul(qp2_t, qp_all[:, :, n0:n0 + NT],
                                     rep_sb.to_broadcast((P, DO, NT)))
                for ec0 in range(0, DO // 2, 2):
                    ps_u = ps.tile([P, 2, NT], F32, tag="psD",
                                   name=f"psnum_{it}_{ec0}")
                    for h in range(2):
                        ec = ec0 + h
                        for j in range(DO // 2):
                            nc.tensor.matmul(
                                ps_u[:, h, :],
                                kv_sb[:, 2 * j:2 * j + 2, ec * P:(ec + 1) * P],
                                qp2_t[:, 2 * j:2 * j + 2, :],
                                start=(j == 0), stop=(j == DO // 2 - 1),
                                perf_mode=DR)
                    nc.vector.scalar_tensor_tensor(acc_t[:, ec0:ec0 + 2, :], ps_u,
                                                   NUM_SC, acc_t[:, ec0:ec0 + 2, :],
                                                   OP.mult, OP.add)
                return qp2_t, acc_t

            def d_front_b(it, T):
                qp2_t, acc_t = T
                for ec0 in range(DO // 2, DO, 2):
                    ps_u = ps.tile([P, 2, NT], F32, tag="psD",
                                   name=f"psnum_{it}_{ec0}")
                    for h in range(2):
                        ec = ec0 + h
                        for j in range(DO // 2):
                            nc.tensor.matmul(
                                ps_u[:, h, :],
                                kv_sb[:, 2 * j:2 * j + 2, ec * P:(ec + 1) * P],
                                qp2_t[:, 2 * j:2 * j + 2, :],
                                start=(j == 0), stop=(j == DO // 2 - 1),
                                perf_mode=DR)
                    nc.vector.scalar_tensor_tensor(acc_t[:, ec0:ec0 + 2, :], ps_u,
                                                   NUM_SC, acc_t[:, ec0:ec0 + 2, :],
                                                   OP.mult, OP.add)
                return acc_t

            def d_mid(it, acc_t):
                """LN1: mean via f32r matmuls on acc, sq-stats bf16; in-place
                apply -> y1 (bf16, two halves so f1 can start early)."""
                accq = mid.tile([P, DO, NT], FP8, tag="accq", name=f"accq_{it}", bufs=1)
                nc.scalar.activation(accq, acc_t, AF.Copy)
                sqq = mid.tile([P, DO, NT], FP8, tag="sqD", name=f"sqD_{it}", bufs=1)
                nc.scalar.activation(sqq, acc_t, AF.Square)
                psst = pst.tile([P, 2, NT], F32, tag="psst", name=f"psst1_{it}")
                stats_q(psst[:, 0, :], accq, NT)
                stats_q(psst[:, 1, :], sqq, NT)
                m1_sb = sm.tile([P, 1, NT], F32, tag="m1sb", name=f"m1_{it}")
                nc.scalar.activation(m1_sb[:, 0, :], psst[:, 0, :], AF.Copy,
                                     scale=1.0 / D)
                var1 = sm.tile([P, 1, NT], F32, tag="varD", name=f"var1_{it}")
                nc.scalar.activation(var1[:, 0, :], psst[:, 0, :], AF.Square,
                                     scale=1.0 / D)
                nc.vector.scalar_tensor_tensor(var1[:, 0, :], psst[:, 1, :],
                                               1.0 / D, var1[:, 0, :],
                                               OP.mult, OP.subtract)
                nc.scalar.activation(var1[:, 0, :], var1[:, 0, :], AF.Sqrt,
                                     bias=eps_ln[:, 0:1])
                nc.vector.reciprocal_approx_fast(out=var1[:, 0, :],
                                                 in_=var1[:, 0, :])
                # u1 in place of acc (acc is dead after the stats);
                # half-chunk interleave so f1's kc-chain starts early
                y1_t = mid.tile([P, DO, NT], BF16, tag="y1", name=f"y1_{it}")
                for hh in range(0, DO, 4):
                    sl = slice(hh, hh + 4)
                    nc.vector.tensor_sub(acc_t[:, sl, :], acc_t[:, sl, :],
                                         m1_sb.to_broadcast((P, 4, NT)))
                    if use_n1g:
                        for o in range(hh, hh + 4):
                            nc.vector.scalar_tensor_tensor(
                                y1_t[:, o, :], acc_t[:, o, :],
                                pp[:, o, N1G:N1G + 1],
                                var1[:, 0, :], OP.mult, OP.mult)
                    else:
                        nc.vector.tensor_mul(y1_t[:, sl, :], acc_t[:, sl, :],
                                             var1.to_broadcast((P, 4, NT)))
                if use_n1b:
                    for o in range(DO):
                        nc.vector.tensor_scalar_add(y1_t[:, o, :], y1_t[:, o, :],
                                                    pp[:, o, N1B:N1B + 1])
                return y1_t

            def d_f1(it, y1_t, tail):
                """f1 matmul groups with the previous tile's LN2-apply (yo)
                DVE ops interleaved under the PE-heavy stretch."""
                f1h_t = mid.tile([P, DO, NT], FP8, tag="f1h", name=f"f1h_{it}")
                for gi, dc0 in enumerate(range(0, DO, 2)):
                    ps_f = ps.tile([P, 2, NT], F32, tag="psD",
                                   name=f"psf1_{it}_{dc0}")
                    for h in range(2):
                        dc = dc0 + h
                        for kc in range(DO):
                            nc.tensor.matmul(ps_f[:, h, :],
                                             f1_sb[:, kc, dc * P:(dc + 1) * P],
                                             y1_t[:, kc, :],
                                             start=(kc == 0), stop=(kc == DO - 1))
                    if gi < len(tail):
                        tail[gi]()
                    for h in range(2):
                        nc.scalar.activation(f1h_t[:, dc0 + h, :], ps_f[:, h, :],
                                             AF.Gelu,
                                             bias=pp[:, dc0 + h, FFB1:FFB1 + 1])
                for fn in tail[len(list(range(0, DO, 2))):]:
                    fn()
                return f1h_t

            def d_f2(it, y1_t, f1h_t):
                y2_t = mid.tile([P, DO, NT], BF16, tag="y2", name=f"y2_{it}")
                for dc0 in range(0, DO, 2):
                    ps_f = ps.tile([P, 2, NT], F32, tag="psD",
                                   name=f"psf2_{it}_{dc0}")
                    for h in range(2):
                        dc = dc0 + h
                        for j in range(DO // 2):
                            nc.tensor.matmul(
                                ps_f[:, h, :],
                                f2_sb[:, 2 * j:2 * j + 2, dc * P:(dc + 1) * P],
                                f1h_t[:, 2 * j:2 * j + 2, :],
                                start=(j == 0), stop=(j == DO // 2 - 1),
                                perf_mode=DR)
                    # y2 = y1 + psum/64
                    nc.vector.scalar_tensor_tensor(y2_t[:, dc0:dc0 + 2, :], ps_f,
                                                   1.0 / SW,
                                                   y1_t[:, dc0:dc0 + 2, :],
                                                   OP.mult, OP.add)
                if use_fb2:
                    for dc in range(DO):
                        nc.vector.tensor_scalar_add(y2_t[:, dc, :], y2_t[:, dc, :],
                                                    pp[:, dc, FFB2:FFB2 + 1])
                return y2_t

            def d_back_head(it, y2_t):
                sq2_t = mid.tile([P, DO, NT], BF16, tag="sq2", name=f"sq2_{it}", bufs=1)
                nc.scalar.activation(sq2_t, y2_t, AF.Square)
                psst = pst.tile([P, 2, NT], F32, tag="psst", name=f"psst2_{it}")
                stats_bf(psst[:, 0, :], y2_t, NT)
                stats_bf(psst[:, 1, :], sq2_t, NT)
                m2_sb = sm.tile([P, 1, NT], F32, tag="m2sb", name=f"m2_{it}")
                nc.scalar.activation(m2_sb[:, 0, :], psst[:, 0, :], AF.Copy)
                var2 = sm.tile([P, 1, NT], F32, tag="var2D", name=f"var2_{it}")
                nc.scalar.activation(var2[:, 0, :], psst[:, 0, :], AF.Square)
                nc.vector.tensor_sub(var2[:, 0, :], psst[:, 1, :], var2[:, 0, :])
                nc.scalar.activation(var2[:, 0, :], var2[:, 0, :], AF.Sqrt,
                                     bias=eps_ln[:, 0:1])
                nc.vector.reciprocal_approx_fast(out=var2[:, 0, :],
                                                 in_=var2[:, 0, :])
                return y2_t, m2_sb, var2

            def d_back_tail(it, state):
                """Returns closures: yo half-ops + output DMA, to be issued
                under the next tile's f1 matmul groups."""
                y2_t, m2_sb, var2 = state
                n0 = it * NT
                yo_t = mid.tile([P, DO, NT], F32, tag="yo", name=f"yo_{it}",
                                bufs=1)
                fns = []
                if use_n2g:
                    def sub_all():
                        nc.vector.tensor_sub(yo_t, y2_t,
                                             m2_sb.to_broadcast((P, DO, NT)))
                        for o in range(DO):
                            nc.vector.scalar_tensor_tensor(
                                yo_t[:, o, :], yo_t[:, o, :],
                                pp[:, o, N2G:N2G + 1], var2[:, 0, :],
                                OP.mult, OP.mult)
                        if use_n2b:
                            for o in range(DO):
                                nc.vector.tensor_scalar_add(
                                    yo_t[:, o, :], yo_t[:, o, :],
                                    pp[:, o, N2B:N2B + 1])
                        nc.sync.dma_start(yT[:, :, n0:n0 + NT], yo_t)
                    return [sub_all]
                for hh in range(0, DO, 4):
                    def half(hh=hh):
                        sl = slice(hh, hh + 4)
                        nc.vector.tensor_sub(yo_t[:, sl, :], y2_t[:, sl, :],
                                             m2_sb.to_broadcast((P, 4, NT)))
                        nc.vector.tensor_mul(yo_t[:, sl, :], yo_t[:, sl, :],
                                             var2.to_broadcast((P, 4, NT)))
                        nc.sync.dma_start(yT[:, hh:hh + 4, n0:n0 + NT],
                                          yo_t[:, sl, :])
                    fns.append(half)
                return fns

            Tnext = d_front_a(0)
            acc_next = d_front_b(0, Tnext)
            tail = []
            for it in range(NTILES):
                acc_cur = acc_next
                if it + 1 < NTILES:
                    Tnext = d_front_a(it + 1)
                y1_cur = d_mid(it, acc_cur)
                if it + 1 < NTILES:
                    acc_next = d_front_b(it + 1, Tnext)
                f1h_cur = d_f1(it, y1_cur, tail)
                y2_cur = d_f2(it, y1_cur, f1h_cur)
                state = d_back_head(it, y2_cur)
                tail = d_back_tail(it, state)
            for fn in tail:
                fn()
    nc.compile()
    return nc


def make_in_maps(inputs, n_cores=8):
    """Host-side preprocessing: fold constants, transpose, cast, shard."""
    x = np.asarray(inputs["x"], np.float32)
    B, N, D_ = x.shape
    dt = float(np.asarray(inputs["delta_t"]))

    def g(k):
        return np.asarray(inputs[k], np.float32)

    diff_w, diff_b = g("diff_w"), g("diff_b")
    tm_w1, tm_cb1 = g("tm_w1"), g("tm_cb1")
    tm_w2, tm_cb2 = g("tm_w2"), g("tm_cb2")

    pp = np.zeros((P, DO, NPARAM), np.float32)

    def put(i, v):
        pp[:, :, i] = v.reshape(DO, P).T

    put(C0, dt * diff_w[:, 0, 0])
    put(C1, dt * diff_w[:, 0, 1] + (1.0 - dt))
    put(C2, dt * diff_w[:, 0, 2])
    put(CB, dt * diff_b + g("lu_b2") + tm_cb2)
    put(T0, tm_w1[:, 0, 0])
    put(T1, tm_w1[:, 0, 1])
    put(T2, tm_w1[:, 0, 2])
    put(TCB1, tm_cb1)
    put(U0, tm_w2[:, 0, 0])
    put(U1, tm_w2[:, 0, 1])
    put(U2, tm_w2[:, 0, 2])
    put(TMG, g("tm_g"))
    put(TMB, g("tm_beta"))
    put(N1G, g("n1_g"))
    put(N1B, g("n1_b"))
    put(N2G, g("n2_g"))
    put(N2B, g("n2_b"))
    put(LUB1, g("lu_b1"))
    put(FFB1, g("ff_b1"))
    put(FFB2, g("ff_b2"))

    diags = np.zeros((P, 3, DO, P), np.float32)
    idx = np.arange(P)
    for tap in range(3):
        for dc in range(DO):
            diags[idx, tap, dc, idx] = tm_w2[dc * P + idx, 0, tap]
    diags = diags.astype(BF16_NP)

    rows = np.zeros((1, 3 * D), np.float32)
    rows[0, 0:D] = SQK * g("bq")
    rows[0, D:2 * D] = SQK * g("bk")
    rows[0, 2 * D:3 * D] = SW * g("bv")
    rows = np.clip(rows, -240, 240).astype(FP8_NP)

    wt = {}
    for name, key, sc in (("wqT", "wq", SQK), ("wkT", "wk", SQK),
                          ("wvT", "wv", SW), ("f2T", "ff_w2", SW)):
        wt[name] = np.clip(
            np.ascontiguousarray(g(key).T) * sc, -240, 240).astype(FP8_NP)
    for name, key in (("w1T", "lu_w1"), ("w2T", "lu_w2"), ("f1T", "ff_w1")):
        wt[name] = np.ascontiguousarray(g(key).T).astype(BF16_NP)

    xt_f = np.ascontiguousarray(x.transpose(0, 2, 1))
    xT = xt_f.astype(BF16_NP)
    xqT = np.clip(xt_f, -240, 240).astype(FP8_NP)

    ones = np.ones((D_,), np.float32)
    flags = dict(
        use_bq=bool(np.any(g("bq"))),
        use_bk=bool(np.any(g("bk"))),
        use_bv=bool(np.any(g("bv"))),
        use_tmb=bool(np.any(g("tm_beta"))),
        use_n1b=bool(np.any(g("n1_b"))),
        use_n2b=bool(np.any(g("n2_b"))),
        use_fb2=bool(np.any(g("ff_b2"))),
        use_tmg=bool(np.any(g("tm_g") != ones)),
        use_n1g=bool(np.any(g("n1_g") != ones)),
        use_n2g=bool(np.any(g("n2_g") != ones)),
    )

    shared = {**wt, "pp": pp, "rows": rows, "diags": diags}
    in_maps = [{**shared, "x_T": xT[b], "xq_T": xqT[b]} for b in range(n_cores)]
    return in_maps, flags, (B, N)


_NC_CACHE = {}


def kernel(**inputs):
    in_maps, flags, (B, N) = make_in_maps(inputs)
    key = (N, tuple(sorted(flags.items())))
    if key not in _NC_CACHE:
        _NC_CACHE[key] = build_nc(N=N, NT=512, **flags)
    nc = _NC_CACHE[key]
    res = run_bass_kernel_spmd(nc, in_maps, list(range(B)))
    y = np.stack([res.results[b]["y_T"] for b in range(B)])
    return np.ascontiguousarray(y.transpose(0, 2, 1)).astype(np.float32)


# revision 21
# speedup vs baseline: 1.1421x; 1.0245x over previous
"""Trainium2 Bass kernel for nn_DiffuRNNLayer (B=8, N=2048, D=1024).

Sharding: data-parallel over batch — one batch element per NeuronCore (8 cores).
v3: mixed fp8-e4m3 DoubleRow / bf16 matmuls, chosen per-path by error budget:
  fp8 DR: wq/wk/wv (phase A), KV (B), f2+f1h (D), numerator+norm (D),
          LN-mean matmuls for tokenmixer-LN/LN1/LN2.
  bf16:   lu_w1/lu_w2 (local MLP, residual-critical), ff_w1, conv diag taps.
Scale bookkeeping: wq,wk x32; wv,f2 x64; qp_all=32*Qp, kp_all=32*Kp, v_all=V,
kv_sb=KV/2, ksum_sb=Ksum/16, rep=2^16/norm, numerator PSUM=2^20*attn.
Kp/V/Qp are SBUF-resident fp8 (no HBM spill); KV+K_sum run back-to-back from
SBUF.  DVE ops are merged across channel chunks wherever gains are 1
(broadcast [P,1,*] operands), and PSUM consumers are paired into [P,2,NT]
tiles to halve instruction count.
"""

import math
import numpy as np
import ml_dtypes
from contextlib import ExitStack

import concourse.bass as bass
import concourse.bacc as bacc
import concourse.tile as tile
import concourse.mybir as mybir
from concourse.bass_utils import run_bass_kernel_spmd

F32 = mybir.dt.float32
BF16 = mybir.dt.bfloat16
FP8 = mybir.dt.float8e4
AF = mybir.ActivationFunctionType
OP = mybir.AluOpType
DR = mybir.MatmulPerfMode.DoubleRow
BF16_NP = ml_dtypes.bfloat16
FP8_NP = ml_dtypes.float8_e4m3

P = 128
D = 1024
DO = D // P  # 8 chunks of the channel dim

SQK = 32.0       # weight scale for wq/wk (keeps 32*Qp < 240 in fp8)
SW = 64.0        # weight scale for wv/f2
LN_SQK = math.log(SQK)
KS_SC = 2.0 ** -9    # ksum fp8 = (sum of 32*Kp) * 2^-9 = Ksum/16
KV_SC = 1.0 / SW     # kv fp8 = (sum 32Kp*V)/64 = KV/2
REP_SC = 2.0 ** 17   # rep = rr * 2^17 = 2^16/norm  (rr = 1/(2*norm))
NUM_SC = 2.0 ** -20  # ps_u = 2^20 * attn  ->  acc += ps_u * 2^-20

# pp param-plane indices (per-partition params, laid out [128, DO, NP])
(C0, C1, C2, CB, T0, T1, T2, TCB1, U0, U1, U2,
 TMG, TMB, N1G, N1B, N2G, N2B, LUB1, FFB1, FFB2) = range(20)
NPARAM = 20



def build_nc(N=2048, NT=512, use_bq=False, use_bk=False, use_bv=False,
             use_tmb=False, use_n1b=False, use_n2b=False, use_fb2=False,
             use_tmg=False, use_n1g=False, use_n2g=False, debug=False):
    NTILES = N // NT
    NTA = 512              # phase-A tile size (independent of C/D tiling)
    NTILES_A = N // NTA
    NCH = NTA // P         # 128-token chunks per phase-A tile
    TOTCH = N // P
    W = NT + 4             # phase-C tile width with +-2 halo
    WST = (W + 15) // 16 * 16  # fp8 stats tile width (16B-aligned)
    assert N % NT == 0 and NT % P == 0 and N % NTA == 0

    nc = bacc.Bacc(None, target_bir_lowering=False, debug=debug)

    xT_d = nc.dram_tensor("x_T", [D, N], BF16, kind="ExternalInput")
    xqT_d = nc.dram_tensor("xq_T", [D, N], FP8, kind="ExternalInput")
    w_d = {}
    for name, dt_ in (("wqT", FP8), ("wkT", FP8), ("wvT", FP8),
                      ("w1T", BF16), ("w2T", BF16), ("f1T", BF16),
                      ("f2T", FP8)):
        w_d[name] = nc.dram_tensor(name, [D, D], dt_, kind="ExternalInput")
    pp_d = nc.dram_tensor("pp", [P, DO, NPARAM], F32, kind="ExternalInput")
    diags_d = nc.dram_tensor("diags", [P, 3, DO, P], BF16, kind="ExternalInput")
    rows_d = nc.dram_tensor("rows", [1, 3 * D], FP8, kind="ExternalInput")
    yT_d = nc.dram_tensor("y_T", [D, N], F32, kind="ExternalOutput")

    acc_sp = nc.dram_tensor("acc_sp", [D, N], F32)

    xT = xT_d.rearrange("(o p) n -> p o n", p=P)
    xqT = xqT_d.rearrange("(o p) n -> p o n", p=P)
    wr = {k: v.rearrange("(o p) n -> p o n", p=P) for k, v in w_d.items()}
    acc_r = acc_sp.rearrange("(o p) n -> p o n", p=P)
    yT = yT_d.rearrange("(o p) n -> p o n", p=P)

    with tile.TileContext(nc) as tc, ExitStack() as top:
        persist = top.enter_context(tc.tile_pool(name="persist", bufs=1))
        pp = persist.tile([P, DO, NPARAM], F32)
        nc.sync.dma_start(pp, pp_d[:])
        rows = ones_row = None
        if use_bq or use_bk or use_bv:
            rows = persist.tile([1, 3 * D], FP8)
            nc.sync.dma_start(rows, rows_d[:])
            ones_row = persist.tile([1, max(NT, 512)], FP8)
            nc.vector.memset(ones_row, 1.0)
        ones_1p_bf = persist.tile([1, P], BF16)
        nc.vector.memset(ones_1p_bf, 1.0)
        ones_pair = persist.tile([P, 2, 16], FP8)
        nc.vector.memset(ones_pair, 1.0)
        ones_q = persist.tile([P, 2, P], FP8)
        nc.vector.memset(ones_q, 1.0)
        ones_one = persist.tile([1, 1], BF16)
        nc.vector.memset(ones_one, 1.0)
        ksrow_sb = persist.tile([1, D], BF16)
        onesD_bf = persist.tile([P, P], BF16)
        nc.vector.memset(onesD_bf, 1.0 / D)
        onesD_f32 = persist.tile([P, P], F32)
        nc.vector.memset(onesD_f32, 1.0 / D)
        eps_ln = persist.tile([P, 1], F32)
        nc.vector.memset(eps_ln, 1e-5)
        ln32_b = persist.tile([P, 1], F32)
        nc.vector.memset(ln32_b, LN_SQK)
        kv_sb = persist.tile([P, DO, D], FP8)
        ksum_sb = persist.tile([P, DO, 16], FP8)
        qp_all = persist.tile([P, DO, N], FP8)
        diags = persist.tile([P, 3, DO, P], BF16)
        nc.sync.dma_start(diags, diags_d[:])

        def stats_q(psum, rhs3, width):
            """Sum over channels (lhs=1.0 fp8 DoubleRow); caller scales by 1/D."""
            for c0 in range(0, width, 512):
                cw = min(512, width - c0)
                for j in range(DO // 2):
                    nc.tensor.matmul(psum[:, c0:c0 + cw], ones_q,
                                     rhs3[:, 2 * j:2 * j + 2, c0:c0 + cw],
                                     start=(j == 0), stop=(j == DO // 2 - 1),
                                     perf_mode=DR)

        def stats_bf(psum, rhs3, width):
            """Mean over channels (lhs=1/D bf16), replicated across partitions."""
            for c0 in range(0, width, 512):
                cw = min(512, width - c0)
                for kc in range(DO):
                    nc.tensor.matmul(psum[:, c0:c0 + cw], onesD_bf,
                                     rhs3[:, kc, c0:c0 + cw],
                                     start=(kc == 0), stop=(kc == DO - 1))

        F32R = mybir.dt.float32r

        # ---------------- Phases A+B scope ----------------
        with ExitStack() as phab:
            kvpool = phab.enter_context(tc.tile_pool(name="kvres", bufs=1))
            kp_all = kvpool.tile([P, TOTCH, D], FP8, tag="kp")
            v_all = kvpool.tile([P, TOTCH, D], FP8, tag="v")

            # ---------------- Phase A: QKV ----------------
            with ExitStack() as ph:
                wpool = ph.enter_context(tc.tile_pool(name="wA", bufs=1))
                wq_sb = wpool.tile([P, DO, D], FP8, tag="wq")
                nc.sync.dma_start(wq_sb, wr["wqT"])
                wk_sb = wpool.tile([P, DO, D], FP8, tag="wk")
                nc.sync.dma_start(wk_sb, wr["wkT"])
                wv_sb = wpool.tile([P, DO, D], FP8, tag="wv")
                nc.sync.dma_start(wv_sb, wr["wvT"])
                io = ph.enter_context(tc.tile_pool(name="ioA", bufs=2))
                ev = ph.enter_context(tc.tile_pool(name="evA", bufs=3))
                ps = ph.enter_context(tc.tile_pool(name="psA", bufs=3, space="PSUM"))

                for it in range(NTILES_A):
                    n0 = it * NTA
                    x_t = io.tile([P, DO, NTA], FP8, tag="xA")
                    nc.sync.dma_start(x_t, xqT[:, :, n0:n0 + NTA])

                    # ---- Q: layout B, out [dout-pair, n]; psum [P, 2*NT] flat
                    for dc0 in range(0, DO, 2):
                        ps_q = ps.tile([P, 2 * NTA], F32, tag="psA")
                        for h in range(2):
                            dc = dc0 + h
                            for j in range(DO // 2):
                                nc.tensor.matmul(
                                    ps_q[:, h * NTA:(h + 1) * NTA],
                                    wq_sb[:, 2 * j:2 * j + 2, dc * P:(dc + 1) * P],
                                    x_t[:, 2 * j:2 * j + 2, :],
                                    start=(j == 0),
                                    stop=(j == DO // 2 - 1 and not use_bq),
                                    perf_mode=DR)
                            if use_bq:
                                nc.tensor.matmul(ps_q[:, h * NTA:(h + 1) * NTA],
                                                 rows[0:1, dc * P:(dc + 1) * P],
                                                 ones_row[0:1, 0:NTA],
                                                 start=False, stop=True)
                        m_t = ev.tile([P, 2 * NTA], BF16, tag="mA")
                        # relu(-q) = -min(q,0) on ACT (frees a DVE psum read)
                        nc.scalar.activation(m_t, ps_q, AF.Relu,
                                             scale=-1.0 / SQK)
                        e_t = ev.tile([P, 2 * NTA], BF16, tag="eA")
                        # 32*exp(min(q,0)) = exp(-relu(-q) + ln32)
                        nc.scalar.activation(e_t, m_t, AF.Exp, scale=-1.0,
                                             bias=ln32_b[:, 0:1])
                        # qp = max(32q,0) + 32*exp(min(q,0)) = 32*(elu(q)+1)
                        nc.vector.scalar_tensor_tensor(
                            qp_all[:, dc0:dc0 + 2, n0:n0 + NTA], ps_q, 0.0, e_t,
                            OP.max, OP.add)

                    # ---- K, V: layout A, out [token-chunk, 1024 douts] ----
                    for ch in range(NCH):
                        c = it * NCH + ch
                        cs = slice(ch * P, (ch + 1) * P)
                        ps_k = ps.tile([P, D], F32, tag="psA")
                        for h in range(2):
                            hs = slice(h * 512, (h + 1) * 512)
                            for j in range(DO // 2):
                                nc.tensor.matmul(
                                    ps_k[:, hs],
                                    x_t[:, 2 * j:2 * j + 2, cs],
                                    wk_sb[:, 2 * j:2 * j + 2, hs],
                                    start=(j == 0),
                                    stop=(j == DO // 2 - 1 and not use_bk),
                                    perf_mode=DR)
                            if use_bk:
                                nc.tensor.matmul(ps_k[:, hs], ones_pair[:, 0, 0:1],
                                                 rows[0:1, D + h * 512:D + (h + 1) * 512],
                                                 start=False, stop=True)
                        m2 = ev.tile([P, D], BF16, tag="mA2")
                        nc.scalar.activation(m2, ps_k, AF.Relu,
                                             scale=-1.0 / SQK)
                        e2 = ev.tile([P, D], BF16, tag="eA2")
                        nc.scalar.activation(e2, m2, AF.Exp, scale=-1.0,
                                             bias=ln32_b[:, 0:1])
                        nc.vector.scalar_tensor_tensor(kp_all[:, c, :], ps_k, 0.0,
                                                       e2, OP.max, OP.add)

                        ps_v = ps.tile([P, D], F32, tag="psA")
                        for h in range(2):
                            hs = slice(h * 512, (h + 1) * 512)
                            for j in range(DO // 2):
                                nc.tensor.matmul(
                                    ps_v[:, hs],
                                    x_t[:, 2 * j:2 * j + 2, cs],
                                    wv_sb[:, 2 * j:2 * j + 2, hs],
                                    start=(j == 0),
                                    stop=(j == DO // 2 - 1 and not use_bv),
                                    perf_mode=DR)
                            if use_bv:
                                nc.tensor.matmul(ps_v[:, hs], ones_pair[:, 0, 0:1],
                                                 rows[0:1, 2 * D + h * 512:2 * D + (h + 1) * 512],
                                                 start=False, stop=True)
                        nc.scalar.activation(v_all[:, c, :], ps_v, AF.Copy,
                                             scale=1.0 / SW)

            # ---------------- Phase B: KV accumulation + K_sum ----------------
            with ExitStack() as ph:
                ps = ph.enter_context(tc.tile_pool(name="psB", bufs=1, space="PSUM"))
                for dcg in range(2):
                    kv_ps = [ps.tile([P, 2, 512], F32, tag=f"kvps{d}",
                                     name=f"kvps{dcg}_{d}")
                             for d in range(4)]
                    for cp in range(TOTCH // 2):
                        for di in range(4):
                            dc = dcg * 4 + di
                            for eh in range(2):
                                nc.tensor.matmul(
                                    kv_ps[di][:, eh, :],
                                    kp_all[:, 2 * cp:2 * cp + 2, dc * P:(dc + 1) * P],
                                    v_all[:, 2 * cp:2 * cp + 2, eh * 512:(eh + 1) * 512],
                                    start=(cp == 0), stop=(cp == TOTCH // 2 - 1),
                                    perf_mode=DR)
                    for di in range(4):
                        dc = dcg * 4 + di
                        nc.scalar.activation(kv_sb[:, dc, :], kv_ps[di], AF.Copy,
                                             scale=KV_SC)
            with ExitStack() as ph:
                ksp = ph.enter_context(tc.tile_pool(name="ksB", bufs=1, space="PSUM"))
                ps_ks = ksp.tile([1, D], F32, tag="ksrow")
                for cp in range(TOTCH // 2):
                    for h in range(2):
                        hs = slice(h * 512, (h + 1) * 512)
                        nc.tensor.matmul(ps_ks[0:1, hs], ones_pair[:, :, 0:1],
                                         kp_all[:, 2 * cp:2 * cp + 2, hs],
                                         start=(cp == 0), stop=(cp == TOTCH // 2 - 1),
                                         perf_mode=DR)
                nc.scalar.activation(ksrow_sb, ps_ks[0:1, :], AF.Copy)
                # transpose K_sum row -> per-partition column layout [P, DO]
                ps_ksc = ksp.tile([P, DO], F32, tag="kscol")
                for dc in range(DO):
                    nc.tensor.matmul(ps_ksc[:, dc:dc + 1],
                                     ksrow_sb[0:1, dc * P:(dc + 1) * P],
                                     ones_one[0:1, 0:1], start=True, stop=True)
                nc.scalar.activation(ksum_sb[:, :, 0], ps_ksc, AF.Copy,
                                     scale=KS_SC)

        # ---------------- Phases C+D (shared FFN-weight prefetch) ----------------
        cd = top.enter_context(ExitStack())
        wpoolD = cd.enter_context(tc.tile_pool(name="wDpre", bufs=1))
        f1_sb = wpoolD.tile([P, DO, D], BF16, tag="f1")
        nc.sync.dma_start(f1_sb, wr["f1T"])
        f2_sb = wpoolD.tile([P, DO, D], FP8, tag="f2")
        nc.sync.dma_start(f2_sb, wr["f2T"])

        # ---------------- Phase C: conv'' + local MLP + token mixer ----------------
        with ExitStack() as ph:
            wpool = ph.enter_context(tc.tile_pool(name="wC", bufs=1))
            w1_sb = wpool.tile([P, DO, D], BF16, tag="w1")
            nc.sync.dma_start(w1_sb, wr["w1T"])
            w2_sb = wpool.tile([P, DO, D], BF16, tag="w2")
            nc.sync.dma_start(w2_sb, wr["w2T"])
            io = ph.enter_context(tc.tile_pool(name="ioC", bufs=2))
            pipe = ph.enter_context(tc.tile_pool(name="pipeC", bufs=2))
            mid = ph.enter_context(tc.tile_pool(name="midC", bufs=1))
            sm = ph.enter_context(tc.tile_pool(name="smC", bufs=1))
            ps = ph.enter_context(tc.tile_pool(name="psC", bufs=2, space="PSUM"))
            pst = ph.enter_context(tc.tile_pool(name="pstC", bufs=1, space="PSUM"))

            def c_front(it):
                n0 = it * NT
                x_t = io.tile([P, DO, W], BF16, tag="xC", name=f"x_{it}")
                xq_t = io.tile([P, DO, WST], FP8, tag="xqC", name=f"xq_{it}")
                lo, hi = n0 - 2, n0 + NT + 2
                if lo < 0:
                    nc.vector.memset(x_t[:, :, 0:2], 0.0)
                    nc.sync.dma_start(x_t[:, :, 2:W], xT[:, :, 0:hi])
                    nc.vector.memset(xq_t[:, :, 0:2], 0.0)
                    nc.sync.dma_start(xq_t[:, :, 2:W], xqT[:, :, 0:hi])
                elif hi > N:
                    nc.vector.memset(x_t[:, :, W - 2:W], 0.0)
                    nc.sync.dma_start(x_t[:, :, 0:W - 2], xT[:, :, lo:N])
                    nc.vector.memset(xq_t[:, :, W - 2:W], 0.0)
                    nc.sync.dma_start(xq_t[:, :, 0:W - 2], xqT[:, :, lo:N])
                else:
                    nc.sync.dma_start(x_t, xT[:, :, lo:hi])
                    nc.sync.dma_start(xq_t[:, :, 0:W], xqT[:, :, lo:hi])

                acc = io.tile([P, DO, NT], F32, tag="accC", name=f"acc_{it}")
                # diffusion dwconv'': center tap on ACT, side taps on DVE
                for o in range(DO):
                    nc.scalar.activation(acc[:, o, :], x_t[:, o, 2:NT + 2],
                                         AF.Identity, bias=pp[:, o, CB:CB + 1],
                                         scale=pp[:, o, C1:C1 + 1])
                for o in range(DO):
                    nc.vector.scalar_tensor_tensor(acc[:, o, :], x_t[:, o, 1:NT + 1],
                                                   pp[:, o, C0:C0 + 1], acc[:, o, :],
                                                   OP.mult, OP.add)
                for o in range(DO):
                    nc.vector.scalar_tensor_tensor(acc[:, o, :], x_t[:, o, 3:NT + 3],
                                                   pp[:, o, C2:C2 + 1], acc[:, o, :],
                                                   OP.mult, OP.add)

                # local MLP first half (bf16, paired dc groups)
                h1_t = pipe.tile([P, DO, NT], BF16, tag="h1", name=f"h1_{it}")
                for dc0 in range(0, DO, 2):
                    ps_h = ps.tile([P, 2, NT], F32, tag="psC",
                                   name=f"psh1_{it}_{dc0}")
                    for h in range(2):
                        dc = dc0 + h
                        for kc in range(DO):
                            nc.tensor.matmul(ps_h[:, h, :],
                                             w1_sb[:, kc, dc * P:(dc + 1) * P],
                                             x_t[:, kc, 2:NT + 2],
                                             start=(kc == 0), stop=(kc == DO - 1))
                    for h in range(2):
                        nc.scalar.activation(h1_t[:, dc0 + h, :], ps_h[:, h, :],
                                             AF.Gelu,
                                             bias=pp[:, dc0 + h, LUB1:LUB1 + 1])

                # token mixer LN stats (fp8 sum-matmuls on xq/sq)
                sq_t = mid.tile([P, DO, WST], FP8, tag="sqC", name=f"sq_{it}")
                nc.scalar.activation(sq_t[:, :, 0:W], x_t, AF.Square)
                ps_m = pst.tile([P, W], F32, tag="psm", name=f"psm_{it}")
                stats_q(ps_m, xq_t, W)
                ps_s = pst.tile([P, W], F32, tag="pss", name=f"pss_{it}")
                stats_q(ps_s, sq_t, W)
                m_sb = sm.tile([P, 1, W], BF16, tag="msb", name=f"msb_{it}")
                nc.scalar.activation(m_sb[:, 0, :], ps_m, AF.Copy, scale=1.0 / D)
                var = sm.tile([P, 1, W], F32, tag="var", name=f"var_{it}")
                nc.scalar.activation(var[:, 0, :], ps_m, AF.Square, scale=1.0 / D)
                nc.vector.scalar_tensor_tensor(var[:, 0, :], ps_s, 1.0 / D,
                                               var[:, 0, :], OP.mult, OP.subtract)
                nc.scalar.activation(var[:, 0, :], var[:, 0, :], AF.Sqrt,
                                     bias=eps_ln[:, 0:1])
                nc.vector.reciprocal_approx_fast(out=var[:, 0, :], in_=var[:, 0, :])
                rstd = sm.tile([P, 1, W], BF16, tag="rstd", name=f"rstd_{it}")
                nc.vector.tensor_copy(rstd, var)
                u_t = mid.tile([P, DO, W], BF16, tag="tokA", name=f"u_{it}")
                nc.vector.tensor_sub(u_t, x_t, m_sb.to_broadcast((P, DO, W)))
                xm_t = mid.tile([P, DO, W], BF16, tag="tokC", name=f"xm_{it}")
                if use_tmg:
                    for o in range(DO):
                        nc.vector.scalar_tensor_tensor(
                            xm_t[:, o, :], u_t[:, o, :], pp[:, o, TMG:TMG + 1],
                            rstd[:, 0, :], OP.mult, OP.mult)
                else:
                    nc.vector.tensor_mul(xm_t, u_t, rstd.to_broadcast((P, DO, W)))
                if use_tmb:
                    for o in range(DO):
                        nc.vector.tensor_scalar_add(xm_t[:, o, :], xm_t[:, o, :],
                                                    pp[:, o, TMB:TMB + 1])
                # conv1: t_s[k] = conv1(xm)[k+1], k in [0, W-2)
                # (reuses u_t's buffer — u is dead once xm is computed)
                t_full = mid.tile([P, DO, W], BF16, tag="tokA", name=f"t_{it}")
                t_t = t_full[:, :, 0:W - 2]
                for o in range(DO):
                    nc.scalar.activation(t_t[:, o, :], xm_t[:, o, 1:W - 1],
                                         AF.Identity, bias=pp[:, o, TCB1:TCB1 + 1],
                                         scale=pp[:, o, T1:T1 + 1])
                for o in range(DO):
                    nc.vector.scalar_tensor_tensor(t_t[:, o, :], xm_t[:, o, 0:W - 2],
                                                   pp[:, o, T0:T0 + 1],
                                                   t_t[:, o, :], OP.mult, OP.add)
                for o in range(DO):
                    nc.vector.scalar_tensor_tensor(t_t[:, o, :], xm_t[:, o, 2:W],
                                                   pp[:, o, T2:T2 + 1],
                                                   t_t[:, o, :], OP.mult, OP.add)
                t2_t = pipe.tile([P, DO, W - 2], BF16, tag="t2", name=f"t2_{it}")
                nc.scalar.activation(t2_t, t_t, AF.Gelu)
                if it == 0:
                    nc.vector.memset(t2_t[:, :, 0:1], 0.0)
                if it == NTILES - 1:
                    nc.vector.memset(t2_t[:, :, W - 3:W - 2], 0.0)
                return x_t, acc, h1_t, t2_t

            def c_back(it, tiles):
                n0 = it * NT
                x_t, acc, h1_t, t2_t = tiles
                for dc0 in range(0, DO, 2):
                    ps_h = ps.tile([P, 2, NT], F32, tag="psC",
                                   name=f"psh2_{it}_{dc0}")
                    for h in range(2):
                        dc = dc0 + h
                        for kc in range(DO):
                            nc.tensor.matmul(ps_h[:, h, :],
                                             w2_sb[:, kc, dc * P:(dc + 1) * P],
                                             h1_t[:, kc, :],
                                             start=(kc == 0), stop=False)
                        for tap in range(3):
                            nc.tensor.matmul(ps_h[:, h, :], diags[:, tap, dc, :],
                                             t2_t[:, dc, tap:NT + tap],
                                             start=False, stop=(tap == 2))
                    nc.vector.tensor_add(acc[:, dc0:dc0 + 2, :],
                                         acc[:, dc0:dc0 + 2, :], ps_h)
                nc.sync.dma_start(acc_r[:, :, n0:n0 + NT], acc)

            pend = {0: c_front(0)}
            for it in range(NTILES):
                if it + 1 < NTILES:
                    pend[it + 1] = c_front(it + 1)
                c_back(it, pend.pop(it))

        # ---------------- Phase D: attention + LN1 + FFN + LN2 ----------------
        with ExitStack() as ph:
            io = ph.enter_context(tc.tile_pool(name="ioD", bufs=2))
            mid = ph.enter_context(tc.tile_pool(name="midD", bufs=2))
            sm = ph.enter_context(tc.tile_pool(name="smD", bufs=2))
            ps = ph.enter_context(tc.tile_pool(name="psD", bufs=2, space="PSUM"))
            pst = ph.enter_context(tc.tile_pool(name="pstD", bufs=1, space="PSUM"))

            def d_front_a(it):
                """loads, norm row, rep fold, numerator halves 0-3."""
                n0 = it * NT
                acc_t = io.tile([P, DO, NT], F32, tag="accD", name=f"accD_{it}")
                nc.sync.dma_start(acc_t, acc_r[:, :, n0:n0 + NT])
                ps_n = pst.tile([P, NT], F32, tag="psrep", name=f"psn_{it}")
                for j in range(DO // 2):
                    nc.tensor.matmul(ps_n[0:1, :], ksum_sb[:, 2 * j:2 * j + 2, 0:1],
                                     qp_all[:, 2 * j:2 * j + 2, n0:n0 + NT],
                                     start=(j == 0), stop=(j == DO // 2 - 1),
                                     perf_mode=DR)
                rr32 = sm.tile([1, NT], F32, tag="rr32D", name=f"rr32_{it}")
                nc.vector.reciprocal_approx_fast(out=rr32, in_=ps_n[0:1, :])
                rr = sm.tile([1, NT], BF16, tag="rrD", name=f"rr_{it}")
                nc.vector.tensor_copy(rr, rr32)
                ps_rep = pst.tile([P, NT], F32, tag="psrep", name=f"psrep_{it}")
                nc.tensor.matmul(ps_rep, ones_1p_bf[0:1, :], rr, start=True,
                                 stop=True)
                rep_sb = mid.tile([P, 1, NT], BF16, tag="repsb", name=f"rep_{it}")
                nc.scalar.activation(rep_sb[:, 0, :], ps_rep, AF.Copy,
                                     scale=REP_SC)
                qp2_t = mid.tile([P, DO, NT], FP8, tag="qp2", name=f"qp2_{it}")
                nc.vector.tensor_mul(qp2_t, qp_all[:, :, n0:n0 + NT],
                                     rep_sb.to_broadcast((P, DO, NT)))
                for ec0 in range(0, DO // 2, 2):
                    ps_u = ps.tile([P, 2, NT], F32, tag="psD",
                                   name=f"psnum_{it}_{ec0}")
                    for h in range(2):
                        ec = ec0 + h
                        for j in range(DO // 2):
                            nc.tensor.matmul(
                                ps_u[:, h, :],
                                kv_sb[:, 2 * j:2 * j + 2, ec * P:(ec + 1) * P],
                                qp2_t[:, 2 * j:2 * j + 2, :],
                                start=(j == 0), stop=(j == DO // 2 - 1),
                                perf_mode=DR)
                    nc.vector.scalar_tensor_tensor(acc_t[:, ec0:ec0 + 2, :], ps_u,
                                                   NUM_SC, acc_t[:, ec0:ec0 + 2, :],
                                                   OP.mult, OP.add)
                return qp2_t, acc_t

            def d_front_b(it, T):
                qp2_t, acc_t = T
                for ec0 in range(DO // 2, DO, 2):
                    ps_u = ps.tile([P, 2, NT], F32, tag="psD",
                                   name=f"psnum_{it}_{ec0}")
                    for h in range(2):
                        ec = ec0 + h
                        for j in range(DO // 2):
                            nc.tensor.matmul(
                                ps_u[:, h, :],
                                kv_sb[:, 2 * j:2 * j + 2, ec * P:(ec + 1) * P],
                                qp2_t[:, 2 * j:2 * j + 2, :],
                                start=(j == 0), stop=(j == DO // 2 - 1),
                                perf_mode=DR)
                    nc.vector.scalar_tensor_tensor(acc_t[:, ec0:ec0 + 2, :], ps_u,
                                                   NUM_SC, acc_t[:, ec0:ec0 + 2, :],
                                                   OP.mult, OP.add)
                return acc_t

            def d_stats(it, acc_t):
                """LN1 stats (fp8 sum-matmuls) + m1/rstd chain."""
                accq = mid.tile([P, DO, NT], FP8, tag="accq", name=f"accq_{it}", bufs=1)
                nc.scalar.activation(accq, acc_t, AF.Copy)
                sqq = mid.tile([P, DO, NT], FP8, tag="sqD", name=f"sqD_{it}", bufs=1)
                nc.scalar.activation(sqq, acc_t, AF.Square)
                psst = pst.tile([P, 2, NT], F32, tag="psst", name=f"psst1_{it}")
                stats_q(psst[:, 0, :], accq, NT)
                stats_q(psst[:, 1, :], sqq, NT)
                m1_sb = sm.tile([P, 1, NT], F32, tag="m1sb", name=f"m1_{it}")
                nc.scalar.activation(m1_sb[:, 0, :], psst[:, 0, :], AF.Copy,
                                     scale=1.0 / D)
                var1 = sm.tile([P, 1, NT], F32, tag="varD", name=f"var1_{it}")
                nc.scalar.activation(var1[:, 0, :], psst[:, 0, :], AF.Square,
                                     scale=1.0 / D)
                nc.vector.scalar_tensor_tensor(var1[:, 0, :], psst[:, 1, :],
                                               1.0 / D, var1[:, 0, :],
                                               OP.mult, OP.subtract)
                nc.scalar.activation(var1[:, 0, :], var1[:, 0, :], AF.Sqrt,
                                     bias=eps_ln[:, 0:1])
                nc.vector.reciprocal_approx_fast(out=var1[:, 0, :],
                                                 in_=var1[:, 0, :])
                return m1_sb, var1

            def d_apply(it, acc_t, m1_sb, var1):
                """u1 (in place of acc) + y1, half-chunk interleaved."""
                y1_t = mid.tile([P, DO, NT], BF16, tag="y1", name=f"y1_{it}")
                for hh in range(0, DO, 4):
                    sl = slice(hh, hh + 4)
                    nc.vector.tensor_sub(acc_t[:, sl, :], acc_t[:, sl, :],
                                         m1_sb.to_broadcast((P, 4, NT)))
                    if use_n1g:
                        for o in range(hh, hh + 4):
                            nc.vector.scalar_tensor_tensor(
                                y1_t[:, o, :], acc_t[:, o, :],
                                pp[:, o, N1G:N1G + 1],
                                var1[:, 0, :], OP.mult, OP.mult)
                    else:
                        nc.vector.tensor_mul(y1_t[:, sl, :], acc_t[:, sl, :],
                                             var1.to_broadcast((P, 4, NT)))
                if use_n1b:
                    for o in range(DO):
                        nc.vector.tensor_scalar_add(y1_t[:, o, :], y1_t[:, o, :],
                                                    pp[:, o, N1B:N1B + 1])
                return y1_t

            def d_f1(it, y1_t, tail):
                """f1 matmul groups with the previous tile's LN2-apply (yo)
                DVE ops interleaved under the PE-heavy stretch."""
                f1h_t = mid.tile([P, DO, NT], FP8, tag="f1h", name=f"f1h_{it}")
                for gi, dc0 in enumerate(range(0, DO, 2)):
                    ps_f = ps.tile([P, 2, NT], F32, tag="psD",
                                   name=f"psf1_{it}_{dc0}")
                    for h in range(2):
                        dc = dc0 + h
                        for kc in range(DO):
                            nc.tensor.matmul(ps_f[:, h, :],
                                             f1_sb[:, kc, dc * P:(dc + 1) * P],
                                             y1_t[:, kc, :],
                                             start=(kc == 0), stop=(kc == DO - 1))
                    if gi < len(tail):
                        tail[gi]()
                    for h in range(2):
                        nc.scalar.activation(f1h_t[:, dc0 + h, :], ps_f[:, h, :],
                                             AF.Gelu,
                                             bias=pp[:, dc0 + h, FFB1:FFB1 + 1])
                for fn in tail[len(list(range(0, DO, 2))):]:
                    fn()
                return f1h_t

            def d_f2(it, y1_t, f1h_t):
                y2_t = mid.tile([P, DO, NT], BF16, tag="y2", name=f"y2_{it}")
                for dc0 in range(0, DO, 2):
                    ps_f = ps.tile([P, 2, NT], F32, tag="psD",
                                   name=f"psf2_{it}_{dc0}")
                    for h in range(2):
                        dc = dc0 + h
                        for j in range(DO // 2):
                            nc.tensor.matmul(
                                ps_f[:, h, :],
                                f2_sb[:, 2 * j:2 * j + 2, dc * P:(dc + 1) * P],
                                f1h_t[:, 2 * j:2 * j + 2, :],
                                start=(j == 0), stop=(j == DO // 2 - 1),
                                perf_mode=DR)
                    # y2 = y1 + psum/64
                    nc.vector.scalar_tensor_tensor(y2_t[:, dc0:dc0 + 2, :], ps_f,
                                                   1.0 / SW,
                                                   y1_t[:, dc0:dc0 + 2, :],
                                                   OP.mult, OP.add)
                if use_fb2:
                    for dc in range(DO):
                        nc.vector.tensor_scalar_add(y2_t[:, dc, :], y2_t[:, dc, :],
                                                    pp[:, dc, FFB2:FFB2 + 1])
                sq2_t = mid.tile([P, DO, NT], BF16, tag="sq2", name=f"sq2_{it}",
                                 bufs=2)
                nc.scalar.activation(sq2_t, y2_t, AF.Square)
                return y2_t, sq2_t

            def d_back_head(it, y2_t, sq2_t):
                psst = pst.tile([P, 2, NT], F32, tag="psst", name=f"psst2_{it}")
                stats_bf(psst[:, 0, :], y2_t, NT)
                stats_bf(psst[:, 1, :], sq2_t, NT)
                m2_sb = sm.tile([P, 1, NT], F32, tag="m2sb", name=f"m2_{it}")
                nc.scalar.activation(m2_sb[:, 0, :], psst[:, 0, :], AF.Copy)
                var2 = sm.tile([P, 1, NT], F32, tag="var2D", name=f"var2_{it}")
                nc.scalar.activation(var2[:, 0, :], psst[:, 0, :], AF.Square)
                nc.vector.tensor_sub(var2[:, 0, :], psst[:, 1, :], var2[:, 0, :])
                nc.scalar.activation(var2[:, 0, :], var2[:, 0, :], AF.Sqrt,
                                     bias=eps_ln[:, 0:1])
                nc.vector.reciprocal_approx_fast(out=var2[:, 0, :],
                                                 in_=var2[:, 0, :])
                return y2_t, m2_sb, var2

            def d_back_tail(it, state):
                """Returns closures: yo half-ops + output DMA, to be issued
                under the next tile's f1 matmul groups."""
                y2_t, m2_sb, var2 = state
                n0 = it * NT
                yo_t = mid.tile([P, DO, NT], F32, tag="yo", name=f"yo_{it}",
                                bufs=1)
                fns = []
                if use_n2g:
                    def sub_all():
                        nc.vector.tensor_sub(yo_t, y2_t,
                                             m2_sb.to_broadcast((P, DO, NT)))
                        for o in range(DO):
                            nc.vector.scalar_tensor_tensor(
                                yo_t[:, o, :], yo_t[:, o, :],
                                pp[:, o, N2G:N2G + 1], var2[:, 0, :],
                                OP.mult, OP.mult)
                        if use_n2b:
                            for o in range(DO):
                                nc.vector.tensor_scalar_add(
                                    yo_t[:, o, :], yo_t[:, o, :],
                                    pp[:, o, N2B:N2B + 1])
                        nc.sync.dma_start(yT[:, :, n0:n0 + NT], yo_t)
                    return [sub_all]
                for hh in range(0, DO, 4):
                    def half(hh=hh):
                        sl = slice(hh, hh + 4)
                        nc.vector.tensor_sub(yo_t[:, sl, :], y2_t[:, sl, :],
                                             m2_sb.to_broadcast((P, 4, NT)))
                        nc.vector.tensor_mul(yo_t[:, sl, :], yo_t[:, sl, :],
                                             var2.to_broadcast((P, 4, NT)))
                        nc.sync.dma_start(yT[:, hh:hh + 4, n0:n0 + NT],
                                          yo_t[:, sl, :])
                    fns.append(half)
                return fns

            holder = {}
            Tcur = d_front_a(0)
            acc_hold = d_front_b(0, Tcur)
            st_hold = d_stats(0, acc_hold)
            prev = None  # (it-1) -> (y2, sq2)
            for it in range(NTILES):
                acc_cur, st_cur = acc_hold, st_hold
                y1_cur = d_apply(it, acc_cur, *st_cur)
                tail = []
                if prev is not None:
                    state = d_back_head(it - 1, *prev)
                    tail = d_back_tail(it - 1, state)
                pre = []
                if it + 1 < NTILES:
                    def fa(it=it):
                        holder['T'] = d_front_a(it + 1)
                    pre = [fa]
                f1h_cur = d_f1(it, y1_cur, pre + tail)
                prev = d_f2(it, y1_cur, f1h_cur)
                if it + 1 < NTILES:
                    acc_hold = d_front_b(it + 1, holder['T'])
                    st_hold = d_stats(it + 1, acc_hold)
            state = d_back_head(NTILES - 1, *prev)
            for fn in d_back_tail(NTILES - 1, state):
                fn()
    nc.compile()
    return nc


def make_in_maps(inputs, n_cores=8):
    """Host-side preprocessing: fold constants, transpose, cast, shard."""
    x = np.asarray(inputs["x"], np.float32)
    B, N, D_ = x.shape
    dt = float(np.asarray(inputs["delta_t"]))

    def g(k):
        return np.asarray(inputs[k], np.float32)

    diff_w, diff_b = g("diff_w"), g("diff_b")
    tm_w1, tm_cb1 = g("tm_w1"), g("tm_cb1")
    tm_w2, tm_cb2 = g("tm_w2"), g("tm_cb2")

    pp = np.zeros((P, DO, NPARAM), np.float32)

    def put(i, v):
        pp[:, :, i] = v.reshape(DO, P).T

    put(C0, dt * diff_w[:, 0, 0])
    put(C1, dt * diff_w[:, 0, 1] + (1.0 - dt))
    put(C2, dt * diff_w[:, 0, 2])
    put(CB, dt * diff_b + g("lu_b2") + tm_cb2)
    put(T0, tm_w1[:, 0, 0])
    put(T1, tm_w1[:, 0, 1])
    put(T2, tm_w1[:, 0, 2])
    put(TCB1, tm_cb1)
    put(U0, tm_w2[:, 0, 0])
    put(U1, tm_w2[:, 0, 1])
    put(U2, tm_w2[:, 0, 2])
    put(TMG, g("tm_g"))
    put(TMB, g("tm_beta"))
    put(N1G, g("n1_g"))
    put(N1B, g("n1_b"))
    put(N2G, g("n2_g"))
    put(N2B, g("n2_b"))
    put(LUB1, g("lu_b1"))
    put(FFB1, g("ff_b1"))
    put(FFB2, g("ff_b2"))

    diags = np.zeros((P, 3, DO, P), np.float32)
    idx = np.arange(P)
    for tap in range(3):
        for dc in range(DO):
            diags[idx, tap, dc, idx] = tm_w2[dc * P + idx, 0, tap]
    diags = diags.astype(BF16_NP)

    rows = np.zeros((1, 3 * D), np.float32)
    rows[0, 0:D] = SQK * g("bq")
    rows[0, D:2 * D] = SQK * g("bk")
    rows[0, 2 * D:3 * D] = SW * g("bv")
    rows = np.clip(rows, -240, 240).astype(FP8_NP)

    wt = {}
    for name, key, sc in (("wqT", "wq", SQK), ("wkT", "wk", SQK),
                          ("wvT", "wv", SW), ("f2T", "ff_w2", SW)):
        wt[name] = np.clip(
            np.ascontiguousarray(g(key).T) * sc, -240, 240).astype(FP8_NP)
    for name, key in (("w1T", "lu_w1"), ("w2T", "lu_w2"), ("f1T", "ff_w1")):
        wt[name] = np.ascontiguousarray(g(key).T).astype(BF16_NP)

    xt_f = np.ascontiguousarray(x.transpose(0, 2, 1))
    xT = xt_f.astype(BF16_NP)
    xqT = np.clip(xt_f, -240, 240).astype(FP8_NP)

    ones = np.ones((D_,), np.float32)
    flags = dict(
        use_bq=bool(np.any(g("bq"))),
        use_bk=bool(np.any(g("bk"))),
        use_bv=bool(np.any(g("bv"))),
        use_tmb=bool(np.any(g("tm_beta"))),
        use_n1b=bool(np.any(g("n1_b"))),
        use_n2b=bool(np.any(g("n2_b"))),
        use_fb2=bool(np.any(g("ff_b2"))),
        use_tmg=bool(np.any(g("tm_g") != ones)),
        use_n1g=bool(np.any(g("n1_g") != ones)),
        use_n2g=bool(np.any(g("n2_g") != ones)),
    )

    shared = {**wt, "pp": pp, "rows": rows, "diags": diags}
    in_maps = [{**shared, "x_T": xT[b], "xq_T": xqT[b]} for b in range(n_cores)]
    return in_maps, flags, (B, N)


_NC_CACHE = {}


def kernel(**inputs):
    in_maps, flags, (B, N) = make_in_maps(inputs)
    key = (N, tuple(sorted(flags.items())))
    if key not in _NC_CACHE:
        _NC_CACHE[key] = build_nc(N=N, NT=512, **flags)
    nc = _NC_CACHE[key]
    res = run_bass_kernel_spmd(nc, in_maps, list(range(B)))
    y = np.stack([res.results[b]["y_T"] for b in range(B)])
    return np.ascontiguousarray(y.transpose(0, 2, 1)).astype(np.float32)
